# revision 41
# baseline (speedup 1.0000x reference)
"""CohortAwareBlock Trainium2 kernel.

Data-parallel over batch B=8 across 8 NeuronCores (one sample per core).
Cohort routing (gather of cohort_q_w by per-sample cohort id) happens on the
host while building each core's weight tensors; the device kernel is a plain
attention block.

Numerics:
  - QK-gen runs as fp8-e4m3 DoubleRow matmuls (weights pre-scaled x32 to
    dodge fp8 subnormals; the inverse scale is folded into the exp scale).
  - q/k are stored as fp8 in a DoubleRow-interleaved layout ([32, 2, N] per
    head, 4 heads stacked across 128 partitions) so the scores matmul also
    runs fp8-DR: 2x fewer PE cycles than fp16 scores.
  - exp is split across the ACT engine (exact table exp, fp16 out) and the
    DVE (Schraudolph bit-trick: y = int16(A*s + B) bit-read as fp16, ~1.8%
    rms sawtooth error) so neither engine is the bottleneck.
  - v / attn weights / projection stay fp16.

Per-core structure:
  q4k4 [128, 8, 2, N] fp8  (4 q-head groups + 4 k-head groups, DR layout)
  v_aug [keys, h, 65] fp16 (col 64 = 1.0 so the flipped AV emits the
                            softmax denominator per q-partition)
  per (q-quarter, head pair):
    scores -> 2-bank PSUM [128, 4, 256] via fp8-DR -> exp (ACT fp16 or DVE
    Schraudolph, routed by backlog) ->
    flipped attn@v: av_ps [128, 2, 2, 65]; col 64 = den ->
    batched DVE reciprocal [128,2,2,1] + broadcast mult -> nm fp16 ->
    DMA-XBAR transpose -> nmT [d, q] -> proj (fp16) + bias

PE emission is software-pipelined with virtual engine clocks (pe/act/dve);
QK/V generation and the projection fill PE slack under the exp window, and
dummy warmup matmuls keep the PE p-state ramped before the first real work.
"""

import numpy as np

import concourse.bass as bass
import concourse.bacc as bacc
import concourse.mybir as mybir
import concourse.tile as tile
from concourse.bass_utils import run_bass_kernel_spmd

P = 128
N = 1024            # sequence length
D = 1024            # model dim
H = 16              # heads
HD = 64             # head dim
NQ = 4              # q-quarters (256 q each)
QW = N // NQ        # 256
SCALE = HD ** -0.5
NCORES = 8

WS = 32.0           # fp8 pre-scale on w_q/w_k (and so on q/k values)
EXP_SCALE = SCALE / (WS * WS)

# Schraudolph fp16-bitcast exp on DVE: y_bits = int16(s * A + B); bits read
# as fp16 give exp(s*EXP_SCALE) with ~1.8% rms sawtooth error.
LOG2E = 1.4426950408889634
SCHR_A = EXP_SCALE * LOG2E * 1024.0
SCHR_B = 15301.0
SCHR_MAX = 30        # max exp groups routed to DVE (of 128); error budget cap

F32 = mybir.dt.float32
FP16 = mybir.dt.float16
BF16 = mybir.dt.bfloat16
FP8 = mybir.dt.float8e4
I16 = mybir.dt.int16
DR = mybir.MatmulPerfMode.DoubleRow
EXP = mybir.ActivationFunctionType.Exp
MUL = mybir.AluOpType.mult
ADD = mybir.AluOpType.add


def build_nc():
    nc = bacc.Bacc(
        "TRN2",
        target_bir_lowering=False,
        debug=False,
        num_devices=NCORES,
    )

    # ---- external I/O (per-core shards, host-prepped layouts) ----
    # DoubleRow-interleaved d-dim: d = (t2*2 + dj)*128 + p
    xdr = nc.dram_tensor("xdr", [P, 4, 2, N], FP8, kind="ExternalInput")
    # wqk[p, g, j, t2, dj, ec]: g = 4-head group (0..3 q, 4..7 k); j = d-half
    # of the head (e_local = j*32 + i); ec = hh*32 + i -> head 4*(g%4)+hh.
    wqk = nc.dram_tensor("wqk", [P, 8, 2, 4, 2, P], FP8, kind="ExternalInput")
    bqk = nc.dram_tensor("bqk", [P, 8, 2], F32, kind="ExternalInput")
    xt = nc.dram_tensor("xt", [P, 8, N], FP16, kind="ExternalInput")   # x^T
    wv = nc.dram_tensor("wv", [P, 8, D], FP16, kind="ExternalInput")
    bv = nc.dram_tensor("bv", [D], BF16, kind="ExternalInput")
    wp = nc.dram_tensor("wp", [P, 8, D], FP16, kind="ExternalInput")
    bp = nc.dram_tensor("bp", [D], BF16, kind="ExternalInput")
    ident = nc.dram_tensor("ident", [P, P], FP16, kind="ExternalInput")
    bqk8 = nc.dram_tensor("bqk8", [1, 8, 2, P], FP8, kind="ExternalInput")
    out = nc.dram_tensor("out", [N, D], F32, kind="ExternalOutput")

    with tile.TileContext(nc) as tc:
        kernel_body(tc, xdr, wqk, bqk, xt, wv, bv, wp, bp, ident, bqk8, out)
    nc.compile()
    return nc


EMIT_LOG = []


def kernel_body(tc, xdr, wqk, bqk, xt, wv, bv, wp, bp, ident, bqk8, out):
    nc = tc.nc
    from contextlib import ExitStack

    with ExitStack() as ctx:
        ctx.enter_context(
            nc.allow_low_precision(reason="fp16/fp8 matmul inputs by design")
        )
        res = ctx.enter_context(tc.tile_pool(name="res", bufs=1))
        shared = ctx.enter_context(tc.tile_pool(name="shared", bufs=1))
        gen_ps = ctx.enter_context(tc.tile_pool(name="gen_ps", bufs=2, space="PSUM"))
        sc_ps = ctx.enter_context(tc.tile_pool(name="sc_ps", bufs=2, space="PSUM"))
        av_ps = ctx.enter_context(tc.tile_pool(name="av_ps", bufs=2, space="PSUM"))
        exp_pool = ctx.enter_context(tc.tile_pool(name="exp_pool", bufs=36))
        rc_pool = ctx.enter_context(tc.tile_pool(name="rc_pool", bufs=4))
        nm_pool = ctx.enter_context(tc.tile_pool(name="nm_pool", bufs=4))
        oev_pool = ctx.enter_context(tc.tile_pool(name="oev_pool", bufs=2))

        # ---- resident tiles ----
        warm = res.tile([1, 513], FP16)
        nc.gpsimd.memset(warm[:], 1.0)

        xdr_sb = res.tile([P, 4, 2, N], FP8)
        wqk_sb = shared.tile([P, 8, 2, 4, 2, P], FP8, name="wqk_sb")
        bqk_sb = res.tile([P, 8, 2], F32)
        # q/k in scores-DR layout: group g (0..3 q, 4..7 k), partition
        # (hh*32+i), j, token -> value of head 4*(g%4)+hh, d = j*32+i
        q4k4 = res.tile([P, 8, 2, N], FP8)
        xt_sb = res.tile([P, 8, N], FP16)
        wv_sb = res.tile([P, 8, D], FP16)
        bv_rep = res.tile([P, D], BF16)
        wp_holder = []   # allocated from `shared` after QK-gen is emitted
        bp_rep = res.tile([P, D], BF16)

        # v_aug[p, nt, h, :]: cols 0:64 = v for head h at key chunk nt,
        # col 64 = 1.0 (flipped attn@v then emits the softmax denominator
        # in output column 64, one value per q-partition)
        v_aug = res.tile([P, 8, H, HD + 1], FP16)
        nc.gpsimd.memset(v_aug[:, :, :, HD : HD + 1], 1.0)

        # transposed normalized att, packed for proj: [d-part, qc, co, q]
        nmT = res.tile([P, 8, 8, P], FP16)
        ident_sb = res.tile([P, P], FP16)
        bqk8_sb = res.tile([1, 8, 2, P], FP8)
        ones8 = res.tile([1, 512], FP8)
        nc.gpsimd.memset(ones8[:], 1.0)

        # ---- input DMAs (sync queue, need-order; wp follows in the
        # filler queue, reusing wqk's SBUF once QK-gen is done) ----
        nc.sync.dma_start(xdr_sb[:], xdr[:])
        for g in (0, 4):
            for j in range(2):
                nc.sync.dma_start(wqk_sb[:, g, j], wqk[:, g, j])
        nc.sync.dma_start(bqk8_sb[:], bqk8[:])
        nc.sync.dma_start(bqk_sb[:], bqk[:])
        for g in (1, 5):
            for j in range(2):
                nc.sync.dma_start(wqk_sb[:, g, j], wqk[:, g, j])
        for dc in range(8):
            nc.sync.dma_start(wv_sb[:, dc], wv[:, dc])
        nc.sync.dma_start(bv_rep[:], bv[None, :].to_broadcast([P, D]))
        for dc in range(8):
            nc.sync.dma_start(xt_sb[:, dc], xt[:, dc])
        for g in (2, 6, 3, 7):
            for j in range(2):
                nc.sync.dma_start(wqk_sb[:, g, j], wqk[:, g, j])
        nc.sync.dma_start(bp_rep[:], bp[None, :].to_broadcast([P, D]))
        nc.sync.dma_start(ident_sb[:], ident[:])

        # ---------------- emission helpers ----------------
        def warmup():
            # keep the PE p-state ramped while input DMAs land
            ps = gen_ps.tile([P, 512], F32, tag="gps", name="gps")
            nc.tensor.matmul(
                ps[0:1, :],
                lhsT=warm[:, 512:513],
                rhs=warm[:, 0:512],
                start=True,
                stop=True,
            )

        def qk_unit(g, j, ch, act_evac=False):
            # one QK-gen psum group: 4 fp8-DR matmuls + biased fp8 evac into
            # the scores-DR layout (GPSIMD cannot read PSUM, so evac on DVE;
            # the first units evac via ACT-Copy instead -- ACT idles during
            # startup -- with the bias folded in as a ones-row matmul)
            ps = gen_ps.tile([P, 512], F32, tag="gps", name="gps")
            for t2 in range(4):
                nc.tensor.matmul(
                    ps[:],
                    lhsT=wqk_sb[:, g, j, t2],
                    rhs=xdr_sb[:, t2, :, ch * 512 : (ch + 1) * 512],
                    start=(t2 == 0),
                    stop=(t2 == 3) and not act_evac,
                    perf_mode=DR,
                )
            if act_evac:
                nc.tensor.matmul(
                    ps[:],
                    lhsT=bqk8_sb[:, g, j],
                    rhs=ones8[:],
                    start=False,
                    stop=True,
                )
                nc.scalar.activation(
                    q4k4[:, g, j, ch * 512 : (ch + 1) * 512],
                    ps[:],
                    mybir.ActivationFunctionType.Copy,
                )
            else:
                nc.vector.tensor_scalar_add(
                    q4k4[:, g, j, ch * 512 : (ch + 1) * 512],
                    ps[:],
                    bqk_sb[:, g, j : j + 1],
                )

        def v_unit(eq, nt):
            # v[keys nt-chunk, 256 cols (4 heads) of quarter eq]: one psum
            # group + evac; quarter granularity staggers the AV deadlines
            ps = gen_ps.tile([P, 256], F32, tag="gps", name="gps")
            for dc in range(8):
                nc.tensor.matmul(
                    ps[:],
                    lhsT=xt_sb[:, dc, nt * P : (nt + 1) * P],
                    rhs=wv_sb[:, dc, eq * 256 : (eq + 1) * 256],
                    start=(dc == 0),
                    stop=(dc == 7),
                )
            nc.vector.tensor_add(
                v_aug[:, nt, eq * 4 : (eq + 1) * 4, 0:HD],
                ps[:].rearrange("p (h d) -> p h d", d=HD),
                bv_rep[:, eq * 256 : (eq + 1) * 256].rearrange(
                    "p (h d) -> p h d", d=HD
                ),
            )

        def sc_group(qh, co, g, hh, use_dve):
            # one kt-group of scores (fp8-DR) + its batched exp (ACT exact
            # or DVE Schraudolph); returns the exp tile
            h = 2 * co + hh
            grp = h // 4
            r = 32 * (h % 4)
            q0 = qh * QW
            ps = sc_ps.tile([P, 4, QW], F32, tag="scps", name="scps")
            for ki in range(4):
                kt = g * 4 + ki
                nc.tensor.matmul(
                    ps[:, ki],
                    lhsT=q4k4[r : r + 32, 4 + grp, :, kt * P : (kt + 1) * P],
                    rhs=q4k4[r : r + 32, grp, :, q0 : q0 + QW],
                    start=True,
                    stop=True,
                    perf_mode=DR,
                    tile_position=(r, 0),
                )
            ex = exp_pool.tile([P, 4, QW], FP16, tag="exp", name="exp")
            if use_dve:
                nc.vector.tensor_scalar(
                    ex[:].bitcast(I16),
                    ps[:],
                    SCHR_A,
                    SCHR_B,
                    op0=MUL,
                    op1=ADD,
                )
            else:
                nc.scalar.activation(ex[:], ps[:], EXP, scale=EXP_SCALE)
            return ex

        def av_halves(qh, co, exps):
            # flipped attn@v for one head pair, split per head; the batched
            # norm runs after the second half; the nm -> nmT transpose is a
            # separate unit (PE-array transpose + DVE evac) emitted later
            hold = []

            def half(hh):
                h = 2 * co + hh
                if hh == 0:
                    t = av_ps.tile([P, 392], F32, tag="avps", name="avps")
                    hold.append(t)
                ps = hold[0][:, 0:260].rearrange(
                    "p (a b c) -> p a b c", a=2, b=2
                )
                for qs in range(2):
                    for kt in range(8):
                        nc.tensor.matmul(
                            ps[:, qs, hh],
                            lhsT=exps[(hh, kt // 4)][:, kt % 4,
                                                     qs * P : (qs + 1) * P],
                            rhs=v_aug[:, kt, h, :],
                            start=(kt == 0),
                            stop=(kt == 7),
                        )
                if hh == 1:
                    rc = rc_pool.tile([P, 2, 2, 1], F32, tag="rc", name="rc")
                    nc.vector.reciprocal(rc[:], ps[:, :, :, HD : HD + 1])
                    nm = nm_pool.tile([P, 2, 2, HD], FP16, tag="nm", name="nm")
                    nc.vector.tensor_tensor(
                        nm[:],
                        ps[:, :, :, 0:HD],
                        rc[:].broadcast_to([P, 2, 2, HD]),
                        op=MUL,
                    )
                    hold.append(nm)

            def tp():
                # PE-array transpose of nm into proj layout + DVE evac;
                # keeps the nmT chain off the slow DMA queues
                t, nm = hold
                tpv = t[:, 264:392].bitcast(FP16).rearrange(
                    "p (a q) -> p a q", a=2
                )
                for qs in range(2):
                    nc.tensor.transpose(
                        tpv[:, qs], nm[:, qs], ident_sb[:]
                    )
                nc.vector.tensor_copy(
                    nmT[:, qh * 2 : qh * 2 + 2, co, :], tpv[:]
                )

            return (lambda: half(0)), (lambda: half(1)), tp

        def pj_halves(qh, nt, fh):
            # one projection output group split into two PE units
            qc = qh * 2 + nt
            n0 = qc * P
            hold = []

            def a():
                ps = gen_ps.tile([P, 512], F32, tag="gps", name="gps")
                hold.append(ps)
                for co in range(4):
                    nc.tensor.matmul(
                        ps[:],
                        lhsT=nmT[:, qc, co, :],
                        rhs=wp_holder[0][:, co, fh * 512 : (fh + 1) * 512],
                        start=(co == 0),
                        stop=False,
                    )

            def b():
                ps = hold[0]
                for co in range(4, 8):
                    nc.tensor.matmul(
                        ps[:],
                        lhsT=nmT[:, qc, co, :],
                        rhs=wp_holder[0][:, co, fh * 512 : (fh + 1) * 512],
                        start=False,
                        stop=(co == 7),
                    )
                ev = oev_pool.tile([P, 512], F32, tag="oev", name="oev")
                nc.vector.tensor_add(
                    ev[:], ps[:], bp_rep[:, fh * 512 : (fh + 1) * 512]
                )
                nc.gpsimd.dma_start(
                    out[n0 : n0 + P, fh * 512 : (fh + 1) * 512], ev[:]
                )

            return a, b

        # ---------------- schedule ----------------
        # Priority scheduler with virtual engine clocks (pe/act/dve busy-until
        # estimates under the cost model). The exp stream (ACT + DVE
        # Schraudolph, routed by backlog) is the critical path; score groups
        # are emitted as fast as the sc_ps double-buffer allows. AV pairs and
        # QK/V/proj units fill PE slack one unit at a time from per-kind
        # queues, so a unit needed soon never forces a burst-drain of
        # unrelated work (which would starve the exp engines).
        from collections import deque

        C_SC = 220.0          # score group PE (4 fp8-DR matmuls)
        C_EXP_ACT = 1110.0
        C_EXP_DVE = 1280.0
        C_AVH = 440.0         # AV half PE
        C_NORM = 800.0        # batched recip+mult DVE
        C_QK = 430.0          # QK unit PE
        C_QK_EV = 750.0       # QK evac DVE
        C_VQ = 858.0          # V quarter-unit PE
        C_VQ_EV = 485.0       # V quarter evac DVE
        C_PJ = 1704.0         # proj unit PE (a+b)
        C_PJ_EV = 705.0       # proj evac DVE
        C_TP = 115.0          # nm transpose PE (2 PE-array transposes)
        C_TP_EV = 320.0       # nmT evac DVE
        TARGET_BL = 3600.0

        for _ in range(12):
            warmup()

        # QK units: first 4 unblock (qh0, co0, g0) scores; the rest are
        # popped on demand (per-unit) or as slack fillers
        qk_first = [(0, 0, 0), (0, 1, 0), (4, 0, 0), (4, 1, 0)]
        qk_q = deque()
        for (g, j, ch, est) in [(4, 0, 1, 5200.0), (4, 1, 1, 5600.0),
                                (0, 0, 1, 6000.0), (0, 1, 1, 6400.0)]:
            qk_q.append((est, (g, j, ch)))
        for grp, est in ((1, 7800.0), (2, 20700.0), (3, 22100.0)):
            for (g, ch) in ((grp, 0), (4 + grp, 0), (4 + grp, 1), (grp, 1)):
                for j in range(2):
                    qk_q.append((est, (g, j, ch)))
        v_q = deque()
        for eq in range(4):
            for nt in range(8):
                v_q.append((19300.0, (eq, nt)))
        pj_q = deque()

        pe_t = 6300.0
        act_t = 0.0
        dve_t = 0.0
        qk_done = set(qk_first)
        for u in qk_first:
            qk_unit(*u, act_evac=True)
            pe_t += C_QK + 213.0
            act_t = max(act_t, pe_t + 100.0) + 700.0

        av_pend = deque()
        exp_fin = []
        schr_n = 0
        v_pops = [0, 0, 0, 0]
        counts = [0] * NQ
        wp_loaded = [False]

        def wall():
            return max(pe_t, act_t - 2.0 * C_EXP_ACT, dve_t - 2.0 * C_EXP_DVE)

        def emit_qk(u):
            nonlocal pe_t, dve_t
            est = None
            for (e, uu) in qk_q:
                if uu == u:
                    est = e
                    break
            qk_q.remove((est, u))
            qk_unit(*u)
            qk_done.add(u)
            pe_t = max(pe_t, est) + C_QK
            dve_t = max(dve_t, pe_t + 100.0) + C_QK_EV
            if not qk_q and not wp_loaded[0]:
                wp_loaded[0] = True
                wp_load()

        def wp_load():
            wp_holder.append(shared.tile([P, 8, D], FP16, name="wp_sb"))
            for co in range(8):
                nc.sync.dma_start(wp_holder[0][:, co], wp[:, co])

        def emit_v():
            nonlocal pe_t, dve_t
            est, (eq, nt) = v_q.popleft()
            v_unit(eq, nt)
            v_pops[eq] += 1
            pe_t = max(pe_t, est) + C_VQ
            dve_t = max(dve_t, pe_t + 100.0) + C_VQ_EV

        def emit_pj():
            nonlocal pe_t, dve_t
            est, (qh0, nt, fh) = pj_q.popleft()
            a, b = pj_halves(qh0, nt, fh)
            a()
            b()
            pe_t = max(pe_t, est) + C_PJ
            dve_t = max(dve_t, pe_t + 100.0) + C_PJ_EV

        def pick_filler():
            # one slack unit, earliest-est first; False if nothing eligible
            cands = []
            if qk_q:
                cands.append((qk_q[0][0], 0))
            if v_q:
                cands.append((v_q[0][0], 1))
            if pj_q:
                cands.append((pj_q[0][0], 2))
            cands = [c for c in cands if c[0] <= wall() + 400.0]
            if not cands:
                return False
            cands.sort()
            kind = cands[0][1]
            if kind == 0:
                emit_qk(qk_q[0][1])
            elif kind == 1:
                emit_v()
            else:
                emit_pj()
            return True

        pending_tp = []

        def flush_tp():
            nonlocal pe_t, dve_t
            while pending_tp:
                qh0, co0, tp = pending_tp.pop(0)
                tp()
                pe_t += C_TP
                dve_t = max(dve_t, pe_t + 100.0) + C_TP_EV
                counts[qh0] += 1
                if counts[qh0] == 8:
                    est_pj = max(wall(), dve_t) + 1600.0
                    for nt in range(2):
                        for fh in range(2):
                            pj_q.append((est_pj, (qh0, nt, fh)))

        def av_ready():
            if not av_pend:
                return False
            qh0, co0, _ = av_pend[0]
            return v_pops[co0 // 2] >= 8

        def av_emit():
            nonlocal pe_t, dve_t
            flush_tp()
            qh0, co0, exps0 = av_pend.popleft()
            a, b, tp = av_halves(qh0, co0, exps0)
            a()
            pe_t += C_AVH
            b()
            pe_t += C_AVH
            dve_t = max(dve_t, pe_t + 100.0) + C_NORM
            pending_tp.append((qh0, co0, tp))

        FILL_TOTAL = 24 * C_QK + 32 * C_VQ + 32 * C_PJ / 2.0
        fill_pe = [0.0]

        def quota_fill(limit):
            n = 0
            while (
                n < limit
                and fill_pe[0] < (it + 1) * (FILL_TOTAL / 32.0)
            ):
                before = (len(qk_q), len(v_q), len(pj_q))
                if not pick_filler():
                    break
                after = (len(qk_q), len(v_q), len(pj_q))
                if before[0] != after[0]:
                    fill_pe[0] += C_QK
                elif before[1] != after[1]:
                    fill_pe[0] += C_VQ
                else:
                    fill_pe[0] += C_PJ
                n += 1

        BLOCKS = [(0, 0), (1, 0), (0, 4), (2, 0), (1, 4), (3, 0), (2, 4), (3, 4)]
        it = -1
        for (qh, co0_blk) in BLOCKS:
            for co in range(co0_blk, co0_blk + 4):
                it += 1
                keep = 2 if it < 29 else 1
                if av_ready() and len(av_pend) > keep:
                    av_emit()
                quota_fill(3)
                exps = {}
                for g in range(2):
                    for hh in range(2):
                        grp = co // 2
                        for u in [(grp, 0, qh // 2), (grp, 1, qh // 2),
                                  (4 + grp, 0, g), (4 + grp, 1, g)]:
                            if u not in qk_done:
                                emit_qk(u)
                        use_dve = (
                            schr_n < SCHR_MAX
                            and it >= 4
                            and act_t - dve_t > 1200.0
                        )
                        busy_t = dve_t if use_dve else act_t
                        ni = len(exp_fin)
                        cap = exp_fin[ni - 2] if ni >= 2 else 0.0
                        # fill PE while the psum cap blocks or the exp
                        # engine is well-fed
                        while True:
                            gate = max(pe_t, cap)
                            if busy_t - gate < TARGET_BL and pe_t >= cap - 100.0:
                                break
                            if av_ready() and len(av_pend) >= 2:
                                av_emit()
                            elif pick_filler():
                                pass
                            else:
                                break
                            busy_t = dve_t if use_dve else act_t
                        # exp-pool pressure: drain AVs (or the V units
                        # blocking them) before allocating another tile
                        while 4 * len(av_pend) + 6 > 36:
                            if av_ready():
                                av_emit()
                            elif v_q:
                                emit_v()
                            elif not pick_filler():
                                break
                        pe_t = max(pe_t, cap) + C_SC
                        ex = sc_group(qh, co, g, hh, use_dve)
                        flush_tp()
                        if use_dve:
                            schr_n += 1
                            st = max(dve_t, pe_t + 100.0)
                            dve_t = st + C_EXP_DVE
                            exp_fin.append(dve_t)
                        else:
                            st = max(act_t, pe_t + 100.0)
                            act_t = st + C_EXP_ACT
                            exp_fin.append(act_t)
                        exps[(hh, g)] = ex
                av_pend.append((qh, co, exps))
        while av_pend:
            if not av_ready():
                emit_v()
                continue
            av_emit()
        flush_tp()
        while qk_q:
            emit_qk(qk_q[0][1])
        while v_q:
            emit_v()
        while pj_q:
            emit_pj()


def make_in_maps(x, c, kv_w, kv_b, shared_q_w, shared_q_b, cohort_q_w, cohort_q_b,
                 proj_w, proj_b):
    f32 = np.float32
    fp16 = np.float16
    fp8 = mybir.dt.np(FP8)
    x = np.asarray(x, dtype=f32)
    c = np.asarray(c).astype(np.int64)
    kv_w = np.asarray(kv_w, dtype=f32)
    kv_b = np.asarray(kv_b, dtype=f32)
    shared_q_w = np.asarray(shared_q_w, dtype=f32)
    shared_q_b = np.asarray(shared_q_b, dtype=f32)
    cohort_q_w = np.asarray(cohort_q_w, dtype=f32)
    cohort_q_b = np.asarray(cohort_q_b, dtype=f32)
    proj_w = np.asarray(proj_w, dtype=f32)
    proj_b = np.asarray(proj_b, dtype=f32)

    wk = kv_w[:D] * WS
    wv_ = kv_w[D:]
    bk = kv_b[:D] * WS
    bv_ = kv_b[D:]

    wv_h = np.ascontiguousarray(
        wv_.T.reshape(8, P, D).transpose(1, 0, 2)
    ).astype(fp16)
    wp_h = np.ascontiguousarray(
        proj_w.T.reshape(8, P, D).transpose(1, 0, 2)
    ).astype(fp16)

    in_maps = []
    for b in range(x.shape[0]):
        wq = np.concatenate([shared_q_w, cohort_q_w[c[b]]], axis=0) * WS
        bq = np.concatenate([shared_q_b, cohort_q_b[c[b]]], axis=0) * WS
        wqk_cols = np.concatenate([wq, wk], axis=0)     # [2048 e, 1024 d]
        # e = qk*1024 + head*64 + j*32 + i with head = 4*g4 + hh;
        # device wants [p, g(qk,g4), j, t2, dj, ec(hh,i)]
        wqk_e = wqk_cols.reshape(2, 4, 4, 2, 32, D)   # [qk, g4, hh, j, i, d]
        wqk_e = wqk_e.transpose(0, 1, 3, 2, 4, 5).reshape(8, 2, P, D)
        wqk_full = wqk_e.reshape(8, 2, P, 4, 2, P)    # [g, j, ec, t2, dj, p]
        wqk_h = np.ascontiguousarray(
            wqk_full.transpose(5, 0, 1, 3, 4, 2)
        ).astype(fp8)
        bqk_e = np.concatenate([bq, bk]).reshape(2, 4, 4, 2, 32)
        bqk_h = np.ascontiguousarray(
            bqk_e.transpose(0, 1, 3, 2, 4).reshape(8, 2, P).transpose(2, 0, 1)
        ).astype(f32)
        bqk8_h = np.ascontiguousarray(
            bqk_e.transpose(0, 1, 3, 2, 4).reshape(8, 2, P)[None]
        ).astype(fp8)
        xt_h = np.ascontiguousarray(
            x[b].T.reshape(8, P, N).transpose(1, 0, 2)
        ).astype(fp16)
        xdr_h = np.ascontiguousarray(
            x[b].T.reshape(4, 2, P, N).transpose(2, 0, 1, 3)
        ).astype(fp8)
        m = {
            "ident": np.eye(P, dtype=fp16),
            "bqk8": bqk8_h,
            "xdr": xdr_h,
            "wqk": wqk_h,
            "bqk": bqk_h,
            "xt": xt_h,
            "wv": wv_h,
            "bv": np.ascontiguousarray(bv_).astype(mybir.dt.np(BF16)),
            "wp": wp_h,
            "bp": np.ascontiguousarray(proj_b).astype(mybir.dt.np(BF16)),
        }
        in_maps.append(m)
    return in_maps


_NC_CACHE = {}


def kernel(**inputs) -> np.ndarray:
    in_maps = make_in_maps(**inputs)
    if "nc" not in _NC_CACHE:
        _NC_CACHE["nc"] = build_nc()
    nc = _NC_CACHE["nc"]
    res = run_bass_kernel_spmd(nc, in_maps, core_ids=list(range(NCORES)))
    out = np.stack([res.results[i]["out"] for i in range(NCORES)], axis=0)
    return out.astype(np.float32)


# revision 43
# speedup vs baseline: 1.0089x; 1.0089x over previous
"""CohortAwareBlock Trainium2 kernel.

Data-parallel over batch B=8 across 8 NeuronCores (one sample per core).
Cohort routing (gather of cohort_q_w by per-sample cohort id) happens on the
host while building each core's weight tensors; the device kernel is a plain
attention block.

Numerics:
  - QK-gen runs as fp8-e4m3 DoubleRow matmuls (weights pre-scaled x32 to
    dodge fp8 subnormals; the inverse scale is folded into the exp scale).
  - q/k are stored as fp8 in a DoubleRow-interleaved layout ([32, 2, N] per
    head, 4 heads stacked across 128 partitions at 32-partition tile
    positions) so the scores matmul also runs fp8-DR: 2x fewer PE cycles
    than fp16 scores.
  - exp splits across the ACT engine (exact table exp, fp16 out) and the
    DVE (Schraudolph bit-trick: int16(A*s + B) written through a bitcast
    view and read back as fp16; ~1.8% rms sawtooth error, SCHR_MAX-capped
    for the error budget) so the exp stream is not ACT-bound.
  - v / attn weights / projection stay fp16.

Per-core structure:
  q4k4 [128, 8, 2, N] fp8  (4 q-head groups + 4 k-head groups, DR layout)
  v_aug [keys, h, 65] fp16 (col 64 = 1.0 so the flipped AV emits the
                            softmax denominator per q-partition)
  per (q-quarter, head pair):
    scores -> 2-bank PSUM [128, 4, 256] via fp8-DR -> exp (ACT or DVE,
    routed by backlog) ->
    flipped attn@v: av psum [128, 2, 2, 65]; col 64 = den ->
    batched DVE reciprocal + broadcast mult -> nm fp16 ->
    PE-array transpose (vs identity) + DVE evac -> nmT [d, q] ->
    proj (fp16) + bias -> out DMA on the idle GPSIMD queue

Scheduling: a priority scheduler with virtual engine clocks emits score
groups as fast as the sc_ps double-buffer allows (the exp stream is the
critical path), drains attn@v pairs as the preferred PE slack-filler, and
paces QK/V/proj units from per-kind queues (quota per iteration, popped
on demand for data dependencies) so no slow unit head-of-line-blocks the
in-order PE queue. The first QK units evac via ACT-Copy (bias folded in as
a ones-row matmul) while ACT is otherwise idle during startup; dummy
warmup matmuls keep the PE p-state ramped until the first real work; the
iteration visits head pairs in a block order that staggers the V-gen
deadlines.
"""

import numpy as np

import concourse.bass as bass
import concourse.bacc as bacc
import concourse.mybir as mybir
import concourse.tile as tile
from concourse.bass_utils import run_bass_kernel_spmd

P = 128
N = 1024            # sequence length
D = 1024            # model dim
H = 16              # heads
HD = 64             # head dim
NQ = 4              # q-quarters (256 q each)
QW = N // NQ        # 256
SCALE = HD ** -0.5
NCORES = 8

WS = 32.0           # fp8 pre-scale on w_q/w_k (and so on q/k values)
EXP_SCALE = SCALE / (WS * WS)

# Schraudolph fp16-bitcast exp on DVE: y_bits = int16(s * A + B); bits read
# as fp16 give exp(s*EXP_SCALE) with ~1.8% rms sawtooth error.
LOG2E = 1.4426950408889634
SCHR_A = EXP_SCALE * LOG2E * 1024.0
SCHR_B = 15301.0
SCHR_MAX = 30        # max exp groups routed to DVE (of 128); error budget cap

F32 = mybir.dt.float32
FP16 = mybir.dt.float16
BF16 = mybir.dt.bfloat16
FP8 = mybir.dt.float8e4
I16 = mybir.dt.int16
DR = mybir.MatmulPerfMode.DoubleRow
EXP = mybir.ActivationFunctionType.Exp
MUL = mybir.AluOpType.mult
ADD = mybir.AluOpType.add


def build_nc():
    nc = bacc.Bacc(
        "TRN2",
        target_bir_lowering=False,
        debug=False,
        num_devices=NCORES,
    )

    # ---- external I/O (per-core shards, host-prepped layouts) ----
    # DoubleRow-interleaved d-dim: d = (t2*2 + dj)*128 + p
    xdr = nc.dram_tensor("xdr", [P, 4, 2, N], FP8, kind="ExternalInput")
    # wqk[p, g, j, t2, dj, ec]: g = 4-head group (0..3 q, 4..7 k); j = d-half
    # of the head (e_local = j*32 + i); ec = hh*32 + i -> head 4*(g%4)+hh.
    wqk = nc.dram_tensor("wqk", [P, 8, 2, 4, 2, P], FP8, kind="ExternalInput")
    bqk = nc.dram_tensor("bqk", [P, 8, 2], F32, kind="ExternalInput")
    xt = nc.dram_tensor("xt", [P, 8, N], FP16, kind="ExternalInput")   # x^T
    wv = nc.dram_tensor("wv", [P, 8, D], FP16, kind="ExternalInput")
    bv = nc.dram_tensor("bv", [D], BF16, kind="ExternalInput")
    wp = nc.dram_tensor("wp", [P, 8, D], FP16, kind="ExternalInput")
    bp = nc.dram_tensor("bp", [D], BF16, kind="ExternalInput")
    ident = nc.dram_tensor("ident", [P, P], FP16, kind="ExternalInput")
    bqk8 = nc.dram_tensor("bqk8", [1, 8, 2, P], FP8, kind="ExternalInput")
    out = nc.dram_tensor("out", [N, D], F32, kind="ExternalOutput")

    with tile.TileContext(nc) as tc:
        kernel_body(tc, xdr, wqk, bqk, xt, wv, bv, wp, bp, ident, bqk8, out)
    nc.compile()
    return nc


def kernel_body(tc, xdr, wqk, bqk, xt, wv, bv, wp, bp, ident, bqk8, out):
    nc = tc.nc
    from contextlib import ExitStack

    with ExitStack() as ctx:
        ctx.enter_context(
            nc.allow_low_precision(reason="fp16/fp8 matmul inputs by design")
        )
        res = ctx.enter_context(tc.tile_pool(name="res", bufs=1))
        shared = ctx.enter_context(tc.tile_pool(name="shared", bufs=1))
        gen_ps = ctx.enter_context(tc.tile_pool(name="gen_ps", bufs=2, space="PSUM"))
        sc_ps = ctx.enter_context(tc.tile_pool(name="sc_ps", bufs=2, space="PSUM"))
        av_ps = ctx.enter_context(tc.tile_pool(name="av_ps", bufs=2, space="PSUM"))
        exp_pool = ctx.enter_context(tc.tile_pool(name="exp_pool", bufs=36))
        rc_pool = ctx.enter_context(tc.tile_pool(name="rc_pool", bufs=4))
        nm_pool = ctx.enter_context(tc.tile_pool(name="nm_pool", bufs=4))
        oev_pool = ctx.enter_context(tc.tile_pool(name="oev_pool", bufs=2))

        # ---- resident tiles ----
        warm = res.tile([1, 513], FP16)
        nc.gpsimd.memset(warm[:], 1.0)

        xdr_sb = res.tile([P, 4, 2, N], FP8)
        wqk_sb = shared.tile([P, 8, 2, 4, 2, P], FP8, name="wqk_sb")
        bqk_sb = res.tile([P, 8, 2], F32)
        # q/k in scores-DR layout: group g (0..3 q, 4..7 k), partition
        # (hh*32+i), j, token -> value of head 4*(g%4)+hh, d = j*32+i
        q4k4 = res.tile([P, 8, 2, N], FP8)
        xt_sb = res.tile([P, 8, N], FP16)
        wv_sb = res.tile([P, 8, D], FP16)
        bv_rep = res.tile([P, D], BF16)
        wp_holder = []   # allocated from `shared` after QK-gen is emitted
        bp_rep = res.tile([P, D], BF16)

        # v_aug[p, nt, h, :]: cols 0:64 = v for head h at key chunk nt,
        # col 64 = 1.0 (flipped attn@v then emits the softmax denominator
        # in output column 64, one value per q-partition)
        v_aug = res.tile([P, 8, H, HD + 1], FP16)
        nc.gpsimd.memset(v_aug[:, :, :, HD : HD + 1], 1.0)

        # transposed normalized att, packed for proj: [d-part, qc, co, q]
        nmT = res.tile([P, 8, 8, P], FP16)
        ident_sb = res.tile([P, P], FP16)
        bqk8_sb = res.tile([1, 8, 2, P], FP8)
        ones8 = res.tile([1, 512], FP8)
        nc.gpsimd.memset(ones8[:], 1.0)

        # ---- input DMAs (sync queue, need-order; wp follows in the
        # filler queue, reusing wqk's SBUF once QK-gen is done) ----
        nc.sync.dma_start(xdr_sb[:], xdr[:])
        for g in (0, 4):
            for j in range(2):
                nc.sync.dma_start(wqk_sb[:, g, j], wqk[:, g, j])
        nc.sync.dma_start(bqk8_sb[:], bqk8[:])
        nc.sync.dma_start(bqk_sb[:], bqk[:])
        for g in (1, 5):
            for j in range(2):
                nc.sync.dma_start(wqk_sb[:, g, j], wqk[:, g, j])
        for dc in range(8):
            nc.sync.dma_start(wv_sb[:, dc], wv[:, dc])
        nc.sync.dma_start(bv_rep[:], bv[None, :].to_broadcast([P, D]))
        for dc in range(8):
            nc.sync.dma_start(xt_sb[:, dc], xt[:, dc])
        for g in (2, 6, 3, 7):
            for j in range(2):
                nc.sync.dma_start(wqk_sb[:, g, j], wqk[:, g, j])
        nc.sync.dma_start(bp_rep[:], bp[None, :].to_broadcast([P, D]))
        nc.sync.dma_start(ident_sb[:], ident[:])

        # ---------------- emission helpers ----------------
        def warmup():
            # keep the PE p-state ramped while input DMAs land
            ps = gen_ps.tile([P, 512], F32, tag="gps", name="gps")
            nc.tensor.matmul(
                ps[0:1, :],
                lhsT=warm[:, 512:513],
                rhs=warm[:, 0:512],
                start=True,
                stop=True,
            )

        def qk_unit(g, j, ch, act_evac=False):
            # one QK-gen psum group: 4 fp8-DR matmuls + biased fp8 evac into
            # the scores-DR layout (GPSIMD cannot read PSUM, so evac on DVE;
            # the first units evac via ACT-Copy instead -- ACT idles during
            # startup -- with the bias folded in as a ones-row matmul)
            ps = gen_ps.tile([P, 512], F32, tag="gps", name="gps")
            for t2 in range(4):
                nc.tensor.matmul(
                    ps[:],
                    lhsT=wqk_sb[:, g, j, t2],
                    rhs=xdr_sb[:, t2, :, ch * 512 : (ch + 1) * 512],
                    start=(t2 == 0),
                    stop=(t2 == 3) and not act_evac,
                    perf_mode=DR,
                )
            if act_evac:
                nc.tensor.matmul(
                    ps[:],
                    lhsT=bqk8_sb[:, g, j],
                    rhs=ones8[:],
                    start=False,
                    stop=True,
                )
                nc.scalar.activation(
                    q4k4[:, g, j, ch * 512 : (ch + 1) * 512],
                    ps[:],
                    mybir.ActivationFunctionType.Copy,
                )
            else:
                nc.vector.tensor_scalar_add(
                    q4k4[:, g, j, ch * 512 : (ch + 1) * 512],
                    ps[:],
                    bqk_sb[:, g, j : j + 1],
                )

        def v_unit(eq, nt):
            # v[keys nt-chunk, 256 cols (4 heads) of quarter eq]: one psum
            # group + evac; quarter granularity staggers the AV deadlines
            ps = gen_ps.tile([P, 256], F32, tag="gps", name="gps")
            for dc in range(8):
                nc.tensor.matmul(
                    ps[:],
                    lhsT=xt_sb[:, dc, nt * P : (nt + 1) * P],
                    rhs=wv_sb[:, dc, eq * 256 : (eq + 1) * 256],
                    start=(dc == 0),
                    stop=(dc == 7),
                )
            nc.vector.tensor_add(
                v_aug[:, nt, eq * 4 : (eq + 1) * 4, 0:HD],
                ps[:].rearrange("p (h d) -> p h d", d=HD),
                bv_rep[:, eq * 256 : (eq + 1) * 256].rearrange(
                    "p (h d) -> p h d", d=HD
                ),
            )

        def sc_group(qh, co, g, hh, use_dve):
            # one kt-group of scores (fp8-DR) + its batched exp (ACT exact
            # or DVE Schraudolph); returns the exp tile
            h = 2 * co + hh
            grp = h // 4
            r = 32 * (h % 4)
            q0 = qh * QW
            ps = sc_ps.tile([P, 4, QW], F32, tag="scps", name="scps")
            for ki in range(4):
                kt = g * 4 + ki
                nc.tensor.matmul(
                    ps[:, ki],
                    lhsT=q4k4[r : r + 32, 4 + grp, :, kt * P : (kt + 1) * P],
                    rhs=q4k4[r : r + 32, grp, :, q0 : q0 + QW],
                    start=True,
                    stop=True,
                    perf_mode=DR,
                    tile_position=(r, 0),
                )
            ex = exp_pool.tile([P, 4, QW], FP16, tag="exp", name="exp")
            if use_dve:
                nc.vector.tensor_scalar(
                    ex[:].bitcast(I16),
                    ps[:],
                    SCHR_A,
                    SCHR_B,
                    op0=MUL,
                    op1=ADD,
                )
            else:
                nc.scalar.activation(ex[:], ps[:], EXP, scale=EXP_SCALE)
            return ex

        def av_halves(qh, co, exps):
            # flipped attn@v for one head pair, split per head; the batched
            # norm runs after the second half; the nm -> nmT transpose is a
            # separate unit (PE-array transpose + DVE evac) emitted later
            hold = []

            def half(hh):
                h = 2 * co + hh
                if hh == 0:
                    t = av_ps.tile([P, 392], F32, tag="avps", name="avps")
                    hold.append(t)
                ps = hold[0][:, 0:260].rearrange(
                    "p (a b c) -> p a b c", a=2, b=2
                )
                for qs in range(2):
                    for kt in range(8):
                        nc.tensor.matmul(
                            ps[:, qs, hh],
                            lhsT=exps[(hh, kt // 4)][:, kt % 4,
                                                     qs * P : (qs + 1) * P],
                            rhs=v_aug[:, kt, h, :],
                            start=(kt == 0),
                            stop=(kt == 7),
                        )
                if hh == 1:
                    rc = rc_pool.tile([P, 2, 2, 1], F32, tag="rc", name="rc")
                    nc.vector.reciprocal(rc[:], ps[:, :, :, HD : HD + 1])
                    nm = nm_pool.tile([P, 2, 2, HD], FP16, tag="nm", name="nm")
                    nc.vector.tensor_tensor(
                        nm[:],
                        ps[:, :, :, 0:HD],
                        rc[:].broadcast_to([P, 2, 2, HD]),
                        op=MUL,
                    )
                    hold.append(nm)

            def tp():
                # PE-array transpose of nm into proj layout + DVE evac;
                # keeps the nmT chain off the slow DMA queues
                t, nm = hold
                tpv = t[:, 264:392].bitcast(FP16).rearrange(
                    "p (a q) -> p a q", a=2
                )
                for qs in range(2):
                    nc.tensor.transpose(
                        tpv[:, qs], nm[:, qs], ident_sb[:]
                    )
                nc.vector.tensor_copy(
                    nmT[:, qh * 2 : qh * 2 + 2, co, :], tpv[:]
                )

            return (lambda: half(0)), (lambda: half(1)), tp

        def pj_halves(qh, nt, fh):
            # one projection output group split into two PE units
            qc = qh * 2 + nt
            n0 = qc * P
            hold = []

            def a():
                ps = gen_ps.tile([P, 512], F32, tag="gps", name="gps")
                hold.append(ps)
                for co in range(4):
                    nc.tensor.matmul(
                        ps[:],
                        lhsT=nmT[:, qc, co, :],
                        rhs=wp_holder[0][:, co, fh * 512 : (fh + 1) * 512],
                        start=(co == 0),
                        stop=False,
                    )

            def b():
                ps = hold[0]
                for co in range(4, 8):
                    nc.tensor.matmul(
                        ps[:],
                        lhsT=nmT[:, qc, co, :],
                        rhs=wp_holder[0][:, co, fh * 512 : (fh + 1) * 512],
                        start=False,
                        stop=(co == 7),
                    )
                ev = oev_pool.tile([P, 512], F32, tag="oev", name="oev")
                nc.vector.tensor_add(
                    ev[:], ps[:], bp_rep[:, fh * 512 : (fh + 1) * 512]
                )
                nc.sync.dma_start(
                    out[n0 : n0 + P, fh * 512 : (fh + 1) * 512], ev[:]
                )

            return a, b

        # ---------------- schedule ----------------
        # Priority scheduler with virtual engine clocks (pe/act/dve busy-until
        # estimates under the cost model). The exp stream (ACT + DVE
        # Schraudolph, routed by backlog) is the critical path; score groups
        # are emitted as fast as the sc_ps double-buffer allows. AV pairs and
        # QK/V/proj units fill PE slack one unit at a time from per-kind
        # queues, so a unit needed soon never forces a burst-drain of
        # unrelated work (which would starve the exp engines).
        from collections import deque

        C_SC = 220.0          # score group PE (4 fp8-DR matmuls)
        C_EXP_ACT = 1110.0
        C_EXP_DVE = 1280.0
        C_AVH = 440.0         # AV half PE
        C_NORM = 800.0        # batched recip+mult DVE
        C_QK = 430.0          # QK unit PE
        C_QK_EV = 750.0       # QK evac DVE
        C_VQ = 858.0          # V quarter-unit PE
        C_VQ_EV = 485.0       # V quarter evac DVE
        C_PJ = 1704.0         # proj unit PE (a+b)
        C_PJ_EV = 705.0       # proj evac DVE
        C_TP = 115.0          # nm transpose PE (2 PE-array transposes)
        C_TP_EV = 320.0       # nmT evac DVE
        TARGET_BL = 3600.0

        for _ in range(12):
            warmup()

        # QK units: first 4 unblock (qh0, co0, g0) scores; the rest are
        # popped on demand (per-unit) or as slack fillers
        qk_first = [(0, 0, 0), (0, 1, 0), (4, 0, 0), (4, 1, 0)]
        qk_q = deque()
        for (g, j, ch, est) in [(4, 0, 1, 5200.0), (4, 1, 1, 5600.0),
                                (0, 0, 1, 6000.0), (0, 1, 1, 6400.0)]:
            qk_q.append((est, (g, j, ch)))
        for grp, est in ((1, 7800.0), (2, 20700.0), (3, 22100.0)):
            for (g, ch) in ((grp, 0), (4 + grp, 0), (4 + grp, 1), (grp, 1)):
                for j in range(2):
                    qk_q.append((est, (g, j, ch)))
        v_q = deque()
        for eq in range(4):
            for nt in range(8):
                v_q.append((19300.0, (eq, nt)))
        pj_q = deque()

        pe_t = 6300.0
        act_t = 0.0
        dve_t = 0.0
        qk_done = set(qk_first)
        for u in qk_first:
            qk_unit(*u, act_evac=True)
            pe_t += C_QK + 213.0
            act_t = max(act_t, pe_t + 100.0) + 700.0

        av_pend = deque()
        exp_fin = []
        schr_n = 0
        v_pops = [0, 0, 0, 0]
        counts = [0] * NQ
        wp_loaded = [False]

        def wall():
            return max(pe_t, act_t - 2.0 * C_EXP_ACT, dve_t - 2.0 * C_EXP_DVE)

        def emit_qk(u):
            nonlocal pe_t, dve_t
            est = None
            for (e, uu) in qk_q:
                if uu == u:
                    est = e
                    break
            qk_q.remove((est, u))
            qk_unit(*u)
            qk_done.add(u)
            pe_t = max(pe_t, est) + C_QK
            dve_t = max(dve_t, pe_t + 100.0) + C_QK_EV
            if not qk_q and not wp_loaded[0]:
                wp_loaded[0] = True
                wp_load()

        def wp_load():
            wp_holder.append(shared.tile([P, 8, D], FP16, name="wp_sb"))
            for co in range(8):
                nc.sync.dma_start(wp_holder[0][:, co], wp[:, co])

        def emit_v():
            nonlocal pe_t, dve_t
            est, (eq, nt) = v_q.popleft()
            v_unit(eq, nt)
            v_pops[eq] += 1
            pe_t = max(pe_t, est) + C_VQ
            dve_t = max(dve_t, pe_t + 100.0) + C_VQ_EV

        def emit_pj():
            nonlocal pe_t, dve_t
            est, (qh0, nt, fh) = pj_q.popleft()
            a, b = pj_halves(qh0, nt, fh)
            a()
            b()
            pe_t = max(pe_t, est) + C_PJ
            dve_t = max(dve_t, pe_t + 100.0) + C_PJ_EV

        def pick_filler():
            # one slack unit, earliest-est first; False if nothing eligible
            cands = []
            if qk_q:
                cands.append((qk_q[0][0], 0))
            if v_q:
                cands.append((v_q[0][0], 1))
            if pj_q:
                cands.append((pj_q[0][0], 2))
            cands = [c for c in cands if c[0] <= wall() + 400.0]
            if not cands:
                return False
            cands.sort()
            kind = cands[0][1]
            if kind == 0:
                emit_qk(qk_q[0][1])
            elif kind == 1:
                emit_v()
            else:
                emit_pj()
            return True

        pending_tp = []

        def flush_tp():
            nonlocal pe_t, dve_t
            while pending_tp:
                qh0, co0, tp = pending_tp.pop(0)
                tp()
                pe_t += C_TP
                dve_t = max(dve_t, pe_t + 100.0) + C_TP_EV
                counts[qh0] += 1
                if counts[qh0] == 8:
                    est_pj = max(wall(), dve_t) + 1600.0
                    for nt in range(2):
                        for fh in range(2):
                            pj_q.append((est_pj, (qh0, nt, fh)))

        def av_ready():
            if not av_pend:
                return False
            qh0, co0, _ = av_pend[0]
            return v_pops[co0 // 2] >= 8

        def av_emit():
            nonlocal pe_t, dve_t
            flush_tp()
            qh0, co0, exps0 = av_pend.popleft()
            a, b, tp = av_halves(qh0, co0, exps0)
            a()
            pe_t += C_AVH
            b()
            pe_t += C_AVH
            dve_t = max(dve_t, pe_t + 100.0) + C_NORM
            pending_tp.append((qh0, co0, tp))

        FILL_TOTAL = 24 * C_QK + 32 * C_VQ + 32 * C_PJ / 2.0
        fill_pe = [0.0]

        def quota_fill(limit):
            n = 0
            while (
                n < limit
                and fill_pe[0] < (it + 1) * (FILL_TOTAL / 32.0)
            ):
                before = (len(qk_q), len(v_q), len(pj_q))
                if not pick_filler():
                    break
                after = (len(qk_q), len(v_q), len(pj_q))
                if before[0] != after[0]:
                    fill_pe[0] += C_QK
                elif before[1] != after[1]:
                    fill_pe[0] += C_VQ
                else:
                    fill_pe[0] += C_PJ
                n += 1

        BLOCKS = [(0, 0), (1, 0), (0, 4), (2, 0), (1, 4), (3, 0), (2, 4), (3, 4)]
        it = -1
        for (qh, co0_blk) in BLOCKS:
            for co in range(co0_blk, co0_blk + 4):
                it += 1
                keep = 2 if it < 29 else 1
                if av_ready() and len(av_pend) > keep:
                    av_emit()
                quota_fill(3)
                exps = {}
                for g in range(2):
                    for hh in range(2):
                        grp = co // 2
                        for u in [(grp, 0, qh // 2), (grp, 1, qh // 2),
                                  (4 + grp, 0, g), (4 + grp, 1, g)]:
                            if u not in qk_done:
                                emit_qk(u)
                        use_dve = (
                            schr_n < SCHR_MAX
                            and it >= 4
                            and act_t - dve_t > 1200.0
                        )
                        busy_t = dve_t if use_dve else act_t
                        ni = len(exp_fin)
                        cap = exp_fin[ni - 2] if ni >= 2 else 0.0
                        # fill PE while the psum cap blocks or the exp
                        # engine is well-fed
                        while True:
                            gate = max(pe_t, cap)
                            if busy_t - gate < TARGET_BL and pe_t >= cap - 100.0:
                                break
                            if av_ready() and len(av_pend) >= 2:
                                av_emit()
                            elif pick_filler():
                                pass
                            else:
                                break
                            busy_t = dve_t if use_dve else act_t
                        # exp-pool pressure: drain AVs (or the V units
                        # blocking them) before allocating another tile
                        while 4 * len(av_pend) + 6 > 36:
                            if av_ready():
                                av_emit()
                            elif v_q:
                                emit_v()
                            elif not pick_filler():
                                break
                        pe_t = max(pe_t, cap) + C_SC
                        ex = sc_group(qh, co, g, hh, use_dve)
                        flush_tp()
                        if use_dve:
                            schr_n += 1
                            st = max(dve_t, pe_t + 100.0)
                            dve_t = st + C_EXP_DVE
                            exp_fin.append(dve_t)
                        else:
                            st = max(act_t, pe_t + 100.0)
                            act_t = st + C_EXP_ACT
                            exp_fin.append(act_t)
                        exps[(hh, g)] = ex
                av_pend.append((qh, co, exps))
        while av_pend:
            if not av_ready():
                emit_v()
                continue
            av_emit()
        flush_tp()
        while qk_q:
            emit_qk(qk_q[0][1])
        while v_q:
            emit_v()
        while pj_q:
            emit_pj()


def make_in_maps(x, c, kv_w, kv_b, shared_q_w, shared_q_b, cohort_q_w, cohort_q_b,
                 proj_w, proj_b):
    f32 = np.float32
    fp16 = np.float16
    fp8 = mybir.dt.np(FP8)
    x = np.asarray(x, dtype=f32)
    c = np.asarray(c).astype(np.int64)
    kv_w = np.asarray(kv_w, dtype=f32)
    kv_b = np.asarray(kv_b, dtype=f32)
    shared_q_w = np.asarray(shared_q_w, dtype=f32)
    shared_q_b = np.asarray(shared_q_b, dtype=f32)
    cohort_q_w = np.asarray(cohort_q_w, dtype=f32)
    cohort_q_b = np.asarray(cohort_q_b, dtype=f32)
    proj_w = np.asarray(proj_w, dtype=f32)
    proj_b = np.asarray(proj_b, dtype=f32)

    wk = kv_w[:D] * WS
    wv_ = kv_w[D:]
    bk = kv_b[:D] * WS
    bv_ = kv_b[D:]

    wv_h = np.ascontiguousarray(
        wv_.T.reshape(8, P, D).transpose(1, 0, 2)
    ).astype(fp16)
    wp_h = np.ascontiguousarray(
        proj_w.T.reshape(8, P, D).transpose(1, 0, 2)
    ).astype(fp16)

    in_maps = []
    for b in range(x.shape[0]):
        wq = np.concatenate([shared_q_w, cohort_q_w[c[b]]], axis=0) * WS
        bq = np.concatenate([shared_q_b, cohort_q_b[c[b]]], axis=0) * WS
        wqk_cols = np.concatenate([wq, wk], axis=0)     # [2048 e, 1024 d]
        # e = qk*1024 + head*64 + j*32 + i with head = 4*g4 + hh;
        # device wants [p, g(qk,g4), j, t2, dj, ec(hh,i)]
        wqk_e = wqk_cols.reshape(2, 4, 4, 2, 32, D)   # [qk, g4, hh, j, i, d]
        wqk_e = wqk_e.transpose(0, 1, 3, 2, 4, 5).reshape(8, 2, P, D)
        wqk_full = wqk_e.reshape(8, 2, P, 4, 2, P)    # [g, j, ec, t2, dj, p]
        wqk_h = np.ascontiguousarray(
            wqk_full.transpose(5, 0, 1, 3, 4, 2)
        ).astype(fp8)
        bqk_e = np.concatenate([bq, bk]).reshape(2, 4, 4, 2, 32)
        bqk_h = np.ascontiguousarray(
            bqk_e.transpose(0, 1, 3, 2, 4).reshape(8, 2, P).transpose(2, 0, 1)
        ).astype(f32)
        bqk8_h = np.ascontiguousarray(
            bqk_e.transpose(0, 1, 3, 2, 4).reshape(8, 2, P)[None]
        ).astype(fp8)
        xt_h = np.ascontiguousarray(
            x[b].T.reshape(8, P, N).transpose(1, 0, 2)
        ).astype(fp16)
        xdr_h = np.ascontiguousarray(
            x[b].T.reshape(4, 2, P, N).transpose(2, 0, 1, 3)
        ).astype(fp8)
        m = {
            "ident": np.eye(P, dtype=fp16),
            "bqk8": bqk8_h,
            "xdr": xdr_h,
            "wqk": wqk_h,
            "bqk": bqk_h,
            "xt": xt_h,
            "wv": wv_h,
            "bv": np.ascontiguousarray(bv_).astype(mybir.dt.np(BF16)),
            "wp": wp_h,
            "bp": np.ascontiguousarray(proj_b).astype(mybir.dt.np(BF16)),
        }
        in_maps.append(m)
    return in_maps


_NC_CACHE = {}


def kernel(**inputs) -> np.ndarray:
    in_maps = make_in_maps(**inputs)
    if "nc" not in _NC_CACHE:
        _NC_CACHE["nc"] = build_nc()
    nc = _NC_CACHE["nc"]
    res = run_bass_kernel_spmd(nc, in_maps, core_ids=list(range(NCORES)))
    out = np.stack([res.results[i]["out"] for i in range(NCORES)], axis=0)
    return out.astype(np.float32)


# revision 60
# speedup vs baseline: 1.0282x; 1.0192x over previous
"""CohortAwareBlock Trainium2 kernel.

Data-parallel over batch B=8 across 8 NeuronCores (one sample per core).
Cohort routing (gather of cohort_q_w by per-sample cohort id) happens on the
host while building each core's weight tensors; the device kernel is a plain
attention block.

Numerics:
  - QK-gen runs as fp8-e4m3 DoubleRow matmuls (weights pre-scaled x32 to
    dodge fp8 subnormals; the inverse scale is folded into the exp scale).
  - q/k are stored as fp8 in a DoubleRow-interleaved layout ([32, 2, N] per
    head, 4 heads stacked across 128 partitions at 32-partition tile
    positions) so the scores matmul also runs fp8-DR: 2x fewer PE cycles
    than fp16 scores.
  - exp splits across the ACT engine (exact table exp, fp16 out) and the
    DVE (Schraudolph bit-trick: int16(A*s + B) written through a bitcast
    view and read back as fp16; ~1.8% rms sawtooth error, SCHR_MAX-capped
    for the error budget) so the exp stream is not ACT-bound.
  - v / attn weights / projection stay fp16.

Per-core structure:
  q4k4 [128, 8, 2, N] fp8  (4 q-head groups + 4 k-head groups, DR layout)
  v_aug [keys, h, 65] fp16 (col 64 = 1.0 so the flipped AV emits the
                            softmax denominator per q-partition)
  per (q-quarter, head pair):
    scores -> 2-bank PSUM [128, 4, 256] via fp8-DR -> exp (ACT or DVE,
    routed by backlog) ->
    flipped attn@v: av psum [128, 2, 2, 65]; col 64 = den ->
    batched DVE reciprocal + broadcast mult -> nm fp16 ->
    PE-array transpose (vs identity) + DVE evac -> nmT [d, q] ->
    proj (fp16) + bias -> out DMA on the idle GPSIMD queue

Scheduling: a priority scheduler with virtual engine clocks emits score
groups as fast as the sc_ps double-buffer allows (the exp stream is the
critical path), drains attn@v pairs as the preferred PE slack-filler, and
paces QK/V/proj units from per-kind queues (quota per iteration, popped
on demand for data dependencies) so no slow unit head-of-line-blocks the
in-order PE queue. The first QK units evac via ACT-Copy (bias folded in as
a ones-row matmul) while ACT is otherwise idle during startup; dummy
warmup matmuls keep the PE p-state ramped until the first real work; the
iteration visits head pairs in a block order that staggers the V-gen
deadlines.
"""

import numpy as np

import concourse.bass as bass
import concourse.bacc as bacc
import concourse.mybir as mybir
import concourse.tile as tile
from concourse.bass_utils import run_bass_kernel_spmd

P = 128
N = 1024            # sequence length
D = 1024            # model dim
H = 16              # heads
HD = 64             # head dim
NQ = 4              # q-quarters (256 q each)
QW = N // NQ        # 256
SCALE = HD ** -0.5
NCORES = 8

WS = 32.0           # fp8 pre-scale on w_q/w_k (and so on q/k values)
EXP_SCALE = SCALE / (WS * WS)

# Schraudolph fp16-bitcast exp on DVE: y_bits = int16(s * A + B); bits read
# as fp16 give exp(s*EXP_SCALE) with ~1.8% rms sawtooth error.
LOG2E = 1.4426950408889634
SCHR_A = EXP_SCALE * LOG2E * 1024.0
SCHR_B = 15301.0
SCHR_MAX = 27        # max exp groups routed to DVE (of 128); error budget cap

F32 = mybir.dt.float32
FP16 = mybir.dt.float16
BF16 = mybir.dt.bfloat16
FP8 = mybir.dt.float8e4
I16 = mybir.dt.int16
DR = mybir.MatmulPerfMode.DoubleRow
EXP = mybir.ActivationFunctionType.Exp
MUL = mybir.AluOpType.mult
ADD = mybir.AluOpType.add


def build_nc():
    nc = bacc.Bacc(
        "TRN2",
        target_bir_lowering=False,
        debug=False,
        num_devices=NCORES,
    )

    # ---- external I/O (per-core shards, host-prepped layouts) ----
    # DoubleRow-interleaved d-dim: d = (t2*2 + dj)*128 + p
    xdr = nc.dram_tensor("xdr", [P, 4, 2, N], FP8, kind="ExternalInput")
    # wqk[p, g, j, t2, dj, ec]: g = 4-head group (0..3 q, 4..7 k); j = d-half
    # of the head (e_local = j*32 + i); ec = hh*32 + i -> head 4*(g%4)+hh.
    wqk = nc.dram_tensor("wqk", [P, 8, 2, 4, 2, P], FP8, kind="ExternalInput")
    bqk = nc.dram_tensor("bqk", [P, 8, 2], F32, kind="ExternalInput")
    xt = nc.dram_tensor("xt", [P, 8, N], FP16, kind="ExternalInput")   # x^T
    wv = nc.dram_tensor("wv", [P, 8, D], FP16, kind="ExternalInput")
    bv = nc.dram_tensor("bv", [D], BF16, kind="ExternalInput")
    wp = nc.dram_tensor("wp", [P, 8, D], FP16, kind="ExternalInput")
    bp = nc.dram_tensor("bp", [D], BF16, kind="ExternalInput")
    ident = nc.dram_tensor("ident", [P, P], FP16, kind="ExternalInput")
    bqk8 = nc.dram_tensor("bqk8", [1, 8, 2, P], FP8, kind="ExternalInput")
    out = nc.dram_tensor("out", [N, D], F32, kind="ExternalOutput")

    with tile.TileContext(nc) as tc:
        kernel_body(tc, xdr, wqk, bqk, xt, wv, bv, wp, bp, ident, bqk8, out)
    nc.compile()
    return nc


def kernel_body(tc, xdr, wqk, bqk, xt, wv, bv, wp, bp, ident, bqk8, out):
    nc = tc.nc
    from contextlib import ExitStack

    with ExitStack() as ctx:
        ctx.enter_context(
            nc.allow_low_precision(reason="fp16/fp8 matmul inputs by design")
        )
        res = ctx.enter_context(tc.tile_pool(name="res", bufs=1))
        shared = ctx.enter_context(tc.tile_pool(name="shared", bufs=1))
        gen_ps = ctx.enter_context(tc.tile_pool(name="gen_ps", bufs=2, space="PSUM"))
        av_ps = ctx.enter_context(tc.tile_pool(name="av_ps", bufs=2, space="PSUM"))
        sc_ps = ctx.enter_context(tc.tile_pool(name="sc_ps", bufs=2, space="PSUM"))
        exp_pool = ctx.enter_context(tc.tile_pool(name="exp_pool", bufs=36))
        rc_pool = ctx.enter_context(tc.tile_pool(name="rc_pool", bufs=4))
        nm_pool = ctx.enter_context(tc.tile_pool(name="nm_pool", bufs=4))
        oev_pool = ctx.enter_context(tc.tile_pool(name="oev_pool", bufs=2))

        # ---- resident tiles ----
        warm = res.tile([1, 513], FP16)
        nc.gpsimd.memset(warm[:], 1.0)

        xdr_sb = res.tile([P, 4, 2, N], FP8)
        wqk_sb = shared.tile([P, 8, 2, 4, 2, P], FP8, name="wqk_sb")
        bqk_sb = res.tile([P, 8, 2], F32)
        # q/k in scores-DR layout: group g (0..3 q, 4..7 k), partition
        # (hh*32+i), j, token -> value of head 4*(g%4)+hh, d = j*32+i
        q4k4 = res.tile([P, 8, 2, N], FP8)
        xt_sb = res.tile([P, 8, N], FP16)
        wv_sb = res.tile([P, 8, D], FP16)
        bv_rep = res.tile([P, D], BF16)
        wp_holder = []   # allocated from `shared` after QK-gen is emitted
        bp_rep = res.tile([P, D], BF16)

        # v_aug[p, nt, h, :]: cols 0:64 = v for head h at key chunk nt,
        # col 64 = 1.0 (flipped attn@v then emits the softmax denominator
        # in output column 64, one value per q-partition)
        v_aug = res.tile([P, 8, H, HD + 1], FP16)
        nc.gpsimd.memset(v_aug[:, :, :, HD : HD + 1], 1.0)

        # transposed normalized att, packed for proj: [d-part, qc, co, q]
        nmT = res.tile([P, 8, 8, P], FP16)
        ident_sb = res.tile([P, P], FP16)
        bqk8_sb = res.tile([1, 8, 2, P], FP8)
        ones8 = res.tile([1, 512], FP8)
        nc.gpsimd.memset(ones8[:], 1.0)

        # ---- input DMAs (sync queue, need-order; wp follows in the
        # filler queue, reusing wqk's SBUF once QK-gen is done) ----
        nc.sync.dma_start(xdr_sb[:], xdr[:])
        for g in (0, 4):
            for j in range(2):
                nc.sync.dma_start(wqk_sb[:, g, j], wqk[:, g, j])
        nc.sync.dma_start(bqk8_sb[:], bqk8[:])
        nc.sync.dma_start(bqk_sb[:], bqk[:])
        for g in (1, 5):
            for j in range(2):
                nc.sync.dma_start(wqk_sb[:, g, j], wqk[:, g, j])
        for dc in range(8):
            nc.sync.dma_start(wv_sb[:, dc], wv[:, dc])
        nc.sync.dma_start(bv_rep[:], bv[None, :].to_broadcast([P, D]))
        for dc in range(8):
            nc.sync.dma_start(xt_sb[:, dc], xt[:, dc])
        for g in (2, 6, 3, 7):
            for j in range(2):
                nc.sync.dma_start(wqk_sb[:, g, j], wqk[:, g, j])
        nc.sync.dma_start(bp_rep[:], bp[None, :].to_broadcast([P, D]))
        nc.sync.dma_start(ident_sb[:], ident[:])

        # ---------------- emission helpers ----------------
        def warmup():
            # keep the PE p-state ramped while input DMAs land
            ps = gen_ps.tile([P, 512], F32, tag="gps", name="gps")
            nc.tensor.matmul(
                ps[0:1, :],
                lhsT=warm[:, 512:513],
                rhs=warm[:, 0:512],
                start=True,
                stop=True,
            )

        def qk_unit(g, j, ch, act_evac=False):
            # one QK-gen psum group: 4 fp8-DR matmuls + biased fp8 evac into
            # the scores-DR layout (GPSIMD cannot read PSUM, so evac on DVE;
            # the first units evac via ACT-Copy instead -- ACT idles during
            # startup -- with the bias folded in as a ones-row matmul)
            ps = gen_ps.tile([P, 512], F32, tag="gps", name="gps")
            for t2 in range(4):
                nc.tensor.matmul(
                    ps[:],
                    lhsT=wqk_sb[:, g, j, t2],
                    rhs=xdr_sb[:, t2, :, ch * 512 : (ch + 1) * 512],
                    start=(t2 == 0),
                    stop=(t2 == 3) and not act_evac,
                    perf_mode=DR,
                )
            if act_evac:
                nc.tensor.matmul(
                    ps[:],
                    lhsT=bqk8_sb[:, g, j],
                    rhs=ones8[:],
                    start=False,
                    stop=True,
                )
                nc.scalar.activation(
                    q4k4[:, g, j, ch * 512 : (ch + 1) * 512],
                    ps[:],
                    mybir.ActivationFunctionType.Copy,
                )
            else:
                nc.vector.tensor_scalar_add(
                    q4k4[:, g, j, ch * 512 : (ch + 1) * 512],
                    ps[:],
                    bqk_sb[:, g, j : j + 1],
                )

        def v_unit(eq, nt):
            # v[keys nt-chunk, 256 cols (4 heads) of quarter eq]: one psum
            # group + evac; quarter granularity staggers the AV deadlines
            ps = gen_ps.tile([P, 256], F32, tag="gps", name="gps")
            for dc in range(8):
                nc.tensor.matmul(
                    ps[:],
                    lhsT=xt_sb[:, dc, nt * P : (nt + 1) * P],
                    rhs=wv_sb[:, dc, eq * 256 : (eq + 1) * 256],
                    start=(dc == 0),
                    stop=(dc == 7),
                )
            nc.vector.tensor_add(
                v_aug[:, nt, eq * 4 : (eq + 1) * 4, 0:HD],
                ps[:].rearrange("p (h d) -> p h d", d=HD),
                bv_rep[:, eq * 256 : (eq + 1) * 256].rearrange(
                    "p (h d) -> p h d", d=HD
                ),
            )

        def sc_group(qh, co, g, hh, use_dve):
            # one kt-group of scores (fp8-DR) + its batched exp (ACT exact
            # or DVE Schraudolph); returns the exp tile
            h = 2 * co + hh
            grp = h // 4
            r = 32 * (h % 4)
            q0 = qh * QW
            ps = sc_ps.tile([P, 4, QW], F32, tag="scps", name="scps")
            for ki in range(4):
                kt = g * 4 + ki
                nc.tensor.matmul(
                    ps[:, ki],
                    lhsT=q4k4[r : r + 32, 4 + grp, :, kt * P : (kt + 1) * P],
                    rhs=q4k4[r : r + 32, grp, :, q0 : q0 + QW],
                    start=True,
                    stop=True,
                    perf_mode=DR,
                    tile_position=(r, 0),
                )
            ex = exp_pool.tile([P, 4, QW], FP16, tag="exp", name="exp")
            if use_dve:
                nc.vector.tensor_scalar(
                    ex[:].bitcast(I16),
                    ps[:],
                    SCHR_A,
                    SCHR_B,
                    op0=MUL,
                    op1=ADD,
                )
            else:
                nc.scalar.activation(ex[:], ps[:], EXP, scale=EXP_SCALE)
            return ex

        def av_halves(qh, co, exps):
            # flipped attn@v for one head pair, split per head; the batched
            # norm runs after the second half; the nm -> nmT transpose is a
            # separate unit (PE-array transpose + DVE evac) emitted later
            hold = []

            def half(hh):
                h = 2 * co + hh
                if hh == 0:
                    t = av_ps.tile([P, 392], F32, tag="avps", name="avps")
                    hold.append(t)
                ps = hold[0][:, 0:260].rearrange(
                    "p (a b c) -> p a b c", a=2, b=2
                )
                for qs in range(2):
                    for kt in range(8):
                        nc.tensor.matmul(
                            ps[:, qs, hh],
                            lhsT=exps[(hh, kt // 4)][:, kt % 4,
                                                     qs * P : (qs + 1) * P],
                            rhs=v_aug[:, kt, h, :],
                            start=(kt == 0),
                            stop=(kt == 7),
                        )
                if hh == 1:
                    rc = rc_pool.tile([P, 2, 2, 1], F32, tag="rc", name="rc")
                    nc.vector.reciprocal(rc[:], ps[:, :, :, HD : HD + 1])
                    nm = nm_pool.tile([P, 2, 2, HD], FP16, tag="nm", name="nm")
                    nc.vector.tensor_tensor(
                        nm[:],
                        ps[:, :, :, 0:HD],
                        rc[:].broadcast_to([P, 2, 2, HD]),
                        op=MUL,
                    )
                    hold.append(nm)

            def tp():
                # PE-array transpose of nm into proj layout + DVE evac;
                # keeps the nmT chain off the slow DMA queues
                t, nm = hold
                tpv = t[:, 264:392].bitcast(FP16).rearrange(
                    "p (a q) -> p a q", a=2
                )
                for qs in range(2):
                    nc.tensor.transpose(
                        tpv[:, qs], nm[:, qs], ident_sb[:]
                    )
                nc.vector.tensor_copy(
                    nmT[:, qh * 2 : qh * 2 + 2, co, :], tpv[:]
                )

            return (lambda: half(0)), (lambda: half(1)), tp

        def pj_halves(qh, nt, fh):
            # one projection output group split into two PE units
            qc = qh * 2 + nt
            n0 = qc * P
            hold = []

            def a():
                ps = gen_ps.tile([P, 512], F32, tag="gps", name="gps")
                hold.append(ps)
                for co in range(4):
                    nc.tensor.matmul(
                        ps[:],
                        lhsT=nmT[:, qc, co, :],
                        rhs=wp_holder[0][:, co, fh * 512 : (fh + 1) * 512],
                        start=(co == 0),
                        stop=False,
                    )

            def b():
                ps = hold[0]
                for co in range(4, 8):
                    nc.tensor.matmul(
                        ps[:],
                        lhsT=nmT[:, qc, co, :],
                        rhs=wp_holder[0][:, co, fh * 512 : (fh + 1) * 512],
                        start=False,
                        stop=(co == 7),
                    )
                ev = oev_pool.tile([P, 512], F32, tag="oev", name="oev")
                nc.vector.tensor_add(
                    ev[:], ps[:], bp_rep[:, fh * 512 : (fh + 1) * 512]
                )
                nc.sync.dma_start(
                    out[n0 : n0 + P, fh * 512 : (fh + 1) * 512], ev[:]
                )

            return a, b

        # ---------------- schedule ----------------
        # Priority scheduler with virtual engine clocks (pe/act/dve busy-until
        # estimates under the cost model). The exp stream (ACT + DVE
        # Schraudolph, routed by backlog) is the critical path; score groups
        # are emitted as fast as the sc_ps double-buffer allows. AV pairs and
        # QK/V/proj units fill PE slack one unit at a time from per-kind
        # queues, so a unit needed soon never forces a burst-drain of
        # unrelated work (which would starve the exp engines).
        from collections import deque

        C_SC = 220.0          # score group PE (4 fp8-DR matmuls)
        C_EXP_ACT = 1110.0
        C_EXP_DVE = 1280.0
        C_AVH = 440.0         # AV half PE
        C_NORM = 800.0        # batched recip+mult DVE
        C_QK = 430.0          # QK unit PE
        C_QK_EV = 750.0       # QK evac DVE
        C_VQ = 858.0          # V quarter-unit PE
        C_VQ_EV = 485.0       # V quarter evac DVE
        C_PJ = 1704.0         # proj unit PE (a+b)
        C_PJ_EV = 705.0       # proj evac DVE
        C_TP = 115.0          # nm transpose PE (2 PE-array transposes)
        C_TP_EV = 320.0       # nmT evac DVE
        TARGET_BL = 3600.0

        for _ in range(12):
            warmup()

        # QK units: first 4 unblock (qh0, co0, g0) scores; the rest are
        # popped on demand (per-unit) or as slack fillers
        qk_first = [(0, 0, 0), (0, 1, 0), (4, 0, 0), (4, 1, 0)]
        qk_q = deque()
        for (g, j, ch, est) in [(4, 0, 1, 5200.0), (4, 1, 1, 5600.0),
                                (0, 0, 1, 6000.0), (0, 1, 1, 6400.0)]:
            qk_q.append((est, (g, j, ch)))
        for grp, est in ((1, 7800.0), (2, 20700.0), (3, 22100.0)):
            for (g, ch) in ((grp, 0), (4 + grp, 0), (4 + grp, 1), (grp, 1)):
                for j in range(2):
                    qk_q.append((est, (g, j, ch)))
        v_q = deque()
        for eq in range(4):
            for nt in range(8):
                v_q.append((19300.0, (eq, nt)))
        pj_q = deque()

        pe_t = 6300.0
        act_t = 0.0
        dve_t = 0.0
        qk_done = set(qk_first)
        for u in qk_first:
            qk_unit(*u, act_evac=True)
            pe_t += C_QK + 213.0
            act_t = max(act_t, pe_t + 100.0) + 700.0

        av_pend = deque()
        exp_fin = []
        schr_n = 0
        v_pops = [0, 0, 0, 0]
        counts = [0] * NQ
        wp_loaded = [False]

        def wall():
            return max(pe_t, act_t - 2.0 * C_EXP_ACT, dve_t - 2.0 * C_EXP_DVE)

        def emit_qk(u):
            nonlocal pe_t, dve_t
            est = None
            for (e, uu) in qk_q:
                if uu == u:
                    est = e
                    break
            qk_q.remove((est, u))
            qk_unit(*u)
            qk_done.add(u)
            pe_t = max(pe_t, est) + C_QK
            dve_t = max(dve_t, pe_t + 100.0) + C_QK_EV
            if not qk_q and not wp_loaded[0]:
                wp_loaded[0] = True
                wp_load()

        def wp_load():
            wp_holder.append(shared.tile([P, 8, D], FP16, name="wp_sb"))
            for co in range(8):
                nc.sync.dma_start(wp_holder[0][:, co], wp[:, co])

        def emit_v():
            nonlocal pe_t, dve_t
            est, (eq, nt) = v_q.popleft()
            v_unit(eq, nt)
            v_pops[eq] += 1
            pe_t = max(pe_t, est) + C_VQ
            dve_t = max(dve_t, pe_t + 100.0) + C_VQ_EV

        def emit_pj():
            nonlocal pe_t, dve_t
            est, (qh0, nt, fh) = pj_q.popleft()
            a, b = pj_halves(qh0, nt, fh)
            a()
            b()
            pe_t = max(pe_t, est) + C_PJ
            dve_t = max(dve_t, pe_t + 100.0) + C_PJ_EV

        def pick_filler():
            # one slack unit, earliest-est first; False if nothing eligible
            cands = []
            if qk_q:
                cands.append((qk_q[0][0], 0))
            if v_q:
                cands.append((v_q[0][0], 1))
            if pj_q:
                cands.append((pj_q[0][0], 2))
            cands = [c for c in cands if c[0] <= wall() + 400.0]
            if not cands:
                return False
            cands.sort()
            kind = cands[0][1]
            if kind == 0:
                emit_qk(qk_q[0][1])
            elif kind == 1:
                emit_v()
            else:
                emit_pj()
            return True

        pending_tp = []

        def flush_tp():
            nonlocal pe_t, dve_t
            while pending_tp:
                qh0, co0, tp = pending_tp.pop(0)
                tp()
                pe_t += C_TP
                dve_t = max(dve_t, pe_t + 100.0) + C_TP_EV
                counts[qh0] += 1
                if counts[qh0] == 8:
                    est_pj = max(wall(), dve_t) + 4200.0
                    for nt in range(2):
                        for fh in range(2):
                            pj_q.append((est_pj, (qh0, nt, fh)))

        def av_ready():
            if not av_pend:
                return False
            qh0, co0, _ = av_pend[0]
            return v_pops[co0 // 2] >= 8

        def av_emit():
            nonlocal pe_t, dve_t
            flush_tp()
            qh0, co0, exps0 = av_pend.popleft()
            a, b, tp = av_halves(qh0, co0, exps0)
            a()
            pe_t += C_AVH
            b()
            pe_t += C_AVH
            dve_t = max(dve_t, pe_t + 100.0) + C_NORM
            pending_tp.append((qh0, co0, tp))

        FILL_TOTAL = 24 * C_QK + 32 * C_VQ + 32 * C_PJ / 2.0
        fill_pe = [0.0]

        def quota_fill(limit):
            n = 0
            while (
                n < limit
                and fill_pe[0] < (it + 1) * (FILL_TOTAL / 32.0)
            ):
                before = (len(qk_q), len(v_q), len(pj_q))
                if not pick_filler():
                    break
                after = (len(qk_q), len(v_q), len(pj_q))
                if before[0] != after[0]:
                    fill_pe[0] += C_QK
                elif before[1] != after[1]:
                    fill_pe[0] += C_VQ
                else:
                    fill_pe[0] += C_PJ
                n += 1

        BLOCKS = [(0, 0), (1, 0), (0, 4), (2, 0), (1, 4), (3, 0), (2, 4), (3, 4)]
        it = -1
        for (qh, co0_blk) in BLOCKS:
            for co in range(co0_blk, co0_blk + 4):
                it += 1
                keep = 3 if it < 29 else 1
                if av_ready() and len(av_pend) > keep:
                    av_emit()
                quota_fill(4)
                exps = {}
                for g in range(2):
                    for hh in range(2):
                        grp = co // 2
                        for u in [(grp, 0, qh // 2), (grp, 1, qh // 2),
                                  (4 + grp, 0, g), (4 + grp, 1, g)]:
                            if u not in qk_done:
                                emit_qk(u)
                        use_dve = (
                            schr_n < SCHR_MAX
                            and it >= 4
                            and act_t - dve_t > 1200.0
                        )
                        busy_t = dve_t if use_dve else act_t
                        ni = len(exp_fin)
                        cap = exp_fin[ni - 2] if ni >= 2 else 0.0
                        # fill PE while the psum cap blocks or the exp
                        # engine is well-fed
                        while True:
                            gate = max(pe_t, cap)
                            if busy_t - gate < TARGET_BL and pe_t >= cap - 100.0:
                                break
                            if av_ready() and len(av_pend) >= 2:
                                av_emit()
                            elif pick_filler():
                                pass
                            else:
                                break
                            busy_t = dve_t if use_dve else act_t
                        # exp-pool pressure: drain AVs (or the V units
                        # blocking them) before allocating another tile
                        while 4 * len(av_pend) + 6 > 34:
                            if av_ready():
                                av_emit()
                            elif v_q:
                                emit_v()
                            elif not pick_filler():
                                break
                        pe_t = max(pe_t, cap) + C_SC
                        ex = sc_group(qh, co, g, hh, use_dve)
                        flush_tp()
                        if use_dve:
                            schr_n += 1
                            st = max(dve_t, pe_t + 100.0)
                            dve_t = st + C_EXP_DVE
                            exp_fin.append(dve_t)
                        else:
                            st = max(act_t, pe_t + 100.0)
                            act_t = st + C_EXP_ACT
                            exp_fin.append(act_t)
                        exps[(hh, g)] = ex
                av_pend.append((qh, co, exps))
        while av_pend:
            if not av_ready():
                emit_v()
                continue
            av_emit()
        flush_tp()
        while qk_q:
            emit_qk(qk_q[0][1])
        while v_q:
            emit_v()
        while pj_q:
            emit_pj()


def make_in_maps(x, c, kv_w, kv_b, shared_q_w, shared_q_b, cohort_q_w, cohort_q_b,
                 proj_w, proj_b):
    f32 = np.float32
    fp16 = np.float16
    fp8 = mybir.dt.np(FP8)
    x = np.asarray(x, dtype=f32)
    c = np.asarray(c).astype(np.int64)
    kv_w = np.asarray(kv_w, dtype=f32)
    kv_b = np.asarray(kv_b, dtype=f32)
    shared_q_w = np.asarray(shared_q_w, dtype=f32)
    shared_q_b = np.asarray(shared_q_b, dtype=f32)
    cohort_q_w = np.asarray(cohort_q_w, dtype=f32)
    cohort_q_b = np.asarray(cohort_q_b, dtype=f32)
    proj_w = np.asarray(proj_w, dtype=f32)
    proj_b = np.asarray(proj_b, dtype=f32)

    wk = kv_w[:D] * WS
    wv_ = kv_w[D:]
    bk = kv_b[:D] * WS
    bv_ = kv_b[D:]

    wv_h = np.ascontiguousarray(
        wv_.T.reshape(8, P, D).transpose(1, 0, 2)
    ).astype(fp16)
    wp_h = np.ascontiguousarray(
        proj_w.T.reshape(8, P, D).transpose(1, 0, 2)
    ).astype(fp16)

    in_maps = []
    for b in range(x.shape[0]):
        wq = np.concatenate([shared_q_w, cohort_q_w[c[b]]], axis=0) * WS
        bq = np.concatenate([shared_q_b, cohort_q_b[c[b]]], axis=0) * WS
        wqk_cols = np.concatenate([wq, wk], axis=0)     # [2048 e, 1024 d]
        # e = qk*1024 + head*64 + j*32 + i with head = 4*g4 + hh;
        # device wants [p, g(qk,g4), j, t2, dj, ec(hh,i)]
        wqk_e = wqk_cols.reshape(2, 4, 4, 2, 32, D)   # [qk, g4, hh, j, i, d]
        wqk_e = wqk_e.transpose(0, 1, 3, 2, 4, 5).reshape(8, 2, P, D)
        wqk_full = wqk_e.reshape(8, 2, P, 4, 2, P)    # [g, j, ec, t2, dj, p]
        wqk_h = np.ascontiguousarray(
            wqk_full.transpose(5, 0, 1, 3, 4, 2)
        ).astype(fp8)
        bqk_e = np.concatenate([bq, bk]).reshape(2, 4, 4, 2, 32)
        bqk_h = np.ascontiguousarray(
            bqk_e.transpose(0, 1, 3, 2, 4).reshape(8, 2, P).transpose(2, 0, 1)
        ).astype(f32)
        bqk8_h = np.ascontiguousarray(
            bqk_e.transpose(0, 1, 3, 2, 4).reshape(8, 2, P)[None]
        ).astype(fp8)
        xt_h = np.ascontiguousarray(
            x[b].T.reshape(8, P, N).transpose(1, 0, 2)
        ).astype(fp16)
        xdr_h = np.ascontiguousarray(
            x[b].T.reshape(4, 2, P, N).transpose(2, 0, 1, 3)
        ).astype(fp8)
        m = {
            "ident": np.eye(P, dtype=fp16),
            "bqk8": bqk8_h,
            "xdr": xdr_h,
            "wqk": wqk_h,
            "bqk": bqk_h,
            "xt": xt_h,
            "wv": wv_h,
            "bv": np.ascontiguousarray(bv_).astype(mybir.dt.np(BF16)),
            "wp": wp_h,
            "bp": np.ascontiguousarray(proj_b).astype(mybir.dt.np(BF16)),
        }
        in_maps.append(m)
    return in_maps


_NC_CACHE = {}


def kernel(**inputs) -> np.ndarray:
    in_maps = make_in_maps(**inputs)
    if "nc" not in _NC_CACHE:
        _NC_CACHE["nc"] = build_nc()
    nc = _NC_CACHE["nc"]
    res = run_bass_kernel_spmd(nc, in_maps, core_ids=list(range(NCORES)))
    out = np.stack([res.results[i]["out"] for i in range(NCORES)], axis=0)
    return out.astype(np.float32)


# revision 67
# speedup vs baseline: 1.0300x; 1.0017x over previous
"""CohortAwareBlock Trainium2 kernel.

Data-parallel over batch B=8 across 8 NeuronCores (one sample per core).
Cohort routing (gather of cohort_q_w by per-sample cohort id) happens on the
host while building each core's weight tensors; the device kernel is a plain
attention block.

Numerics:
  - QK-gen runs as fp8-e4m3 DoubleRow matmuls (weights pre-scaled x32 to
    dodge fp8 subnormals; the inverse scale is folded into the exp scale).
  - q/k are stored as fp8 in a DoubleRow-interleaved layout ([32, 2, N] per
    head, 4 heads stacked across 128 partitions at 32-partition tile
    positions) so the scores matmul also runs fp8-DR: 2x fewer PE cycles
    than fp16 scores.
  - exp splits across the ACT engine (exact table exp, fp16 out) and the
    DVE (Schraudolph bit-trick: int16(A*s + B) written through a bitcast
    view and read back as fp16; ~1.8% rms sawtooth error, SCHR_MAX-capped
    for the error budget) so the exp stream is not ACT-bound.
  - v / attn weights / projection stay fp16.

Per-core structure:
  q4k4 [128, 8, 2, N] fp8  (4 q-head groups + 4 k-head groups, DR layout)
  v_aug [keys, h, 65] fp16 (col 64 = 1.0 so the flipped AV emits the
                            softmax denominator per q-partition)
  per (q-quarter, head pair):
    scores -> 2-bank PSUM [128, 4, 256] via fp8-DR -> exp (ACT or DVE,
    routed by backlog) ->
    flipped attn@v: av psum [128, 2, 2, 65]; col 64 = den ->
    batched DVE reciprocal + broadcast mult -> nm fp16 ->
    PE-array transpose (vs identity) + DVE evac -> nmT [d, q] ->
    proj (fp16) + bias -> out DMA on the idle GPSIMD queue

Scheduling: a priority scheduler with virtual engine clocks emits score
groups as fast as the sc_ps double-buffer allows (the exp stream is the
critical path), drains attn@v pairs as the preferred PE slack-filler, and
paces QK/V/proj units from per-kind queues (quota per iteration, popped
on demand for data dependencies) so no slow unit head-of-line-blocks the
in-order PE queue. The first QK units evac via ACT-Copy (bias folded in as
a ones-row matmul) while ACT is otherwise idle during startup; dummy
warmup matmuls keep the PE p-state ramped until the first real work; the
iteration visits head pairs in a block order that staggers the V-gen
deadlines.
"""

import numpy as np

import concourse.bass as bass
import concourse.bacc as bacc
import concourse.mybir as mybir
import concourse.tile as tile
from concourse.bass_utils import run_bass_kernel_spmd

P = 128
N = 1024            # sequence length
D = 1024            # model dim
H = 16              # heads
HD = 64             # head dim
NQ = 4              # q-quarters (256 q each)
QW = N // NQ        # 256
SCALE = HD ** -0.5
NCORES = 8

WS = 32.0           # fp8 pre-scale on w_q/w_k (and so on q/k values)
EXP_SCALE = SCALE / (WS * WS)

# Schraudolph fp16-bitcast exp on DVE: y_bits = int16(s * A + B); bits read
# as fp16 give exp(s*EXP_SCALE) with ~1.8% rms sawtooth error.
LOG2E = 1.4426950408889634
SCHR_A = EXP_SCALE * LOG2E * 1024.0
SCHR_B = 15301.0
SCHR_MAX = 27        # max exp groups routed to DVE (of 128); error budget cap

F32 = mybir.dt.float32
FP16 = mybir.dt.float16
BF16 = mybir.dt.bfloat16
FP8 = mybir.dt.float8e4
I16 = mybir.dt.int16
DR = mybir.MatmulPerfMode.DoubleRow
EXP = mybir.ActivationFunctionType.Exp
MUL = mybir.AluOpType.mult
ADD = mybir.AluOpType.add


def build_nc():
    nc = bacc.Bacc(
        "TRN2",
        target_bir_lowering=False,
        debug=False,
        num_devices=NCORES,
    )

    # ---- external I/O (per-core shards, host-prepped layouts) ----
    # DoubleRow-interleaved d-dim: d = (t2*2 + dj)*128 + p
    xdr = nc.dram_tensor("xdr", [P, 4, 2, N], FP8, kind="ExternalInput")
    # wqk[p, g, j, t2, dj, ec]: g = 4-head group (0..3 q, 4..7 k); j = d-half
    # of the head (e_local = j*32 + i); ec = hh*32 + i -> head 4*(g%4)+hh.
    wqk = nc.dram_tensor("wqk", [P, 8, 2, 4, 2, P], FP8, kind="ExternalInput")
    bqk = nc.dram_tensor("bqk", [P, 8, 2], F32, kind="ExternalInput")
    xt = nc.dram_tensor("xt", [P, 8, N], FP16, kind="ExternalInput")   # x^T
    wv = nc.dram_tensor("wv", [P, 8, D], FP16, kind="ExternalInput")
    bv = nc.dram_tensor("bv", [D], BF16, kind="ExternalInput")
    wp = nc.dram_tensor("wp", [P, 8, D], FP16, kind="ExternalInput")
    bp = nc.dram_tensor("bp", [D], BF16, kind="ExternalInput")
    ident = nc.dram_tensor("ident", [P, P], FP16, kind="ExternalInput")
    bqk8 = nc.dram_tensor("bqk8", [1, 8, 2, P], FP8, kind="ExternalInput")
    out = nc.dram_tensor("out", [N, D], F32, kind="ExternalOutput")

    with tile.TileContext(nc) as tc:
        kernel_body(tc, xdr, wqk, bqk, xt, wv, bv, wp, bp, ident, bqk8, out)
    nc.compile()
    return nc


def kernel_body(tc, xdr, wqk, bqk, xt, wv, bv, wp, bp, ident, bqk8, out):
    nc = tc.nc
    from contextlib import ExitStack

    with ExitStack() as ctx:
        ctx.enter_context(
            nc.allow_low_precision(reason="fp16/fp8 matmul inputs by design")
        )
        res = ctx.enter_context(tc.tile_pool(name="res", bufs=1))
        shared = ctx.enter_context(tc.tile_pool(name="shared", bufs=1))
        gen_ps = ctx.enter_context(tc.tile_pool(name="gen_ps", bufs=2, space="PSUM"))
        av_ps = ctx.enter_context(tc.tile_pool(name="av_ps", bufs=2, space="PSUM"))
        sc_ps = ctx.enter_context(tc.tile_pool(name="sc_ps", bufs=2, space="PSUM"))
        exp_pool = ctx.enter_context(tc.tile_pool(name="exp_pool", bufs=35))
        rc_pool = ctx.enter_context(tc.tile_pool(name="rc_pool", bufs=4))
        nm_pool = ctx.enter_context(tc.tile_pool(name="nm_pool", bufs=4))
        oev_pool = ctx.enter_context(tc.tile_pool(name="oev_pool", bufs=3))

        # ---- resident tiles ----
        warm = res.tile([1, 513], FP16)
        nc.gpsimd.memset(warm[:], 1.0)

        xdr_sb = res.tile([P, 4, 2, N], FP8)
        wqk_sb = shared.tile([P, 8, 2, 4, 2, P], FP8, name="wqk_sb")
        bqk_sb = res.tile([P, 8, 2], F32)
        # q/k in scores-DR layout: group g (0..3 q, 4..7 k), partition
        # (hh*32+i), j, token -> value of head 4*(g%4)+hh, d = j*32+i
        q4k4 = res.tile([P, 8, 2, N], FP8)
        xt_sb = res.tile([P, 8, N], FP16)
        wv_sb = res.tile([P, 8, D], FP16)
        bv_rep = res.tile([P, D], BF16)
        wp_holder = []   # allocated from `shared` after QK-gen is emitted
        bp_rep = res.tile([P, D], BF16)

        # v_aug[p, nt, h, :]: cols 0:64 = v for head h at key chunk nt,
        # col 64 = 1.0 (flipped attn@v then emits the softmax denominator
        # in output column 64, one value per q-partition)
        v_aug = res.tile([P, 8, H, HD + 1], FP16)
        nc.gpsimd.memset(v_aug[:, :, :, HD : HD + 1], 1.0)

        # transposed normalized att, packed for proj: [d-part, qc, co, q]
        nmT = res.tile([P, 8, 8, P], FP16)
        ident_sb = res.tile([P, P], FP16)
        bqk8_sb = res.tile([1, 8, 2, P], FP8)
        ones8 = res.tile([1, 512], FP8)
        nc.gpsimd.memset(ones8[:], 1.0)

        # ---- input DMAs (sync queue, need-order; wp follows in the
        # filler queue, reusing wqk's SBUF once QK-gen is done) ----
        nc.sync.dma_start(xdr_sb[:], xdr[:])
        for g in (0, 4):
            for j in range(2):
                nc.sync.dma_start(wqk_sb[:, g, j], wqk[:, g, j])
        nc.sync.dma_start(bqk8_sb[:], bqk8[:])
        nc.sync.dma_start(bqk_sb[:], bqk[:])
        for g in (1, 5):
            for j in range(2):
                nc.sync.dma_start(wqk_sb[:, g, j], wqk[:, g, j])
        for dc in range(8):
            nc.sync.dma_start(wv_sb[:, dc], wv[:, dc])
        nc.sync.dma_start(bv_rep[:], bv[None, :].to_broadcast([P, D]))
        for dc in range(8):
            nc.sync.dma_start(xt_sb[:, dc], xt[:, dc])
        for g in (2, 6, 3, 7):
            for j in range(2):
                nc.sync.dma_start(wqk_sb[:, g, j], wqk[:, g, j])
        nc.sync.dma_start(bp_rep[:], bp[None, :].to_broadcast([P, D]))
        nc.sync.dma_start(ident_sb[:], ident[:])

        # ---------------- emission helpers ----------------
        def warmup():
            # keep the PE p-state ramped while input DMAs land
            ps = gen_ps.tile([P, 512], F32, tag="gps", name="gps")
            nc.tensor.matmul(
                ps[0:1, :],
                lhsT=warm[:, 512:513],
                rhs=warm[:, 0:512],
                start=True,
                stop=True,
            )

        def qk_unit(g, j, ch, act_evac=False):
            # one QK-gen psum group: 4 fp8-DR matmuls + biased fp8 evac into
            # the scores-DR layout (GPSIMD cannot read PSUM, so evac on DVE;
            # the first units evac via ACT-Copy instead -- ACT idles during
            # startup -- with the bias folded in as a ones-row matmul)
            ps = gen_ps.tile([P, 512], F32, tag="gps", name="gps")
            for t2 in range(4):
                nc.tensor.matmul(
                    ps[:],
                    lhsT=wqk_sb[:, g, j, t2],
                    rhs=xdr_sb[:, t2, :, ch * 512 : (ch + 1) * 512],
                    start=(t2 == 0),
                    stop=(t2 == 3) and not act_evac,
                    perf_mode=DR,
                )
            if act_evac:
                nc.tensor.matmul(
                    ps[:],
                    lhsT=bqk8_sb[:, g, j],
                    rhs=ones8[:],
                    start=False,
                    stop=True,
                )
                nc.scalar.activation(
                    q4k4[:, g, j, ch * 512 : (ch + 1) * 512],
                    ps[:],
                    mybir.ActivationFunctionType.Copy,
                )
            else:
                nc.vector.tensor_scalar_add(
                    q4k4[:, g, j, ch * 512 : (ch + 1) * 512],
                    ps[:],
                    bqk_sb[:, g, j : j + 1],
                )

        def v_unit(eq, nt):
            # v[keys nt-chunk, 256 cols (4 heads) of quarter eq]: one psum
            # group + evac; quarter granularity staggers the AV deadlines
            ps = gen_ps.tile([P, 256], F32, tag="gps", name="gps")
            for dc in range(8):
                nc.tensor.matmul(
                    ps[:],
                    lhsT=xt_sb[:, dc, nt * P : (nt + 1) * P],
                    rhs=wv_sb[:, dc, eq * 256 : (eq + 1) * 256],
                    start=(dc == 0),
                    stop=(dc == 7),
                )
            nc.vector.tensor_add(
                v_aug[:, nt, eq * 4 : (eq + 1) * 4, 0:HD],
                ps[:].rearrange("p (h d) -> p h d", d=HD),
                bv_rep[:, eq * 256 : (eq + 1) * 256].rearrange(
                    "p (h d) -> p h d", d=HD
                ),
            )

        def sc_group(qh, co, g, hh, use_dve):
            # one kt-group of scores (fp8-DR) + its batched exp (ACT exact
            # or DVE Schraudolph); returns the exp tile
            h = 2 * co + hh
            grp = h // 4
            r = 32 * (h % 4)
            q0 = qh * QW
            ps = sc_ps.tile([P, 4, QW], F32, tag="scps", name="scps")
            for ki in range(4):
                kt = g * 4 + ki
                nc.tensor.matmul(
                    ps[:, ki],
                    lhsT=q4k4[r : r + 32, 4 + grp, :, kt * P : (kt + 1) * P],
                    rhs=q4k4[r : r + 32, grp, :, q0 : q0 + QW],
                    start=True,
                    stop=True,
                    perf_mode=DR,
                    tile_position=(r, 0),
                )
            ex = exp_pool.tile([P, 4, QW], FP16, tag="exp", name="exp")
            if use_dve:
                nc.vector.tensor_scalar(
                    ex[:].bitcast(I16),
                    ps[:],
                    SCHR_A,
                    SCHR_B,
                    op0=MUL,
                    op1=ADD,
                )
            else:
                nc.scalar.activation(ex[:], ps[:], EXP, scale=EXP_SCALE)
            return ex

        def av_halves(qh, co, exps):
            # flipped attn@v for one head pair, split per head; the batched
            # norm runs after the second half; the nm -> nmT transpose is a
            # separate unit (PE-array transpose + DVE evac) emitted later
            hold = []

            def half(hh):
                h = 2 * co + hh
                if hh == 0:
                    t = av_ps.tile([P, 392], F32, tag="avps", name="avps")
                    hold.append(t)
                ps = hold[0][:, 0:260].rearrange(
                    "p (a b c) -> p a b c", a=2, b=2
                )
                for qs in range(2):
                    for kt in range(8):
                        nc.tensor.matmul(
                            ps[:, qs, hh],
                            lhsT=exps[(hh, kt // 4)][:, kt % 4,
                                                     qs * P : (qs + 1) * P],
                            rhs=v_aug[:, kt, h, :],
                            start=(kt == 0),
                            stop=(kt == 7),
                        )
                if hh == 1:
                    rc = rc_pool.tile([P, 2, 2, 1], F32, tag="rc", name="rc")
                    nc.vector.reciprocal(rc[:], ps[:, :, :, HD : HD + 1])
                    nm = nm_pool.tile([P, 2, 2, HD], FP16, tag="nm", name="nm")
                    nc.vector.tensor_tensor(
                        nm[:],
                        ps[:, :, :, 0:HD],
                        rc[:].broadcast_to([P, 2, 2, HD]),
                        op=MUL,
                    )
                    hold.append(nm)

            def tp():
                # PE-array transpose of nm into proj layout + DVE evac;
                # keeps the nmT chain off the slow DMA queues
                t, nm = hold
                tpv = t[:, 264:392].bitcast(FP16).rearrange(
                    "p (a q) -> p a q", a=2
                )
                for qs in range(2):
                    nc.tensor.transpose(
                        tpv[:, qs], nm[:, qs], ident_sb[:]
                    )
                nc.vector.tensor_copy(
                    nmT[:, qh * 2 : qh * 2 + 2, co, :], tpv[:]
                )

            return (lambda: half(0)), (lambda: half(1)), tp

        def pj_halves(qh, nt, fh):
            # one projection output group split into two PE units
            qc = qh * 2 + nt
            n0 = qc * P
            hold = []

            def a():
                ps = gen_ps.tile([P, 512], F32, tag="gps", name="gps")
                hold.append(ps)
                for co in range(4):
                    nc.tensor.matmul(
                        ps[:],
                        lhsT=nmT[:, qc, co, :],
                        rhs=wp_holder[0][:, co, fh * 512 : (fh + 1) * 512],
                        start=(co == 0),
                        stop=False,
                    )

            def b():
                ps = hold[0]
                for co in range(4, 8):
                    nc.tensor.matmul(
                        ps[:],
                        lhsT=nmT[:, qc, co, :],
                        rhs=wp_holder[0][:, co, fh * 512 : (fh + 1) * 512],
                        start=False,
                        stop=(co == 7),
                    )
                ev = oev_pool.tile([P, 512], F32, tag="oev", name="oev")
                nc.vector.tensor_add(
                    ev[:], ps[:], bp_rep[:, fh * 512 : (fh + 1) * 512]
                )
                nc.sync.dma_start(
                    out[n0 : n0 + P, fh * 512 : (fh + 1) * 512], ev[:]
                )

            return a, b

        # ---------------- schedule ----------------
        # Priority scheduler with virtual engine clocks (pe/act/dve busy-until
        # estimates under the cost model). The exp stream (ACT + DVE
        # Schraudolph, routed by backlog) is the critical path; score groups
        # are emitted as fast as the sc_ps double-buffer allows. AV pairs and
        # QK/V/proj units fill PE slack one unit at a time from per-kind
        # queues, so a unit needed soon never forces a burst-drain of
        # unrelated work (which would starve the exp engines).
        from collections import deque

        C_SC = 220.0          # score group PE (4 fp8-DR matmuls)
        C_EXP_ACT = 1110.0
        C_EXP_DVE = 1280.0
        C_AVH = 440.0         # AV half PE
        C_NORM = 800.0        # batched recip+mult DVE
        C_QK = 430.0          # QK unit PE
        C_QK_EV = 750.0       # QK evac DVE
        C_VQ = 858.0          # V quarter-unit PE
        C_VQ_EV = 485.0       # V quarter evac DVE
        C_PJ = 1704.0         # proj unit PE (a+b)
        C_PJ_EV = 705.0       # proj evac DVE
        C_TP = 115.0          # nm transpose PE (2 PE-array transposes)
        C_TP_EV = 320.0       # nmT evac DVE
        TARGET_BL = 3600.0

        for _ in range(14):
            warmup()

        # QK units: first 4 unblock (qh0, co0, g0) scores; the rest are
        # popped on demand (per-unit) or as slack fillers
        qk_first = [(0, 0, 0), (0, 1, 0), (4, 0, 0), (4, 1, 0)]
        qk_q = deque()
        for (g, j, ch, est) in [(4, 0, 1, 5200.0), (4, 1, 1, 5600.0),
                                (0, 0, 1, 6000.0), (0, 1, 1, 6400.0)]:
            qk_q.append((est, (g, j, ch)))
        for grp, est in ((1, 7800.0), (2, 20700.0), (3, 22100.0)):
            for (g, ch) in ((grp, 0), (4 + grp, 0), (4 + grp, 1), (grp, 1)):
                for j in range(2):
                    qk_q.append((est, (g, j, ch)))
        v_q = deque()
        for eq in range(4):
            for nt in range(8):
                v_q.append((19300.0, (eq, nt)))
        pj_q = deque()

        pe_t = 6300.0
        act_t = 0.0
        dve_t = 0.0
        qk_done = set(qk_first)
        for i, u in enumerate(qk_first):
            # evacs split across ACT and DVE so they drain in parallel
            # during startup (both engines are otherwise idle)
            if i < 2:
                qk_unit(*u, act_evac=True)
                pe_t += C_QK + 213.0
                act_t = max(act_t, pe_t + 100.0) + 700.0
            else:
                qk_unit(*u)
                pe_t += C_QK
                dve_t = max(dve_t, pe_t + 100.0) + C_QK_EV

        av_pend = deque()
        exp_fin = []
        schr_n = 0
        v_pops = [0, 0, 0, 0]
        counts = [0] * NQ
        wp_loaded = [False]

        def wall():
            return max(pe_t, act_t - 2.0 * C_EXP_ACT, dve_t - 2.0 * C_EXP_DVE)

        def emit_qk(u):
            nonlocal pe_t, dve_t
            est = None
            for (e, uu) in qk_q:
                if uu == u:
                    est = e
                    break
            qk_q.remove((est, u))
            qk_unit(*u)
            qk_done.add(u)
            pe_t = max(pe_t, est) + C_QK
            dve_t = max(dve_t, pe_t + 100.0) + C_QK_EV
            if not qk_q and not wp_loaded[0]:
                wp_loaded[0] = True
                wp_load()

        def wp_load():
            wp_holder.append(shared.tile([P, 8, D], FP16, name="wp_sb"))
            for co in range(8):
                nc.sync.dma_start(wp_holder[0][:, co], wp[:, co])

        def emit_v():
            nonlocal pe_t, dve_t
            est, (eq, nt) = v_q.popleft()
            v_unit(eq, nt)
            v_pops[eq] += 1
            pe_t = max(pe_t, est) + C_VQ
            dve_t = max(dve_t, pe_t + 100.0) + C_VQ_EV

        def emit_pj():
            nonlocal pe_t, dve_t
            est, (qh0, nt, fh) = pj_q.popleft()
            a, b = pj_halves(qh0, nt, fh)
            a()
            b()
            pe_t = max(pe_t, est) + C_PJ
            dve_t = max(dve_t, pe_t + 100.0) + C_PJ_EV

        def pick_filler():
            # one slack unit, earliest-est first; False if nothing eligible
            cands = []
            if qk_q:
                cands.append((qk_q[0][0], 0))
            if v_q:
                cands.append((v_q[0][0], 1))
            if pj_q:
                cands.append((pj_q[0][0], 2))
            cands = [c for c in cands if c[0] <= wall() + 400.0]
            if not cands:
                return False
            cands.sort()
            kind = cands[0][1]
            if kind == 0:
                emit_qk(qk_q[0][1])
            elif kind == 1:
                emit_v()
            else:
                emit_pj()
            return True

        pending_tp = []

        def flush_tp():
            nonlocal pe_t, dve_t
            while pending_tp:
                qh0, co0, tp = pending_tp.pop(0)
                tp()
                pe_t += C_TP
                dve_t = max(dve_t, pe_t + 100.0) + C_TP_EV
                counts[qh0] += 1
                if counts[qh0] == 8:
                    est_pj = max(wall(), dve_t) + 4200.0
                    for nt in range(2):
                        for fh in range(2):
                            pj_q.append((est_pj, (qh0, nt, fh)))

        def av_ready():
            if not av_pend:
                return False
            qh0, co0, _ = av_pend[0]
            return v_pops[co0 // 2] >= 8

        def av_emit():
            nonlocal pe_t, dve_t
            flush_tp()
            qh0, co0, exps0 = av_pend.popleft()
            a, b, tp = av_halves(qh0, co0, exps0)
            a()
            pe_t += C_AVH
            b()
            pe_t += C_AVH
            dve_t = max(dve_t, pe_t + 100.0) + C_NORM
            pending_tp.append((qh0, co0, tp))

        FILL_TOTAL = 24 * C_QK + 32 * C_VQ + 32 * C_PJ / 2.0
        fill_pe = [0.0]

        def quota_fill(limit):
            n = 0
            while (
                n < limit
                and fill_pe[0] < (it + 1) * (FILL_TOTAL / 32.0)
            ):
                before = (len(qk_q), len(v_q), len(pj_q))
                if not pick_filler():
                    break
                after = (len(qk_q), len(v_q), len(pj_q))
                if before[0] != after[0]:
                    fill_pe[0] += C_QK
                elif before[1] != after[1]:
                    fill_pe[0] += C_VQ
                else:
                    fill_pe[0] += C_PJ
                n += 1

        BLOCKS = [(0, 0), (1, 0), (0, 4), (2, 0), (1, 4), (3, 0), (2, 4), (3, 4)]
        it = -1
        for (qh, co0_blk) in BLOCKS:
            for co in range(co0_blk, co0_blk + 4):
                it += 1
                keep = 3 if it < 29 else 1
                if av_ready() and len(av_pend) > keep:
                    av_emit()
                quota_fill(4)
                exps = {}
                for g in range(2):
                    for hh in range(2):
                        grp = co // 2
                        for u in [(grp, 0, qh // 2), (grp, 1, qh // 2),
                                  (4 + grp, 0, g), (4 + grp, 1, g)]:
                            if u not in qk_done:
                                emit_qk(u)
                        use_dve = (
                            schr_n < SCHR_MAX
                            and it >= 4
                            and act_t - dve_t > 1200.0
                        )
                        busy_t = dve_t if use_dve else act_t
                        ni = len(exp_fin)
                        cap = exp_fin[ni - 2] if ni >= 2 else 0.0
                        # fill PE while the psum cap blocks or the exp
                        # engine is well-fed
                        while True:
                            gate = max(pe_t, cap)
                            if busy_t - gate < TARGET_BL and pe_t >= cap - 100.0:
                                break
                            if av_ready() and len(av_pend) >= 2:
                                av_emit()
                            elif pick_filler():
                                pass
                            else:
                                break
                            busy_t = dve_t if use_dve else act_t
                        # exp-pool pressure: drain AVs (or the V units
                        # blocking them) before allocating another tile
                        while 4 * len(av_pend) + 6 > 33:
                            if av_ready():
                                av_emit()
                            elif v_q:
                                emit_v()
                            elif not pick_filler():
                                break
                        pe_t = max(pe_t, cap) + C_SC
                        ex = sc_group(qh, co, g, hh, use_dve)
                        flush_tp()
                        if use_dve:
                            schr_n += 1
                            st = max(dve_t, pe_t + 100.0)
                            dve_t = st + C_EXP_DVE
                            exp_fin.append(dve_t)
                        else:
                            st = max(act_t, pe_t + 100.0)
                            act_t = st + C_EXP_ACT
                            exp_fin.append(act_t)
                        exps[(hh, g)] = ex
                av_pend.append((qh, co, exps))
        while av_pend:
            if not av_ready():
                emit_v()
                continue
            av_emit()
        flush_tp()
        while qk_q:
            emit_qk(qk_q[0][1])
        while v_q:
            emit_v()
        while pj_q:
            emit_pj()


def make_in_maps(x, c, kv_w, kv_b, shared_q_w, shared_q_b, cohort_q_w, cohort_q_b,
                 proj_w, proj_b):
    f32 = np.float32
    fp16 = np.float16
    fp8 = mybir.dt.np(FP8)
    x = np.asarray(x, dtype=f32)
    c = np.asarray(c).astype(np.int64)
    kv_w = np.asarray(kv_w, dtype=f32)
    kv_b = np.asarray(kv_b, dtype=f32)
    shared_q_w = np.asarray(shared_q_w, dtype=f32)
    shared_q_b = np.asarray(shared_q_b, dtype=f32)
    cohort_q_w = np.asarray(cohort_q_w, dtype=f32)
    cohort_q_b = np.asarray(cohort_q_b, dtype=f32)
    proj_w = np.asarray(proj_w, dtype=f32)
    proj_b = np.asarray(proj_b, dtype=f32)

    wk = kv_w[:D] * WS
    wv_ = kv_w[D:]
    bk = kv_b[:D] * WS
    bv_ = kv_b[D:]

    wv_h = np.ascontiguousarray(
        wv_.T.reshape(8, P, D).transpose(1, 0, 2)
    ).astype(fp16)
    wp_h = np.ascontiguousarray(
        proj_w.T.reshape(8, P, D).transpose(1, 0, 2)
    ).astype(fp16)

    in_maps = []
    for b in range(x.shape[0]):
        wq = np.concatenate([shared_q_w, cohort_q_w[c[b]]], axis=0) * WS
        bq = np.concatenate([shared_q_b, cohort_q_b[c[b]]], axis=0) * WS
        wqk_cols = np.concatenate([wq, wk], axis=0)     # [2048 e, 1024 d]
        # e = qk*1024 + head*64 + j*32 + i with head = 4*g4 + hh;
        # device wants [p, g(qk,g4), j, t2, dj, ec(hh,i)]
        wqk_e = wqk_cols.reshape(2, 4, 4, 2, 32, D)   # [qk, g4, hh, j, i, d]
        wqk_e = wqk_e.transpose(0, 1, 3, 2, 4, 5).reshape(8, 2, P, D)
        wqk_full = wqk_e.reshape(8, 2, P, 4, 2, P)    # [g, j, ec, t2, dj, p]
        wqk_h = np.ascontiguousarray(
            wqk_full.transpose(5, 0, 1, 3, 4, 2)
        ).astype(fp8)
        bqk_e = np.concatenate([bq, bk]).reshape(2, 4, 4, 2, 32)
        bqk_h = np.ascontiguousarray(
            bqk_e.transpose(0, 1, 3, 2, 4).reshape(8, 2, P).transpose(2, 0, 1)
        ).astype(f32)
        bqk8_h = np.ascontiguousarray(
            bqk_e.transpose(0, 1, 3, 2, 4).reshape(8, 2, P)[None]
        ).astype(fp8)
        xt_h = np.ascontiguousarray(
            x[b].T.reshape(8, P, N).transpose(1, 0, 2)
        ).astype(fp16)
        xdr_h = np.ascontiguousarray(
            x[b].T.reshape(4, 2, P, N).transpose(2, 0, 1, 3)
        ).astype(fp8)
        m = {
            "ident": np.eye(P, dtype=fp16),
            "bqk8": bqk8_h,
            "xdr": xdr_h,
            "wqk": wqk_h,
            "bqk": bqk_h,
            "xt": xt_h,
            "wv": wv_h,
            "bv": np.ascontiguousarray(bv_).astype(mybir.dt.np(BF16)),
            "wp": wp_h,
            "bp": np.ascontiguousarray(proj_b).astype(mybir.dt.np(BF16)),
        }
        in_maps.append(m)
    return in_maps


_NC_CACHE = {}


def kernel(**inputs) -> np.ndarray:
    in_maps = make_in_maps(**inputs)
    if "nc" not in _NC_CACHE:
        _NC_CACHE["nc"] = build_nc()
    nc = _NC_CACHE["nc"]
    res = run_bass_kernel_spmd(nc, in_maps, core_ids=list(range(NCORES)))
    out = np.stack([res.results[i]["out"] for i in range(NCORES)], axis=0)
    return out.astype(np.float32)


# revision 68
# speedup vs baseline: 1.0644x; 1.0335x over previous
"""CohortAwareBlock Trainium2 kernel.

Data-parallel over batch B=8 across 8 NeuronCores (one sample per core).
Cohort routing (gather of cohort_q_w by per-sample cohort id) happens on the
host while building each core's weight tensors; the device kernel is a plain
attention block.

Numerics:
  - QK-gen runs as fp8-e4m3 DoubleRow matmuls (weights pre-scaled x32 to
    dodge fp8 subnormals; the inverse scale is folded into the exp scale).
  - q/k are stored as fp8 in a DoubleRow-interleaved layout ([32, 2, N] per
    head, 4 heads stacked across 128 partitions at 32-partition tile
    positions) so the scores matmul also runs fp8-DR: 2x fewer PE cycles
    than fp16 scores.
  - exp splits across the ACT engine (exact table exp, fp16 out) and the
    DVE (Schraudolph bit-trick: int16(A*s + B) written through a bitcast
    view and read back as fp16; ~1.8% rms sawtooth error, SCHR_MAX-capped
    for the error budget) so the exp stream is not ACT-bound.
  - v / attn weights / projection stay fp16.

Per-core structure:
  q4k4 [128, 8, 2, N] fp8  (4 q-head groups + 4 k-head groups, DR layout)
  v_aug [keys, h, 65] fp16 (col 64 = 1.0 so the flipped AV emits the
                            softmax denominator per q-partition)
  per (q-quarter, head pair):
    scores -> 2-bank PSUM [128, 4, 256] via fp8-DR -> exp (ACT or DVE,
    routed by backlog) ->
    flipped attn@v: av psum [128, 2, 2, 65]; col 64 = den ->
    batched DVE reciprocal + broadcast mult -> nm fp16 ->
    PE-array transpose (vs identity) + DVE evac -> nmT [d, q] ->
    proj (fp16) + bias -> out DMA on the idle GPSIMD queue

Scheduling: a priority scheduler with virtual engine clocks emits score
groups as fast as the sc_ps double-buffer allows (the exp stream is the
critical path), drains attn@v pairs as the preferred PE slack-filler, and
paces QK/V/proj units from per-kind queues (quota per iteration, popped
on demand for data dependencies) so no slow unit head-of-line-blocks the
in-order PE queue. The first QK units evac via ACT-Copy (bias folded in as
a ones-row matmul) while ACT is otherwise idle during startup; dummy
warmup matmuls keep the PE p-state ramped until the first real work; the
iteration visits head pairs in a block order that staggers the V-gen
deadlines.
"""

import numpy as np

import concourse.bass as bass
import concourse.bacc as bacc
import concourse.mybir as mybir
import concourse.tile as tile
from concourse.bass_utils import run_bass_kernel_spmd

P = 128
N = 1024            # sequence length
D = 1024            # model dim
H = 16              # heads
HD = 64             # head dim
NQ = 4              # q-quarters (256 q each)
QW = N // NQ        # 256
SCALE = HD ** -0.5
NCORES = 8

WS = 32.0           # fp8 pre-scale on w_q/w_k (and so on q/k values)
EXP_SCALE = SCALE / (WS * WS)

# Schraudolph fp16-bitcast exp on DVE: y_bits = int16(s * A + B); bits read
# as fp16 give exp(s*EXP_SCALE) with ~1.8% rms sawtooth error.
LOG2E = 1.4426950408889634
SCHR_A = EXP_SCALE * LOG2E * 1024.0
SCHR_B = 15301.0
SCHR_MAX = 27        # max exp groups routed to DVE (of 128); error budget cap

F32 = mybir.dt.float32
FP16 = mybir.dt.float16
BF16 = mybir.dt.bfloat16
FP8 = mybir.dt.float8e4
I16 = mybir.dt.int16
DR = mybir.MatmulPerfMode.DoubleRow
EXP = mybir.ActivationFunctionType.Exp
MUL = mybir.AluOpType.mult
ADD = mybir.AluOpType.add


def build_nc():
    nc = bacc.Bacc(
        "TRN2",
        target_bir_lowering=False,
        debug=False,
        num_devices=NCORES,
    )

    # ---- external I/O (per-core shards, host-prepped layouts) ----
    # DoubleRow-interleaved d-dim: d = (t2*2 + dj)*128 + p
    xdr = nc.dram_tensor("xdr", [P, 4, 2, N], FP8, kind="ExternalInput")
    # wqk[p, g, j, t2, dj, ec]: g = 4-head group (0..3 q, 4..7 k); j = d-half
    # of the head (e_local = j*32 + i); ec = hh*32 + i -> head 4*(g%4)+hh.
    wqk = nc.dram_tensor("wqk", [P, 8, 2, 4, 2, P], FP8, kind="ExternalInput")
    bqk = nc.dram_tensor("bqk", [P, 8, 2], F32, kind="ExternalInput")
    xdr2 = nc.dram_tensor("xdr2", [P, 4, 2, N], FP8, kind="ExternalInput")
    wv8 = nc.dram_tensor("wv8", [P, 4, 2, D], FP8, kind="ExternalInput")
    dwv8 = nc.dram_tensor("dwv8", [P, 4, 2, D], FP8, kind="ExternalInput")
    bv = nc.dram_tensor("bv", [D], BF16, kind="ExternalInput")
    wp = nc.dram_tensor("wp", [P, 8, D], FP16, kind="ExternalInput")
    bp = nc.dram_tensor("bp", [D], BF16, kind="ExternalInput")
    ident = nc.dram_tensor("ident", [P, P], FP16, kind="ExternalInput")
    bqk8 = nc.dram_tensor("bqk8", [1, 8, 2, P], FP8, kind="ExternalInput")
    out = nc.dram_tensor("out", [N, D], F32, kind="ExternalOutput")

    with tile.TileContext(nc) as tc:
        kernel_body(tc, xdr, wqk, bqk, xdr2, wv8, dwv8, bv, wp, bp, ident, bqk8, out)
    nc.compile()
    return nc


def kernel_body(tc, xdr, wqk, bqk, xdr2, wv8, dwv8, bv, wp, bp, ident, bqk8, out):
    nc = tc.nc
    from contextlib import ExitStack

    with ExitStack() as ctx:
        ctx.enter_context(
            nc.allow_low_precision(reason="fp16/fp8 matmul inputs by design")
        )
        res = ctx.enter_context(tc.tile_pool(name="res", bufs=1))
        shared = ctx.enter_context(tc.tile_pool(name="shared", bufs=1))
        gen_ps = ctx.enter_context(tc.tile_pool(name="gen_ps", bufs=2, space="PSUM"))
        av_ps = ctx.enter_context(tc.tile_pool(name="av_ps", bufs=2, space="PSUM"))
        sc_ps = ctx.enter_context(tc.tile_pool(name="sc_ps", bufs=2, space="PSUM"))
        exp_pool = ctx.enter_context(tc.tile_pool(name="exp_pool", bufs=35))
        rc_pool = ctx.enter_context(tc.tile_pool(name="rc_pool", bufs=4))
        nm_pool = ctx.enter_context(tc.tile_pool(name="nm_pool", bufs=4))
        oev_pool = ctx.enter_context(tc.tile_pool(name="oev_pool", bufs=3))

        # ---- resident tiles ----
        warm = res.tile([1, 513], FP16)
        nc.gpsimd.memset(warm[:], 1.0)

        xdr_sb = res.tile([P, 4, 2, N], FP8)
        wqk_sb = shared.tile([P, 8, 2, 4, 2, P], FP8, name="wqk_sb")
        bqk_sb = res.tile([P, 8, 2], F32)
        # q/k in scores-DR layout: group g (0..3 q, 4..7 k), partition
        # (hh*32+i), j, token -> value of head 4*(g%4)+hh, d = j*32+i
        q4k4 = res.tile([P, 8, 2, N], FP8)
        xdr2_sb = res.tile([P, 4, 2, N], FP8)
        wv8_sb = res.tile([P, 4, 2, D], FP8)
        dwv8_sb = res.tile([P, 4, 2, D], FP8)
        bv_rep = res.tile([P, D], BF16)
        wp_holder = []   # allocated from `shared` after QK-gen is emitted
        bp_rep = res.tile([P, D], BF16)

        # v_aug[p, nt, h, :]: cols 0:64 = v for head h at key chunk nt,
        # col 64 = 1.0 (flipped attn@v then emits the softmax denominator
        # in output column 64, one value per q-partition)
        v_aug = res.tile([P, 8, H, HD + 1], FP16)
        nc.gpsimd.memset(v_aug[:, :, :, HD : HD + 1], 1.0)

        # transposed normalized att, packed for proj: [d-part, qc, co, q]
        nmT = res.tile([P, 8, 8, P], FP16)
        ident_sb = res.tile([P, P], FP16)
        bqk8_sb = res.tile([1, 8, 2, P], FP8)
        ones8 = res.tile([1, 512], FP8)
        nc.gpsimd.memset(ones8[:], 1.0)

        # ---- input DMAs (sync queue, need-order; wp follows in the
        # filler queue, reusing wqk's SBUF once QK-gen is done) ----
        nc.sync.dma_start(xdr_sb[:], xdr[:])
        for g in (0, 4):
            for j in range(2):
                nc.sync.dma_start(wqk_sb[:, g, j], wqk[:, g, j])
        nc.sync.dma_start(bqk8_sb[:], bqk8[:])
        nc.sync.dma_start(bqk_sb[:], bqk[:])
        for g in (1, 5):
            for j in range(2):
                nc.sync.dma_start(wqk_sb[:, g, j], wqk[:, g, j])
        for t2 in range(4):
            nc.sync.dma_start(wv8_sb[:, t2], wv8[:, t2])
        nc.sync.dma_start(bv_rep[:], bv[None, :].to_broadcast([P, D]))
        for t2 in range(4):
            nc.sync.dma_start(xdr2_sb[:, t2], xdr2[:, t2])
        for t2 in range(4):
            nc.sync.dma_start(dwv8_sb[:, t2], dwv8[:, t2])
        for g in (2, 6, 3, 7):
            for j in range(2):
                nc.sync.dma_start(wqk_sb[:, g, j], wqk[:, g, j])
        nc.sync.dma_start(bp_rep[:], bp[None, :].to_broadcast([P, D]))
        nc.sync.dma_start(ident_sb[:], ident[:])

        # ---------------- emission helpers ----------------
        def warmup():
            # keep the PE p-state ramped while input DMAs land
            ps = gen_ps.tile([P, 512], F32, tag="gps", name="gps")
            nc.tensor.matmul(
                ps[0:1, :],
                lhsT=warm[:, 512:513],
                rhs=warm[:, 0:512],
                start=True,
                stop=True,
            )

        def qk_unit(g, j, ch, act_evac=False):
            # one QK-gen psum group: 4 fp8-DR matmuls + biased fp8 evac into
            # the scores-DR layout (GPSIMD cannot read PSUM, so evac on DVE;
            # the first units evac via ACT-Copy instead -- ACT idles during
            # startup -- with the bias folded in as a ones-row matmul)
            ps = gen_ps.tile([P, 512], F32, tag="gps", name="gps")
            for t2 in range(4):
                nc.tensor.matmul(
                    ps[:],
                    lhsT=wqk_sb[:, g, j, t2],
                    rhs=xdr_sb[:, t2, :, ch * 512 : (ch + 1) * 512],
                    start=(t2 == 0),
                    stop=(t2 == 3) and not act_evac,
                    perf_mode=DR,
                )
            if act_evac:
                nc.tensor.matmul(
                    ps[:],
                    lhsT=bqk8_sb[:, g, j],
                    rhs=ones8[:],
                    start=False,
                    stop=True,
                )
                nc.scalar.activation(
                    q4k4[:, g, j, ch * 512 : (ch + 1) * 512],
                    ps[:],
                    mybir.ActivationFunctionType.Copy,
                )
            else:
                nc.vector.tensor_scalar_add(
                    q4k4[:, g, j, ch * 512 : (ch + 1) * 512],
                    ps[:],
                    bqk_sb[:, g, j : j + 1],
                )

        def v_unit(eq, nt):
            # v[keys nt-chunk, 256 cols (4 heads) of quarter eq]: fp8-DR in
            # three equal-scale passes sharing one psum group --
            # 32*v = x8*(32w)8 + dx8*(32w)8 + x8*(32dw)8 -- then a scaled
            # evac; quarter granularity staggers the AV deadlines
            ps = gen_ps.tile([P, 256], F32, tag="gps", name="gps")
            es = eq * 256
            passes = [(xdr_sb, wv8_sb), (xdr2_sb, wv8_sb), (xdr_sb, dwv8_sb)]
            for pi, (xs, ws) in enumerate(passes):
                for t2 in range(4):
                    nc.tensor.matmul(
                        ps[:],
                        lhsT=xs[:, t2, :, nt * P : (nt + 1) * P],
                        rhs=ws[:, t2, :, es : es + 256],
                        start=(pi == 0 and t2 == 0),
                        stop=(pi == 2 and t2 == 3),
                        perf_mode=DR,
                    )
            nc.vector.scalar_tensor_tensor(
                v_aug[:, nt, eq * 4 : (eq + 1) * 4, 0:HD],
                ps[:].rearrange("p (h d) -> p h d", d=HD),
                1.0 / WS,
                bv_rep[:, es : es + 256].rearrange("p (h d) -> p h d", d=HD),
                op0=MUL,
                op1=ADD,
            )

        def sc_group(qh, co, g, hh, use_dve):
            # one kt-group of scores (fp8-DR) + its batched exp (ACT exact
            # or DVE Schraudolph); returns the exp tile
            h = 2 * co + hh
            grp = h // 4
            r = 32 * (h % 4)
            q0 = qh * QW
            ps = sc_ps.tile([P, 4, QW], F32, tag="scps", name="scps")
            for ki in range(4):
                kt = g * 4 + ki
                nc.tensor.matmul(
                    ps[:, ki],
                    lhsT=q4k4[r : r + 32, 4 + grp, :, kt * P : (kt + 1) * P],
                    rhs=q4k4[r : r + 32, grp, :, q0 : q0 + QW],
                    start=True,
                    stop=True,
                    perf_mode=DR,
                    tile_position=(r, 0),
                )
            ex = exp_pool.tile([P, 4, QW], FP16, tag="exp", name="exp")
            if use_dve:
                nc.vector.tensor_scalar(
                    ex[:].bitcast(I16),
                    ps[:],
                    SCHR_A,
                    SCHR_B,
                    op0=MUL,
                    op1=ADD,
                )
            else:
                nc.scalar.activation(ex[:], ps[:], EXP, scale=EXP_SCALE)
            return ex

        def av_halves(qh, co, exps):
            # flipped attn@v for one head pair, split per head; the batched
            # norm runs after the second half; the nm -> nmT transpose is a
            # separate unit (PE-array transpose + DVE evac) emitted later
            hold = []

            def half(hh):
                h = 2 * co + hh
                if hh == 0:
                    t = av_ps.tile([P, 392], F32, tag="avps", name="avps")
                    hold.append(t)
                ps = hold[0][:, 0:260].rearrange(
                    "p (a b c) -> p a b c", a=2, b=2
                )
                for qs in range(2):
                    for kt in range(8):
                        nc.tensor.matmul(
                            ps[:, qs, hh],
                            lhsT=exps[(hh, kt // 4)][:, kt % 4,
                                                     qs * P : (qs + 1) * P],
                            rhs=v_aug[:, kt, h, :],
                            start=(kt == 0),
                            stop=(kt == 7),
                        )
                if hh == 1:
                    rc = rc_pool.tile([P, 2, 2, 1], F32, tag="rc", name="rc")
                    nc.vector.reciprocal(rc[:], ps[:, :, :, HD : HD + 1])
                    nm = nm_pool.tile([P, 2, 2, HD], FP16, tag="nm", name="nm")
                    nc.vector.tensor_tensor(
                        nm[:],
                        ps[:, :, :, 0:HD],
                        rc[:].broadcast_to([P, 2, 2, HD]),
                        op=MUL,
                    )
                    hold.append(nm)

            def tp():
                # PE-array transpose of nm into proj layout + DVE evac;
                # keeps the nmT chain off the slow DMA queues
                t, nm = hold
                tpv = t[:, 264:392].bitcast(FP16).rearrange(
                    "p (a q) -> p a q", a=2
                )
                for qs in range(2):
                    nc.tensor.transpose(
                        tpv[:, qs], nm[:, qs], ident_sb[:]
                    )
                nc.vector.tensor_copy(
                    nmT[:, qh * 2 : qh * 2 + 2, co, :], tpv[:]
                )

            return (lambda: half(0)), (lambda: half(1)), tp

        def pj_halves(qh, nt, fh):
            # one projection output group split into two PE units
            qc = qh * 2 + nt
            n0 = qc * P
            hold = []

            def a():
                ps = gen_ps.tile([P, 512], F32, tag="gps", name="gps")
                hold.append(ps)
                for co in range(4):
                    nc.tensor.matmul(
                        ps[:],
                        lhsT=nmT[:, qc, co, :],
                        rhs=wp_holder[0][:, co, fh * 512 : (fh + 1) * 512],
                        start=(co == 0),
                        stop=False,
                    )

            def b():
                ps = hold[0]
                for co in range(4, 8):
                    nc.tensor.matmul(
                        ps[:],
                        lhsT=nmT[:, qc, co, :],
                        rhs=wp_holder[0][:, co, fh * 512 : (fh + 1) * 512],
                        start=False,
                        stop=(co == 7),
                    )
                ev = oev_pool.tile([P, 512], F32, tag="oev", name="oev")
                nc.vector.tensor_add(
                    ev[:], ps[:], bp_rep[:, fh * 512 : (fh + 1) * 512]
                )
                nc.sync.dma_start(
                    out[n0 : n0 + P, fh * 512 : (fh + 1) * 512], ev[:]
                )

            return a, b

        # ---------------- schedule ----------------
        # Priority scheduler with virtual engine clocks (pe/act/dve busy-until
        # estimates under the cost model). The exp stream (ACT + DVE
        # Schraudolph, routed by backlog) is the critical path; score groups
        # are emitted as fast as the sc_ps double-buffer allows. AV pairs and
        # QK/V/proj units fill PE slack one unit at a time from per-kind
        # queues, so a unit needed soon never forces a burst-drain of
        # unrelated work (which would starve the exp engines).
        from collections import deque

        C_SC = 220.0          # score group PE (4 fp8-DR matmuls)
        C_EXP_ACT = 1110.0
        C_EXP_DVE = 1280.0
        C_AVH = 440.0         # AV half PE
        C_NORM = 800.0        # batched recip+mult DVE
        C_QK = 430.0          # QK unit PE
        C_QK_EV = 750.0       # QK evac DVE
        C_VQ = 645.0          # V quarter-unit PE (12 fp8-DR matmuls)
        C_VQ_EV = 485.0       # V quarter evac DVE
        C_PJ = 1704.0         # proj unit PE (a+b)
        C_PJ_EV = 705.0       # proj evac DVE
        C_TP = 115.0          # nm transpose PE (2 PE-array transposes)
        C_TP_EV = 320.0       # nmT evac DVE
        TARGET_BL = 3600.0

        for _ in range(14):
            warmup()

        # QK units: first 4 unblock (qh0, co0, g0) scores; the rest are
        # popped on demand (per-unit) or as slack fillers
        qk_first = [(0, 0, 0), (0, 1, 0), (4, 0, 0), (4, 1, 0)]
        qk_q = deque()
        for (g, j, ch, est) in [(4, 0, 1, 5200.0), (4, 1, 1, 5600.0),
                                (0, 0, 1, 6000.0), (0, 1, 1, 6400.0)]:
            qk_q.append((est, (g, j, ch)))
        for grp, est in ((1, 7800.0), (2, 20700.0), (3, 22100.0)):
            for (g, ch) in ((grp, 0), (4 + grp, 0), (4 + grp, 1), (grp, 1)):
                for j in range(2):
                    qk_q.append((est, (g, j, ch)))
        v_q = deque()
        for eq in range(4):
            for nt in range(8):
                v_q.append((16500.0, (eq, nt)))
        pj_q = deque()

        pe_t = 6300.0
        act_t = 0.0
        dve_t = 0.0
        qk_done = set(qk_first)
        for i, u in enumerate(qk_first):
            # evacs split across ACT and DVE so they drain in parallel
            # during startup (both engines are otherwise idle)
            if i < 2:
                qk_unit(*u, act_evac=True)
                pe_t += C_QK + 213.0
                act_t = max(act_t, pe_t + 100.0) + 700.0
            else:
                qk_unit(*u)
                pe_t += C_QK
                dve_t = max(dve_t, pe_t + 100.0) + C_QK_EV

        av_pend = deque()
        exp_fin = []
        schr_n = 0
        v_pops = [0, 0, 0, 0]
        counts = [0] * NQ
        wp_loaded = [False]

        def wall():
            return max(pe_t, act_t - 2.0 * C_EXP_ACT, dve_t - 2.0 * C_EXP_DVE)

        def emit_qk(u):
            nonlocal pe_t, dve_t
            est = None
            for (e, uu) in qk_q:
                if uu == u:
                    est = e
                    break
            qk_q.remove((est, u))
            qk_unit(*u)
            qk_done.add(u)
            pe_t = max(pe_t, est) + C_QK
            dve_t = max(dve_t, pe_t + 100.0) + C_QK_EV
            if not qk_q and not wp_loaded[0]:
                wp_loaded[0] = True
                wp_load()

        def wp_load():
            wp_holder.append(shared.tile([P, 8, D], FP16, name="wp_sb"))
            for co in range(8):
                nc.sync.dma_start(wp_holder[0][:, co], wp[:, co])

        def emit_v():
            nonlocal pe_t, dve_t
            est, (eq, nt) = v_q.popleft()
            v_unit(eq, nt)
            v_pops[eq] += 1
            pe_t = max(pe_t, est) + C_VQ
            dve_t = max(dve_t, pe_t + 100.0) + C_VQ_EV

        def emit_pj():
            nonlocal pe_t, dve_t
            est, (qh0, nt, fh) = pj_q.popleft()
            a, b = pj_halves(qh0, nt, fh)
            a()
            b()
            pe_t = max(pe_t, est) + C_PJ
            dve_t = max(dve_t, pe_t + 100.0) + C_PJ_EV

        def pick_filler():
            # one slack unit, earliest-est first; False if nothing eligible
            cands = []
            if qk_q:
                cands.append((qk_q[0][0], 0))
            if v_q:
                cands.append((v_q[0][0], 1))
            if pj_q:
                cands.append((pj_q[0][0], 2))
            cands = [c for c in cands if c[0] <= wall() + 400.0]
            if not cands:
                return False
            cands.sort()
            kind = cands[0][1]
            if kind == 0:
                emit_qk(qk_q[0][1])
            elif kind == 1:
                emit_v()
            else:
                emit_pj()
            return True

        pending_tp = []

        def flush_tp():
            nonlocal pe_t, dve_t
            while pending_tp:
                qh0, co0, tp = pending_tp.pop(0)
                tp()
                pe_t += C_TP
                dve_t = max(dve_t, pe_t + 100.0) + C_TP_EV
                counts[qh0] += 1
                if counts[qh0] == 8:
                    est_pj = max(wall(), dve_t) + 4200.0
                    for nt in range(2):
                        for fh in range(2):
                            pj_q.append((est_pj, (qh0, nt, fh)))

        def av_ready():
            if not av_pend:
                return False
            qh0, co0, _ = av_pend[0]
            return v_pops[co0 // 2] >= 8

        def av_emit():
            nonlocal pe_t, dve_t
            flush_tp()
            qh0, co0, exps0 = av_pend.popleft()
            a, b, tp = av_halves(qh0, co0, exps0)
            a()
            pe_t += C_AVH
            b()
            pe_t += C_AVH
            dve_t = max(dve_t, pe_t + 100.0) + C_NORM
            pending_tp.append((qh0, co0, tp))

        FILL_TOTAL = 24 * C_QK + 32 * C_VQ + 32 * C_PJ / 2.0
        fill_pe = [0.0]

        def quota_fill(limit):
            n = 0
            while (
                n < limit
                and fill_pe[0] < (it + 1) * (FILL_TOTAL / 32.0)
            ):
                before = (len(qk_q), len(v_q), len(pj_q))
                if not pick_filler():
                    break
                after = (len(qk_q), len(v_q), len(pj_q))
                if before[0] != after[0]:
                    fill_pe[0] += C_QK
                elif before[1] != after[1]:
                    fill_pe[0] += C_VQ
                else:
                    fill_pe[0] += C_PJ
                n += 1

        BLOCKS = [(0, 0), (1, 0), (0, 4), (2, 0), (1, 4), (3, 0), (2, 4), (3, 4)]
        it = -1
        for (qh, co0_blk) in BLOCKS:
            for co in range(co0_blk, co0_blk + 4):
                it += 1
                keep = 3 if it < 29 else 1
                if av_ready() and len(av_pend) > keep:
                    av_emit()
                quota_fill(4)
                exps = {}
                for g in range(2):
                    for hh in range(2):
                        grp = co // 2
                        for u in [(grp, 0, qh // 2), (grp, 1, qh // 2),
                                  (4 + grp, 0, g), (4 + grp, 1, g)]:
                            if u not in qk_done:
                                emit_qk(u)
                        use_dve = (
                            schr_n < SCHR_MAX
                            and it >= 4
                            and act_t - dve_t > 1200.0
                        )
                        busy_t = dve_t if use_dve else act_t
                        ni = len(exp_fin)
                        cap = exp_fin[ni - 2] if ni >= 2 else 0.0
                        # fill PE while the psum cap blocks or the exp
                        # engine is well-fed
                        while True:
                            gate = max(pe_t, cap)
                            if busy_t - gate < TARGET_BL and pe_t >= cap - 100.0:
                                break
                            if av_ready() and len(av_pend) >= 2:
                                av_emit()
                            elif pick_filler():
                                pass
                            else:
                                break
                            busy_t = dve_t if use_dve else act_t
                        # exp-pool pressure: drain AVs (or the V units
                        # blocking them) before allocating another tile
                        while 4 * len(av_pend) + 6 > 33:
                            if av_ready():
                                av_emit()
                            elif v_q:
                                emit_v()
                            elif not pick_filler():
                                break
                        pe_t = max(pe_t, cap) + C_SC
                        ex = sc_group(qh, co, g, hh, use_dve)
                        flush_tp()
                        if use_dve:
                            schr_n += 1
                            st = max(dve_t, pe_t + 100.0)
                            dve_t = st + C_EXP_DVE
                            exp_fin.append(dve_t)
                        else:
                            st = max(act_t, pe_t + 100.0)
                            act_t = st + C_EXP_ACT
                            exp_fin.append(act_t)
                        exps[(hh, g)] = ex
                av_pend.append((qh, co, exps))
        while av_pend:
            if not av_ready():
                emit_v()
                continue
            av_emit()
        flush_tp()
        while qk_q:
            emit_qk(qk_q[0][1])
        while v_q:
            emit_v()
        while pj_q:
            emit_pj()


def make_in_maps(x, c, kv_w, kv_b, shared_q_w, shared_q_b, cohort_q_w, cohort_q_b,
                 proj_w, proj_b):
    f32 = np.float32
    fp16 = np.float16
    fp8 = mybir.dt.np(FP8)
    x = np.asarray(x, dtype=f32)
    c = np.asarray(c).astype(np.int64)
    kv_w = np.asarray(kv_w, dtype=f32)
    kv_b = np.asarray(kv_b, dtype=f32)
    shared_q_w = np.asarray(shared_q_w, dtype=f32)
    shared_q_b = np.asarray(shared_q_b, dtype=f32)
    cohort_q_w = np.asarray(cohort_q_w, dtype=f32)
    cohort_q_b = np.asarray(cohort_q_b, dtype=f32)
    proj_w = np.asarray(proj_w, dtype=f32)
    proj_b = np.asarray(proj_b, dtype=f32)

    wk = kv_w[:D] * WS
    wv_ = kv_w[D:]
    bk = kv_b[:D] * WS
    bv_ = kv_b[D:]

    w32 = wv_.T * WS                      # [d, e]
    w8 = w32.astype(fp8)
    dw8 = (w32 - w8.astype(f32)).astype(fp8)
    wv8_h = np.ascontiguousarray(
        w8.reshape(4, 2, P, D).transpose(2, 0, 1, 3)
    )
    dwv8_h = np.ascontiguousarray(
        dw8.reshape(4, 2, P, D).transpose(2, 0, 1, 3)
    )
    wp_h = np.ascontiguousarray(
        proj_w.T.reshape(8, P, D).transpose(1, 0, 2)
    ).astype(fp16)

    in_maps = []
    for b in range(x.shape[0]):
        wq = np.concatenate([shared_q_w, cohort_q_w[c[b]]], axis=0) * WS
        bq = np.concatenate([shared_q_b, cohort_q_b[c[b]]], axis=0) * WS
        wqk_cols = np.concatenate([wq, wk], axis=0)     # [2048 e, 1024 d]
        # e = qk*1024 + head*64 + j*32 + i with head = 4*g4 + hh;
        # device wants [p, g(qk,g4), j, t2, dj, ec(hh,i)]
        wqk_e = wqk_cols.reshape(2, 4, 4, 2, 32, D)   # [qk, g4, hh, j, i, d]
        wqk_e = wqk_e.transpose(0, 1, 3, 2, 4, 5).reshape(8, 2, P, D)
        wqk_full = wqk_e.reshape(8, 2, P, 4, 2, P)    # [g, j, ec, t2, dj, p]
        wqk_h = np.ascontiguousarray(
            wqk_full.transpose(5, 0, 1, 3, 4, 2)
        ).astype(fp8)
        bqk_e = np.concatenate([bq, bk]).reshape(2, 4, 4, 2, 32)
        bqk_h = np.ascontiguousarray(
            bqk_e.transpose(0, 1, 3, 2, 4).reshape(8, 2, P).transpose(2, 0, 1)
        ).astype(f32)
        bqk8_h = np.ascontiguousarray(
            bqk_e.transpose(0, 1, 3, 2, 4).reshape(8, 2, P)[None]
        ).astype(fp8)
        xT = x[b].T
        x8 = xT.astype(fp8)
        dx8 = (xT - x8.astype(f32)).astype(fp8)
        xdr_h = np.ascontiguousarray(
            x8.reshape(4, 2, P, N).transpose(2, 0, 1, 3)
        )
        xdr2_h = np.ascontiguousarray(
            dx8.reshape(4, 2, P, N).transpose(2, 0, 1, 3)
        )
        m = {
            "ident": np.eye(P, dtype=fp16),
            "bqk8": bqk8_h,
            "xdr": xdr_h,
            "xdr2": xdr2_h,
            "wqk": wqk_h,
            "bqk": bqk_h,
            "wv8": wv8_h,
            "dwv8": dwv8_h,
            "bv": np.ascontiguousarray(bv_).astype(mybir.dt.np(BF16)),
            "wp": wp_h,
            "bp": np.ascontiguousarray(proj_b).astype(mybir.dt.np(BF16)),
        }
        in_maps.append(m)
    return in_maps


_NC_CACHE = {}


def kernel(**inputs) -> np.ndarray:
    in_maps = make_in_maps(**inputs)
    if "nc" not in _NC_CACHE:
        _NC_CACHE["nc"] = build_nc()
    nc = _NC_CACHE["nc"]
    res = run_bass_kernel_spmd(nc, in_maps, core_ids=list(range(NCORES)))
    out = np.stack([res.results[i]["out"] for i in range(NCORES)], axis=0)
    return out.astype(np.float32)


# revision 77
# speedup vs baseline: 1.0699x; 1.0051x over previous
"""CohortAwareBlock Trainium2 kernel.

Data-parallel over batch B=8 across 8 NeuronCores (one sample per core).
Cohort routing (gather of cohort_q_w by per-sample cohort id) happens on the
host while building each core's weight tensors; the device kernel is a plain
attention block.

Numerics:
  - QK-gen runs as fp8-e4m3 DoubleRow matmuls (weights pre-scaled x32 to
    dodge fp8 subnormals; the inverse scale is folded into the exp scale).
  - q/k are stored as fp8 in a DoubleRow-interleaved layout ([32, 2, N] per
    head, 4 heads stacked across 128 partitions at 32-partition tile
    positions) so the scores matmul also runs fp8-DR: 2x fewer PE cycles
    than fp16 scores.
  - exp splits across the ACT engine (exact table exp, fp16 out) and the
    DVE (Schraudolph bit-trick: int16(A*s + B) written through a bitcast
    view and read back as fp16; ~1.8% rms sawtooth error, SCHR_MAX-capped
    for the error budget) so the exp stream is not ACT-bound.
  - V-gen runs as fp8-DR in three equal-scale residual passes
    (32v = x8*(32w)8 + dx8*(32w)8 + x8*(32dw)8, one psum group; the
    residual operands ride e4m3 subnormals) for fp16-grade accuracy at
    fp8 cost; attn weights / projection stay fp16.

Per-core structure:
  q4k4 [128, 8, 2, N] fp8  (4 q-head groups + 4 k-head groups, DR layout)
  v_aug [keys, h, 65] fp16 (col 64 = 1.0 so the flipped AV emits the
                            softmax denominator per q-partition)
  per (q-quarter, head pair):
    scores -> 2-bank PSUM [128, 4, 256] via fp8-DR -> exp (ACT or DVE,
    routed by backlog) ->
    flipped attn@v: av psum [128, 2, 2, 65]; col 64 = den ->
    batched DVE reciprocal + broadcast mult -> nm fp16 ->
    PE-array transpose (vs identity) + DVE evac -> nmT [d, q] ->
    proj (fp16) + bias -> out DMA on the idle GPSIMD queue

Scheduling: a priority scheduler with virtual engine clocks emits score
groups as fast as the sc_ps double-buffer allows (the exp stream is the
critical path), drains attn@v pairs as the preferred PE slack-filler, and
paces QK/V/proj units from per-kind queues (quota per iteration, popped
on demand for data dependencies) so no slow unit head-of-line-blocks the
in-order PE queue. The first QK units evac via ACT-Copy (bias folded in as
a ones-row matmul) while ACT is otherwise idle during startup; dummy
warmup matmuls keep the PE p-state ramped until the first real work; the
iteration visits head pairs in a block order that staggers the V-gen
deadlines.
"""

import numpy as np

import concourse.bass as bass
import concourse.bacc as bacc
import concourse.mybir as mybir
import concourse.tile as tile
from concourse.bass_utils import run_bass_kernel_spmd

P = 128
N = 1024            # sequence length
D = 1024            # model dim
H = 16              # heads
HD = 64             # head dim
NQ = 4              # q-quarters (256 q each)
QW = N // NQ        # 256
SCALE = HD ** -0.5
NCORES = 8

WS = 32.0           # fp8 pre-scale on w_q/w_k (and so on q/k values)
EXP_SCALE = SCALE / (WS * WS)

# Schraudolph fp16-bitcast exp on DVE: y_bits = int16(s * A + B); bits read
# as fp16 give exp(s*EXP_SCALE) with ~1.8% rms sawtooth error.
LOG2E = 1.4426950408889634
SCHR_A = EXP_SCALE * LOG2E * 1024.0
SCHR_B = 15301.0
SCHR_MAX = 27        # max exp groups routed to DVE (of 128); error budget cap

F32 = mybir.dt.float32
FP16 = mybir.dt.float16
BF16 = mybir.dt.bfloat16
FP8 = mybir.dt.float8e4
I16 = mybir.dt.int16
DR = mybir.MatmulPerfMode.DoubleRow
EXP = mybir.ActivationFunctionType.Exp
MUL = mybir.AluOpType.mult
ADD = mybir.AluOpType.add


def build_nc():
    nc = bacc.Bacc(
        "TRN2",
        target_bir_lowering=False,
        debug=False,
        num_devices=NCORES,
    )

    # ---- external I/O (per-core shards, host-prepped layouts) ----
    # DoubleRow-interleaved d-dim: d = (t2*2 + dj)*128 + p
    xdr = nc.dram_tensor("xdr", [P, 4, 2, N], FP8, kind="ExternalInput")
    # wqk[p, g, j, t2, dj, ec]: g = 4-head group (0..3 q, 4..7 k); j = d-half
    # of the head (e_local = j*32 + i); ec = hh*32 + i -> head 4*(g%4)+hh.
    wqk = nc.dram_tensor("wqk", [P, 8, 2, 4, 2, P], FP8, kind="ExternalInput")
    bqk = nc.dram_tensor("bqk", [P, 8, 2], F32, kind="ExternalInput")
    xdr2 = nc.dram_tensor("xdr2", [P, 4, 2, N], FP8, kind="ExternalInput")
    wv8 = nc.dram_tensor("wv8", [P, 4, 2, D], FP8, kind="ExternalInput")
    dwv8 = nc.dram_tensor("dwv8", [P, 4, 2, D], FP8, kind="ExternalInput")
    bv = nc.dram_tensor("bv", [D], BF16, kind="ExternalInput")
    wp = nc.dram_tensor("wp", [P, 8, D], FP16, kind="ExternalInput")
    bp = nc.dram_tensor("bp", [D], BF16, kind="ExternalInput")
    ident = nc.dram_tensor("ident", [P, P], FP16, kind="ExternalInput")
    bqk8 = nc.dram_tensor("bqk8", [1, 8, 2, P], FP8, kind="ExternalInput")
    out = nc.dram_tensor("out", [N, D], F32, kind="ExternalOutput")

    with tile.TileContext(nc) as tc:
        kernel_body(tc, xdr, wqk, bqk, xdr2, wv8, dwv8, bv, wp, bp, ident, bqk8, out)
    nc.compile()
    return nc


def kernel_body(tc, xdr, wqk, bqk, xdr2, wv8, dwv8, bv, wp, bp, ident, bqk8, out):
    nc = tc.nc
    from contextlib import ExitStack

    with ExitStack() as ctx:
        ctx.enter_context(
            nc.allow_low_precision(reason="fp16/fp8 matmul inputs by design")
        )
        res = ctx.enter_context(tc.tile_pool(name="res", bufs=1))
        shared = ctx.enter_context(tc.tile_pool(name="shared", bufs=1))
        gen_ps = ctx.enter_context(tc.tile_pool(name="gen_ps", bufs=2, space="PSUM"))
        av_ps = ctx.enter_context(tc.tile_pool(name="av_ps", bufs=2, space="PSUM"))
        sc_ps = ctx.enter_context(tc.tile_pool(name="sc_ps", bufs=2, space="PSUM"))
        exp_pool = ctx.enter_context(tc.tile_pool(name="exp_pool", bufs=35))
        rc_pool = ctx.enter_context(tc.tile_pool(name="rc_pool", bufs=4))
        nm_pool = ctx.enter_context(tc.tile_pool(name="nm_pool", bufs=4))
        oev_pool = ctx.enter_context(tc.tile_pool(name="oev_pool", bufs=3))

        # ---- resident tiles ----
        warm = res.tile([1, 513], FP16)
        nc.gpsimd.memset(warm[:], 1.0)

        xdr_sb = res.tile([P, 4, 2, N], FP8)
        wqk_sb = shared.tile([P, 8, 2, 4, 2, P], FP8, name="wqk_sb")
        bqk_sb = res.tile([P, 8, 2], F32)
        # q/k in scores-DR layout: group g (0..3 q, 4..7 k), partition
        # (hh*32+i), j, token -> value of head 4*(g%4)+hh, d = j*32+i
        q4k4 = res.tile([P, 8, 2, N], FP8)
        xdr2_sb = res.tile([P, 4, 2, N], FP8)
        wv8_sb = res.tile([P, 4, 2, D], FP8)
        dwv8_sb = res.tile([P, 4, 2, D], FP8)
        bv_rep = res.tile([P, D], BF16)
        wp_holder = []   # allocated from `shared` after QK-gen is emitted
        bp_rep = res.tile([P, D], BF16)

        # v_aug[p, nt, h, :]: cols 0:64 = v for head h at key chunk nt,
        # col 64 = 1.0 (flipped attn@v then emits the softmax denominator
        # in output column 64, one value per q-partition)
        v_aug = res.tile([P, 8, H, HD + 1], FP16)
        nc.gpsimd.memset(v_aug[:, :, :, HD : HD + 1], 1.0)

        # transposed normalized att, packed for proj: [d-part, qc, co, q]
        nmT = res.tile([P, 8, 8, P], FP16)
        ident_sb = res.tile([P, P], FP16)
        bqk8_sb = res.tile([1, 8, 2, P], FP8)
        ones8 = res.tile([1, 512], FP8)
        nc.gpsimd.memset(ones8[:], 1.0)

        # ---- input DMAs (sync queue, need-order; wp follows in the
        # filler queue, reusing wqk's SBUF once QK-gen is done) ----
        nc.sync.dma_start(xdr_sb[:], xdr[:])
        for g in (0, 4):
            for j in range(2):
                nc.sync.dma_start(wqk_sb[:, g, j], wqk[:, g, j])
        nc.sync.dma_start(bqk8_sb[:], bqk8[:])
        nc.sync.dma_start(bqk_sb[:], bqk[:])
        for g in (1, 5):
            for j in range(2):
                nc.sync.dma_start(wqk_sb[:, g, j], wqk[:, g, j])
        for t2 in range(4):
            nc.sync.dma_start(wv8_sb[:, t2], wv8[:, t2])
        nc.sync.dma_start(bv_rep[:], bv[None, :].to_broadcast([P, D]))
        for t2 in range(4):
            nc.sync.dma_start(xdr2_sb[:, t2], xdr2[:, t2])
        for t2 in range(4):
            nc.sync.dma_start(dwv8_sb[:, t2], dwv8[:, t2])
        for g in (2, 6, 3, 7):
            for j in range(2):
                nc.sync.dma_start(wqk_sb[:, g, j], wqk[:, g, j])
        nc.sync.dma_start(bp_rep[:], bp[None, :].to_broadcast([P, D]))
        nc.sync.dma_start(ident_sb[:], ident[:])

        # ---------------- emission helpers ----------------
        def warmup():
            # keep the PE p-state ramped while input DMAs land
            ps = gen_ps.tile([P, 512], F32, tag="gps", name="gps")
            nc.tensor.matmul(
                ps[0:1, :],
                lhsT=warm[:, 512:513],
                rhs=warm[:, 0:512],
                start=True,
                stop=True,
            )

        def qk_unit(g, j, ch, act_evac=False):
            # one QK-gen psum group: 4 fp8-DR matmuls + biased fp8 evac into
            # the scores-DR layout (GPSIMD cannot read PSUM, so evac on DVE;
            # the first units evac via ACT-Copy instead -- ACT idles during
            # startup -- with the bias folded in as a ones-row matmul)
            ps = gen_ps.tile([P, 512], F32, tag="gps", name="gps")
            for t2 in range(4):
                nc.tensor.matmul(
                    ps[:],
                    lhsT=wqk_sb[:, g, j, t2],
                    rhs=xdr_sb[:, t2, :, ch * 512 : (ch + 1) * 512],
                    start=(t2 == 0),
                    stop=(t2 == 3) and not act_evac,
                    perf_mode=DR,
                )
            if act_evac:
                nc.tensor.matmul(
                    ps[:],
                    lhsT=bqk8_sb[:, g, j],
                    rhs=ones8[:],
                    start=False,
                    stop=True,
                )
                nc.scalar.activation(
                    q4k4[:, g, j, ch * 512 : (ch + 1) * 512],
                    ps[:],
                    mybir.ActivationFunctionType.Copy,
                )
            else:
                nc.vector.tensor_scalar_add(
                    q4k4[:, g, j, ch * 512 : (ch + 1) * 512],
                    ps[:],
                    bqk_sb[:, g, j : j + 1],
                )

        def v_unit(eq, nt):
            # v[keys nt-chunk, 256 cols (4 heads) of quarter eq]: fp8-DR in
            # three equal-scale passes sharing one psum group --
            # 32*v = x8*(32w)8 + dx8*(32w)8 + x8*(32dw)8 -- then a scaled
            # evac; quarter granularity staggers the AV deadlines
            ps = gen_ps.tile([P, 256], F32, tag="gps", name="gps")
            es = eq * 256
            passes = [(xdr_sb, wv8_sb), (xdr2_sb, wv8_sb), (xdr_sb, dwv8_sb)]
            for pi, (xs, ws) in enumerate(passes):
                for t2 in range(4):
                    nc.tensor.matmul(
                        ps[:],
                        lhsT=xs[:, t2, :, nt * P : (nt + 1) * P],
                        rhs=ws[:, t2, :, es : es + 256],
                        start=(pi == 0 and t2 == 0),
                        stop=(pi == 2 and t2 == 3),
                        perf_mode=DR,
                    )
            nc.vector.scalar_tensor_tensor(
                v_aug[:, nt, eq * 4 : (eq + 1) * 4, 0:HD],
                ps[:].rearrange("p (h d) -> p h d", d=HD),
                1.0 / WS,
                bv_rep[:, es : es + 256].rearrange("p (h d) -> p h d", d=HD),
                op0=MUL,
                op1=ADD,
            )

        def sc_group(qh, co, g, hh, use_dve):
            # one kt-group of scores (fp8-DR) + its batched exp (ACT exact
            # or DVE Schraudolph); returns the exp tile
            h = 2 * co + hh
            grp = h // 4
            r = 32 * (h % 4)
            q0 = qh * QW
            ps = sc_ps.tile([P, 4, QW], F32, tag="scps", name="scps")
            for ki in range(4):
                kt = g * 4 + ki
                nc.tensor.matmul(
                    ps[:, ki],
                    lhsT=q4k4[r : r + 32, 4 + grp, :, kt * P : (kt + 1) * P],
                    rhs=q4k4[r : r + 32, grp, :, q0 : q0 + QW],
                    start=True,
                    stop=True,
                    perf_mode=DR,
                    tile_position=(r, 0),
                )
            ex = exp_pool.tile([P, 4, QW], FP16, tag="exp", name="exp")
            if use_dve:
                nc.vector.tensor_scalar(
                    ex[:].bitcast(I16),
                    ps[:],
                    SCHR_A,
                    SCHR_B,
                    op0=MUL,
                    op1=ADD,
                )
            else:
                nc.scalar.activation(ex[:], ps[:], EXP, scale=EXP_SCALE)
            return ex

        def av_halves(qh, co, exps):
            # flipped attn@v for one head pair, split per head; the batched
            # norm runs after the second half; the nm -> nmT transpose is a
            # separate unit (PE-array transpose + DVE evac) emitted later
            hold = []

            def half(hh):
                h = 2 * co + hh
                if hh == 0:
                    t = av_ps.tile([P, 392], F32, tag="avps", name="avps")
                    hold.append(t)
                ps = hold[0][:, 0:260].rearrange(
                    "p (a b c) -> p a b c", a=2, b=2
                )
                for qs in range(2):
                    for kt in range(8):
                        nc.tensor.matmul(
                            ps[:, qs, hh],
                            lhsT=exps[(hh, kt // 4)][:, kt % 4,
                                                     qs * P : (qs + 1) * P],
                            rhs=v_aug[:, kt, h, :],
                            start=(kt == 0),
                            stop=(kt == 7),
                        )
                if hh == 1:
                    rc = rc_pool.tile([P, 2, 2, 1], F32, tag="rc", name="rc")
                    nc.vector.reciprocal(rc[:], ps[:, :, :, HD : HD + 1])
                    nm = nm_pool.tile([P, 2, 2, HD], FP16, tag="nm", name="nm")
                    nc.vector.tensor_tensor(
                        nm[:],
                        ps[:, :, :, 0:HD],
                        rc[:].broadcast_to([P, 2, 2, HD]),
                        op=MUL,
                    )
                    hold.append(nm)

            def tp():
                # PE-array transpose of nm into proj layout + DVE evac;
                # keeps the nmT chain off the slow DMA queues
                t, nm = hold
                tpv = t[:, 264:392].bitcast(FP16).rearrange(
                    "p (a q) -> p a q", a=2
                )
                for qs in range(2):
                    nc.tensor.transpose(
                        tpv[:, qs], nm[:, qs], ident_sb[:]
                    )
                nc.vector.tensor_copy(
                    nmT[:, qh * 2 : qh * 2 + 2, co, :], tpv[:]
                )

            return (lambda: half(0)), (lambda: half(1)), tp

        def pj_halves(qh, nt, fh):
            # one projection output group split into two PE units
            qc = qh * 2 + nt
            n0 = qc * P
            hold = []

            def a():
                ps = gen_ps.tile([P, 512], F32, tag="gps", name="gps")
                hold.append(ps)
                for co in range(4):
                    nc.tensor.matmul(
                        ps[:],
                        lhsT=nmT[:, qc, co, :],
                        rhs=wp_holder[0][:, co, fh * 512 : (fh + 1) * 512],
                        start=(co == 0),
                        stop=False,
                    )

            def b():
                ps = hold[0]
                for co in range(4, 8):
                    nc.tensor.matmul(
                        ps[:],
                        lhsT=nmT[:, qc, co, :],
                        rhs=wp_holder[0][:, co, fh * 512 : (fh + 1) * 512],
                        start=False,
                        stop=(co == 7),
                    )
                ev = oev_pool.tile([P, 512], F32, tag="oev", name="oev")
                nc.vector.tensor_add(
                    ev[:], ps[:], bp_rep[:, fh * 512 : (fh + 1) * 512]
                )
                nc.sync.dma_start(
                    out[n0 : n0 + P, fh * 512 : (fh + 1) * 512], ev[:]
                )

            return a, b

        # ---------------- schedule ----------------
        # Priority scheduler with virtual engine clocks (pe/act/dve busy-until
        # estimates under the cost model). The exp stream (ACT + DVE
        # Schraudolph, routed by backlog) is the critical path; score groups
        # are emitted as fast as the sc_ps double-buffer allows. AV pairs and
        # QK/V/proj units fill PE slack one unit at a time from per-kind
        # queues, so a unit needed soon never forces a burst-drain of
        # unrelated work (which would starve the exp engines).
        from collections import deque

        C_SC = 220.0          # score group PE (4 fp8-DR matmuls)
        C_EXP_ACT = 1110.0
        C_EXP_DVE = 1280.0
        C_AVH = 440.0         # AV half PE
        C_NORM = 800.0        # batched recip+mult DVE
        C_QK = 430.0          # QK unit PE
        C_QK_EV = 750.0       # QK evac DVE
        C_VQ = 645.0          # V quarter-unit PE (12 fp8-DR matmuls)
        C_VQ_EV = 485.0       # V quarter evac DVE
        C_PJ = 1704.0         # proj unit PE (a+b)
        C_PJ_EV = 705.0       # proj evac DVE
        C_TP = 115.0          # nm transpose PE (2 PE-array transposes)
        C_TP_EV = 320.0       # nmT evac DVE
        TARGET_BL = 3600.0

        for _ in range(14):
            warmup()

        # QK units: first 4 unblock (qh0, co0, g0) scores; the rest are
        # popped on demand (per-unit) or as slack fillers
        qk_first = [(0, 0, 0), (0, 1, 0), (4, 0, 0), (4, 1, 0)]
        qk_q = deque()
        for (g, j, ch, est) in [(4, 0, 1, 5200.0), (4, 1, 1, 5600.0),
                                (0, 0, 1, 6000.0), (0, 1, 1, 6400.0)]:
            qk_q.append((est, (g, j, ch)))
        for grp, est in ((1, 7800.0), (2, 19000.0), (3, 20400.0)):
            for (g, ch) in ((grp, 0), (4 + grp, 0), (4 + grp, 1), (grp, 1)):
                for j in range(2):
                    qk_q.append((est, (g, j, ch)))
        v_q = deque()
        for eq in range(4):
            for nt in range(8):
                v_q.append((17500.0, (eq, nt)))
        pj_q = deque()

        pe_t = 6300.0
        act_t = 0.0
        dve_t = 0.0
        qk_done = set(qk_first)
        for i, u in enumerate(qk_first):
            # evacs split across ACT and DVE so they drain in parallel
            # during startup (both engines are otherwise idle)
            if i < 2:
                qk_unit(*u, act_evac=True)
                pe_t += C_QK + 213.0
                act_t = max(act_t, pe_t + 100.0) + 700.0
            else:
                qk_unit(*u)
                pe_t += C_QK
                dve_t = max(dve_t, pe_t + 100.0) + C_QK_EV

        av_pend = deque()
        exp_fin = []
        schr_n = 0
        v_pops = [0, 0, 0, 0]
        counts = [0] * NQ
        wp_loaded = [False]

        def wall():
            return max(pe_t, act_t - 2.0 * C_EXP_ACT, dve_t - 2.0 * C_EXP_DVE)

        def emit_qk(u):
            nonlocal pe_t, dve_t
            est = None
            for (e, uu) in qk_q:
                if uu == u:
                    est = e
                    break
            qk_q.remove((est, u))
            qk_unit(*u)
            qk_done.add(u)
            pe_t = max(pe_t, est) + C_QK
            dve_t = max(dve_t, pe_t + 100.0) + C_QK_EV
            if not qk_q and not wp_loaded[0]:
                wp_loaded[0] = True
                wp_load()

        def wp_load():
            wp_holder.append(shared.tile([P, 8, D], FP16, name="wp_sb"))
            for co in range(8):
                nc.sync.dma_start(wp_holder[0][:, co], wp[:, co])

        def emit_v():
            nonlocal pe_t, dve_t
            est, (eq, nt) = v_q.popleft()
            v_unit(eq, nt)
            v_pops[eq] += 1
            pe_t = max(pe_t, est) + C_VQ
            dve_t = max(dve_t, pe_t + 100.0) + C_VQ_EV

        def emit_pj():
            nonlocal pe_t, dve_t
            est, (qh0, nt, fh) = pj_q.popleft()
            a, b = pj_halves(qh0, nt, fh)
            a()
            b()
            pe_t = max(pe_t, est) + C_PJ
            dve_t = max(dve_t, pe_t + 100.0) + C_PJ_EV

        def pick_filler():
            # one slack unit, earliest-est first; False if nothing eligible
            cands = []
            if qk_q:
                cands.append((qk_q[0][0], 0))
            if v_q:
                cands.append((v_q[0][0], 1))
            if pj_q:
                cands.append((pj_q[0][0], 2))
            cands = [c for c in cands if c[0] <= wall() + 400.0]
            if not cands:
                return False
            cands.sort()
            kind = cands[0][1]
            if kind == 0:
                emit_qk(qk_q[0][1])
            elif kind == 1:
                emit_v()
            else:
                emit_pj()
            return True

        pending_tp = []

        def flush_tp():
            nonlocal pe_t, dve_t
            while pending_tp:
                qh0, co0, tp = pending_tp.pop(0)
                tp()
                pe_t += C_TP
                dve_t = max(dve_t, pe_t + 100.0) + C_TP_EV
                counts[qh0] += 1
                if counts[qh0] == 8:
                    est_pj = max(wall(), dve_t) + 4200.0
                    for nt in range(2):
                        for fh in range(2):
                            pj_q.append((est_pj, (qh0, nt, fh)))

        def av_ready():
            if not av_pend:
                return False
            qh0, co0, _ = av_pend[0]
            return v_pops[co0 // 2] >= 8

        def av_emit():
            nonlocal pe_t, dve_t
            flush_tp()
            qh0, co0, exps0 = av_pend.popleft()
            a, b, tp = av_halves(qh0, co0, exps0)
            a()
            pe_t += C_AVH
            b()
            pe_t += C_AVH
            dve_t = max(dve_t, pe_t + 100.0) + C_NORM
            pending_tp.append((qh0, co0, tp))

        FILL_TOTAL = 24 * C_QK + 32 * C_VQ + 32 * C_PJ / 2.0
        fill_pe = [0.0]

        def quota_fill(limit):
            n = 0
            while (
                n < limit
                and fill_pe[0] < (it + 1) * (FILL_TOTAL / 32.0)
            ):
                before = (len(qk_q), len(v_q), len(pj_q))
                if not pick_filler():
                    break
                after = (len(qk_q), len(v_q), len(pj_q))
                if before[0] != after[0]:
                    fill_pe[0] += C_QK
                elif before[1] != after[1]:
                    fill_pe[0] += C_VQ
                else:
                    fill_pe[0] += C_PJ
                n += 1

        BLOCKS = [(0, 0), (1, 0), (0, 4), (2, 0), (1, 4), (3, 0), (2, 4), (3, 4)]
        it = -1
        for (qh, co0_blk) in BLOCKS:
            for co in range(co0_blk, co0_blk + 4):
                it += 1
                keep = 3 if it < 29 else 1
                if av_ready() and len(av_pend) > keep:
                    av_emit()
                quota_fill(4)
                exps = {}
                for g in range(2):
                    for hh in range(2):
                        grp = co // 2
                        for u in [(grp, 0, qh // 2), (grp, 1, qh // 2),
                                  (4 + grp, 0, g), (4 + grp, 1, g)]:
                            if u not in qk_done:
                                emit_qk(u)
                        use_dve = (
                            schr_n < SCHR_MAX
                            and it >= 4
                            and act_t - dve_t > 1200.0
                        )
                        busy_t = dve_t if use_dve else act_t
                        ni = len(exp_fin)
                        cap = exp_fin[ni - 2] if ni >= 2 else 0.0
                        # fill PE while the psum cap blocks or the exp
                        # engine is well-fed
                        while True:
                            gate = max(pe_t, cap)
                            if busy_t - gate < TARGET_BL and pe_t >= cap - 100.0:
                                break
                            if av_ready() and len(av_pend) >= 2:
                                av_emit()
                            elif pick_filler():
                                pass
                            else:
                                break
                            busy_t = dve_t if use_dve else act_t
                        # exp-pool pressure: drain AVs (or the V units
                        # blocking them) before allocating another tile
                        while 4 * len(av_pend) + 6 > 33:
                            if av_ready():
                                av_emit()
                            elif v_q:
                                emit_v()
                            elif not pick_filler():
                                break
                        pe_t = max(pe_t, cap) + C_SC
                        ex = sc_group(qh, co, g, hh, use_dve)
                        flush_tp()
                        if use_dve:
                            schr_n += 1
                            st = max(dve_t, pe_t + 100.0)
                            dve_t = st + C_EXP_DVE
                            exp_fin.append(dve_t)
                        else:
                            st = max(act_t, pe_t + 100.0)
                            act_t = st + C_EXP_ACT
                            exp_fin.append(act_t)
                        exps[(hh, g)] = ex
                av_pend.append((qh, co, exps))
        while av_pend:
            if not av_ready():
                emit_v()
                continue
            av_emit()
        flush_tp()
        while qk_q:
            emit_qk(qk_q[0][1])
        while v_q:
            emit_v()
        while pj_q:
            emit_pj()


def make_in_maps(x, c, kv_w, kv_b, shared_q_w, shared_q_b, cohort_q_w, cohort_q_b,
                 proj_w, proj_b):
    f32 = np.float32
    fp16 = np.float16
    fp8 = mybir.dt.np(FP8)
    x = np.asarray(x, dtype=f32)
    c = np.asarray(c).astype(np.int64)
    kv_w = np.asarray(kv_w, dtype=f32)
    kv_b = np.asarray(kv_b, dtype=f32)
    shared_q_w = np.asarray(shared_q_w, dtype=f32)
    shared_q_b = np.asarray(shared_q_b, dtype=f32)
    cohort_q_w = np.asarray(cohort_q_w, dtype=f32)
    cohort_q_b = np.asarray(cohort_q_b, dtype=f32)
    proj_w = np.asarray(proj_w, dtype=f32)
    proj_b = np.asarray(proj_b, dtype=f32)

    wk = kv_w[:D] * WS
    wv_ = kv_w[D:]
    bk = kv_b[:D] * WS
    bv_ = kv_b[D:]

    w32 = wv_.T * WS                      # [d, e]
    w8 = w32.astype(fp8)
    dw8 = (w32 - w8.astype(f32)).astype(fp8)
    wv8_h = np.ascontiguousarray(
        w8.reshape(4, 2, P, D).transpose(2, 0, 1, 3)
    )
    dwv8_h = np.ascontiguousarray(
        dw8.reshape(4, 2, P, D).transpose(2, 0, 1, 3)
    )
    wp_h = np.ascontiguousarray(
        proj_w.T.reshape(8, P, D).transpose(1, 0, 2)
    ).astype(fp16)

    in_maps = []
    for b in range(x.shape[0]):
        wq = np.concatenate([shared_q_w, cohort_q_w[c[b]]], axis=0) * WS
        bq = np.concatenate([shared_q_b, cohort_q_b[c[b]]], axis=0) * WS
        wqk_cols = np.concatenate([wq, wk], axis=0)     # [2048 e, 1024 d]
        # e = qk*1024 + head*64 + j*32 + i with head = 4*g4 + hh;
        # device wants [p, g(qk,g4), j, t2, dj, ec(hh,i)]
        wqk_e = wqk_cols.reshape(2, 4, 4, 2, 32, D)   # [qk, g4, hh, j, i, d]
        wqk_e = wqk_e.transpose(0, 1, 3, 2, 4, 5).reshape(8, 2, P, D)
        wqk_full = wqk_e.reshape(8, 2, P, 4, 2, P)    # [g, j, ec, t2, dj, p]
        wqk_h = np.ascontiguousarray(
            wqk_full.transpose(5, 0, 1, 3, 4, 2)
        ).astype(fp8)
        bqk_e = np.concatenate([bq, bk]).reshape(2, 4, 4, 2, 32)
        bqk_h = np.ascontiguousarray(
            bqk_e.transpose(0, 1, 3, 2, 4).reshape(8, 2, P).transpose(2, 0, 1)
        ).astype(f32)
        bqk8_h = np.ascontiguousarray(
            bqk_e.transpose(0, 1, 3, 2, 4).reshape(8, 2, P)[None]
        ).astype(fp8)
        xT = x[b].T
        x8 = xT.astype(fp8)
        dx8 = (xT - x8.astype(f32)).astype(fp8)
        xdr_h = np.ascontiguousarray(
            x8.reshape(4, 2, P, N).transpose(2, 0, 1, 3)
        )
        xdr2_h = np.ascontiguousarray(
            dx8.reshape(4, 2, P, N).transpose(2, 0, 1, 3)
        )
        m = {
            "ident": np.eye(P, dtype=fp16),
            "bqk8": bqk8_h,
            "xdr": xdr_h,
            "xdr2": xdr2_h,
            "wqk": wqk_h,
            "bqk": bqk_h,
            "wv8": wv8_h,
            "dwv8": dwv8_h,
            "bv": np.ascontiguousarray(bv_).astype(mybir.dt.np(BF16)),
            "wp": wp_h,
            "bp": np.ascontiguousarray(proj_b).astype(mybir.dt.np(BF16)),
        }
        in_maps.append(m)
    return in_maps


_NC_CACHE = {}


def kernel(**inputs) -> np.ndarray:
    in_maps = make_in_maps(**inputs)
    if "nc" not in _NC_CACHE:
        _NC_CACHE["nc"] = build_nc()
    nc = _NC_CACHE["nc"]
    res = run_bass_kernel_spmd(nc, in_maps, core_ids=list(range(NCORES)))
    out = np.stack([res.results[i]["out"] for i in range(NCORES)], axis=0)
    return out.astype(np.float32)


# revision 82
# speedup vs baseline: 1.0720x; 1.0019x over previous
"""CohortAwareBlock Trainium2 kernel.

Data-parallel over batch B=8 across 8 NeuronCores (one sample per core).
Cohort routing (gather of cohort_q_w by per-sample cohort id) happens on the
host while building each core's weight tensors; the device kernel is a plain
attention block.

Numerics:
  - QK-gen runs as fp8-e4m3 DoubleRow matmuls (weights pre-scaled x32 to
    dodge fp8 subnormals; the inverse scale is folded into the exp scale).
  - q/k are stored as fp8 in a DoubleRow-interleaved layout ([32, 2, N] per
    head, 4 heads stacked across 128 partitions at 32-partition tile
    positions) so the scores matmul also runs fp8-DR: 2x fewer PE cycles
    than fp16 scores.
  - exp splits across the ACT engine (exact table exp, fp16 out) and the
    DVE (Schraudolph bit-trick: int16(A*s + B) written through a bitcast
    view and read back as fp16; ~1.8% rms sawtooth error, SCHR_MAX-capped
    for the error budget) so the exp stream is not ACT-bound.
  - V-gen runs as fp8-DR in three equal-scale residual passes
    (32v = x8*(32w)8 + dx8*(32w)8 + x8*(32dw)8, one psum group; the
    residual operands ride e4m3 subnormals) for fp16-grade accuracy at
    fp8 cost; attn weights / projection stay fp16.

Per-core structure:
  q4k4 [128, 8, 2, N] fp8  (4 q-head groups + 4 k-head groups, DR layout)
  v_aug [keys, h, 65] fp16 (col 64 = 1.0 so the flipped AV emits the
                            softmax denominator per q-partition)
  per (q-quarter, head pair):
    scores -> 2-bank PSUM [128, 4, 256] via fp8-DR -> exp (ACT or DVE,
    routed by backlog) ->
    flipped attn@v: av psum [128, 2, 2, 65]; col 64 = den ->
    batched DVE reciprocal + broadcast mult -> nm fp16 ->
    PE-array transpose (vs identity) + DVE evac -> nmT [d, q] ->
    proj (fp16) + bias -> out DMA on the idle GPSIMD queue

Scheduling: a priority scheduler with virtual engine clocks emits score
groups as fast as the sc_ps double-buffer allows (the exp stream is the
critical path), drains attn@v pairs as the preferred PE slack-filler, and
paces QK/V/proj units from per-kind queues (quota per iteration, popped
on demand for data dependencies) so no slow unit head-of-line-blocks the
in-order PE queue. The first QK units evac via ACT-Copy (bias folded in as
a ones-row matmul) while ACT is otherwise idle during startup; dummy
warmup matmuls keep the PE p-state ramped until the first real work; the
iteration visits head pairs in a block order that staggers the V-gen
deadlines.
"""

import numpy as np

import concourse.bass as bass
import concourse.bacc as bacc
import concourse.mybir as mybir
import concourse.tile as tile
from concourse.bass_utils import run_bass_kernel_spmd

P = 128
N = 1024            # sequence length
D = 1024            # model dim
H = 16              # heads
HD = 64             # head dim
NQ = 4              # q-quarters (256 q each)
QW = N // NQ        # 256
SCALE = HD ** -0.5
NCORES = 8

WS = 32.0           # fp8 pre-scale on w_q/w_k (and so on q/k values)
EXP_SCALE = SCALE / (WS * WS)

# Schraudolph fp16-bitcast exp on DVE: y_bits = int16(s * A + B); bits read
# as fp16 give exp(s*EXP_SCALE) with ~1.8% rms sawtooth error.
LOG2E = 1.4426950408889634
SCHR_A = EXP_SCALE * LOG2E * 1024.0
SCHR_B = 15301.0
SCHR_MAX = 27        # max exp groups routed to DVE (of 128); error budget cap

F32 = mybir.dt.float32
FP16 = mybir.dt.float16
BF16 = mybir.dt.bfloat16
FP8 = mybir.dt.float8e4
I16 = mybir.dt.int16
DR = mybir.MatmulPerfMode.DoubleRow
EXP = mybir.ActivationFunctionType.Exp
MUL = mybir.AluOpType.mult
ADD = mybir.AluOpType.add


def build_nc():
    nc = bacc.Bacc(
        "TRN2",
        target_bir_lowering=False,
        debug=False,
        num_devices=NCORES,
    )

    # ---- external I/O (per-core shards, host-prepped layouts) ----
    # DoubleRow-interleaved d-dim: d = (t2*2 + dj)*128 + p
    xdr = nc.dram_tensor("xdr", [P, 4, 2, N], FP8, kind="ExternalInput")
    # wqk[p, g, j, t2, dj, ec]: g = 4-head group (0..3 q, 4..7 k); j = d-half
    # of the head (e_local = j*32 + i); ec = hh*32 + i -> head 4*(g%4)+hh.
    wqk = nc.dram_tensor("wqk", [P, 8, 2, 4, 2, P], FP8, kind="ExternalInput")
    bqk = nc.dram_tensor("bqk", [P, 8, 2], F32, kind="ExternalInput")
    xdr2 = nc.dram_tensor("xdr2", [P, 4, 2, N], FP8, kind="ExternalInput")
    wv8 = nc.dram_tensor("wv8", [P, 4, 2, D], FP8, kind="ExternalInput")
    dwv8 = nc.dram_tensor("dwv8", [P, 4, 2, D], FP8, kind="ExternalInput")
    bv = nc.dram_tensor("bv", [D], BF16, kind="ExternalInput")
    wp = nc.dram_tensor("wp", [P, 8, D], FP16, kind="ExternalInput")
    bp = nc.dram_tensor("bp", [D], BF16, kind="ExternalInput")
    ident = nc.dram_tensor("ident", [P, P], FP16, kind="ExternalInput")
    bqk8 = nc.dram_tensor("bqk8", [1, 8, 2, P], FP8, kind="ExternalInput")
    out = nc.dram_tensor("out", [N, D], F32, kind="ExternalOutput")

    with tile.TileContext(nc) as tc:
        kernel_body(tc, xdr, wqk, bqk, xdr2, wv8, dwv8, bv, wp, bp, ident, bqk8, out)
    nc.compile()
    return nc


def kernel_body(tc, xdr, wqk, bqk, xdr2, wv8, dwv8, bv, wp, bp, ident, bqk8, out):
    nc = tc.nc
    from contextlib import ExitStack

    with ExitStack() as ctx:
        ctx.enter_context(
            nc.allow_low_precision(reason="fp16/fp8 matmul inputs by design")
        )
        res = ctx.enter_context(tc.tile_pool(name="res", bufs=1))
        shared = ctx.enter_context(tc.tile_pool(name="shared", bufs=1))
        gen_ps = ctx.enter_context(tc.tile_pool(name="gen_ps", bufs=2, space="PSUM"))
        av_ps = ctx.enter_context(tc.tile_pool(name="av_ps", bufs=2, space="PSUM"))
        sc_ps = ctx.enter_context(tc.tile_pool(name="sc_ps", bufs=2, space="PSUM"))
        exp_pool = ctx.enter_context(tc.tile_pool(name="exp_pool", bufs=35))
        rc_pool = ctx.enter_context(tc.tile_pool(name="rc_pool", bufs=4))
        nm_pool = ctx.enter_context(tc.tile_pool(name="nm_pool", bufs=4))
        oev_pool = ctx.enter_context(tc.tile_pool(name="oev_pool", bufs=3))

        # ---- resident tiles ----
        warm = res.tile([1, 513], FP16)
        nc.gpsimd.memset(warm[:], 1.0)

        xdr_sb = res.tile([P, 4, 2, N], FP8)
        wqk_sb = shared.tile([P, 8, 2, 4, 2, P], FP8, name="wqk_sb")
        bqk_sb = res.tile([P, 8, 2], F32)
        # q/k in scores-DR layout: group g (0..3 q, 4..7 k), partition
        # (hh*32+i), j, token -> value of head 4*(g%4)+hh, d = j*32+i
        q4k4 = res.tile([P, 8, 2, N], FP8)
        xdr2_sb = res.tile([P, 4, 2, N], FP8)
        wv8_sb = res.tile([P, 4, 2, D], FP8)
        dwv8_sb = res.tile([P, 4, 2, D], FP8)
        bv_rep = res.tile([P, D], BF16)
        wp_holder = []   # allocated from `shared` after QK-gen is emitted
        bp_rep = res.tile([P, D], BF16)

        # v_aug[p, nt, h, :]: cols 0:64 = v for head h at key chunk nt,
        # col 64 = 1.0 (flipped attn@v then emits the softmax denominator
        # in output column 64, one value per q-partition)
        v_aug = res.tile([P, 8, H, HD + 1], FP16)
        nc.gpsimd.memset(v_aug[:, :, :, HD : HD + 1], 1.0)

        # transposed normalized att, packed for proj: [d-part, qc, co, q]
        nmT = res.tile([P, 8, 8, P], FP16)
        ident_sb = res.tile([P, P], FP16)
        bqk8_sb = res.tile([1, 8, 2, P], FP8)
        ones8 = res.tile([1, 512], FP8)
        nc.gpsimd.memset(ones8[:], 1.0)

        # ---- input DMAs (sync queue, need-order; wp follows in the
        # filler queue, reusing wqk's SBUF once QK-gen is done) ----
        nc.sync.dma_start(xdr_sb[:], xdr[:])
        for g in (0, 4):
            for j in range(2):
                nc.sync.dma_start(wqk_sb[:, g, j], wqk[:, g, j])
        nc.sync.dma_start(bqk8_sb[:], bqk8[:])
        nc.sync.dma_start(bqk_sb[:], bqk[:])
        for g in (1, 5):
            for j in range(2):
                nc.sync.dma_start(wqk_sb[:, g, j], wqk[:, g, j])
        for t2 in range(4):
            nc.sync.dma_start(wv8_sb[:, t2], wv8[:, t2])
        nc.sync.dma_start(bv_rep[:], bv[None, :].to_broadcast([P, D]))
        for t2 in range(4):
            nc.sync.dma_start(xdr2_sb[:, t2], xdr2[:, t2])
        for t2 in range(4):
            nc.sync.dma_start(dwv8_sb[:, t2], dwv8[:, t2])
        for g in (2, 6, 3, 7):
            for j in range(2):
                nc.sync.dma_start(wqk_sb[:, g, j], wqk[:, g, j])
        nc.sync.dma_start(bp_rep[:], bp[None, :].to_broadcast([P, D]))
        nc.sync.dma_start(ident_sb[:], ident[:])

        # ---------------- emission helpers ----------------
        def warmup():
            # keep the PE p-state ramped while input DMAs land
            ps = gen_ps.tile([P, 512], F32, tag="gps", name="gps")
            nc.tensor.matmul(
                ps[0:1, :],
                lhsT=warm[:, 512:513],
                rhs=warm[:, 0:512],
                start=True,
                stop=True,
            )

        def qk_unit(g, j, ch, act_evac=False):
            # one QK-gen psum group: 4 fp8-DR matmuls + biased fp8 evac into
            # the scores-DR layout (GPSIMD cannot read PSUM, so evac on DVE;
            # the first units evac via ACT-Copy instead -- ACT idles during
            # startup -- with the bias folded in as a ones-row matmul)
            ps = gen_ps.tile([P, 512], F32, tag="gps", name="gps")
            for t2 in range(4):
                nc.tensor.matmul(
                    ps[:],
                    lhsT=wqk_sb[:, g, j, t2],
                    rhs=xdr_sb[:, t2, :, ch * 512 : (ch + 1) * 512],
                    start=(t2 == 0),
                    stop=(t2 == 3) and not act_evac,
                    perf_mode=DR,
                )
            if act_evac:
                nc.tensor.matmul(
                    ps[:],
                    lhsT=bqk8_sb[:, g, j],
                    rhs=ones8[:],
                    start=False,
                    stop=True,
                )
                nc.scalar.activation(
                    q4k4[:, g, j, ch * 512 : (ch + 1) * 512],
                    ps[:],
                    mybir.ActivationFunctionType.Copy,
                )
            else:
                nc.vector.tensor_scalar_add(
                    q4k4[:, g, j, ch * 512 : (ch + 1) * 512],
                    ps[:],
                    bqk_sb[:, g, j : j + 1],
                )

        def v_unit(eq, nt):
            # v[keys nt-chunk, 256 cols (4 heads) of quarter eq]: fp8-DR in
            # three equal-scale passes sharing one psum group --
            # 32*v = x8*(32w)8 + dx8*(32w)8 + x8*(32dw)8 -- then a scaled
            # evac; quarter granularity staggers the AV deadlines
            ps = gen_ps.tile([P, 256], F32, tag="gps", name="gps")
            es = eq * 256
            passes = [(xdr_sb, wv8_sb), (xdr2_sb, wv8_sb), (xdr_sb, dwv8_sb)]
            for pi, (xs, ws) in enumerate(passes):
                for t2 in range(4):
                    nc.tensor.matmul(
                        ps[:],
                        lhsT=xs[:, t2, :, nt * P : (nt + 1) * P],
                        rhs=ws[:, t2, :, es : es + 256],
                        start=(pi == 0 and t2 == 0),
                        stop=(pi == 2 and t2 == 3),
                        perf_mode=DR,
                    )
            nc.vector.scalar_tensor_tensor(
                v_aug[:, nt, eq * 4 : (eq + 1) * 4, 0:HD],
                ps[:].rearrange("p (h d) -> p h d", d=HD),
                1.0 / WS,
                bv_rep[:, es : es + 256].rearrange("p (h d) -> p h d", d=HD),
                op0=MUL,
                op1=ADD,
            )

        def sc_group(qh, co, g, hh, use_dve):
            # one kt-group of scores (fp8-DR) + its batched exp (ACT exact
            # or DVE Schraudolph); returns the exp tile
            h = 2 * co + hh
            grp = h // 4
            r = 32 * (h % 4)
            q0 = qh * QW
            ps = sc_ps.tile([P, 4, QW], F32, tag="scps", name="scps")
            for ki in range(4):
                kt = g * 4 + ki
                nc.tensor.matmul(
                    ps[:, ki],
                    lhsT=q4k4[r : r + 32, 4 + grp, :, kt * P : (kt + 1) * P],
                    rhs=q4k4[r : r + 32, grp, :, q0 : q0 + QW],
                    start=True,
                    stop=True,
                    perf_mode=DR,
                    tile_position=(r, 0),
                )
            ex = exp_pool.tile([P, 4, QW], FP16, tag="exp", name="exp")
            if use_dve:
                nc.vector.tensor_scalar(
                    ex[:].bitcast(I16),
                    ps[:],
                    SCHR_A,
                    SCHR_B,
                    op0=MUL,
                    op1=ADD,
                )
            else:
                nc.scalar.activation(ex[:], ps[:], EXP, scale=EXP_SCALE)
            return ex

        def av_halves(qh, co, exps):
            # flipped attn@v for one head pair, split per head; the batched
            # norm runs after the second half; the nm -> nmT transpose is a
            # separate unit (PE-array transpose + DVE evac) emitted later
            hold = []

            def half(hh):
                h = 2 * co + hh
                if hh == 0:
                    t = av_ps.tile([P, 392], F32, tag="avps", name="avps")
                    hold.append(t)
                ps = hold[0][:, 0:260].rearrange(
                    "p (a b c) -> p a b c", a=2, b=2
                )
                for qs in range(2):
                    for kt in range(8):
                        nc.tensor.matmul(
                            ps[:, qs, hh],
                            lhsT=exps[(hh, kt // 4)][:, kt % 4,
                                                     qs * P : (qs + 1) * P],
                            rhs=v_aug[:, kt, h, :],
                            start=(kt == 0),
                            stop=(kt == 7),
                        )
                if hh == 1:
                    rc = rc_pool.tile([P, 2, 2, 1], F32, tag="rc", name="rc")
                    nc.vector.reciprocal(rc[:], ps[:, :, :, HD : HD + 1])
                    nm = nm_pool.tile([P, 2, 2, HD], FP16, tag="nm", name="nm")
                    nc.vector.tensor_tensor(
                        nm[:],
                        ps[:, :, :, 0:HD],
                        rc[:].broadcast_to([P, 2, 2, HD]),
                        op=MUL,
                    )
                    hold.append(nm)

            def tp():
                # PE-array transpose of nm into proj layout + DVE evac;
                # keeps the nmT chain off the slow DMA queues
                t, nm = hold
                tpv = t[:, 264:392].bitcast(FP16).rearrange(
                    "p (a q) -> p a q", a=2
                )
                for qs in range(2):
                    nc.tensor.transpose(
                        tpv[:, qs], nm[:, qs], ident_sb[:]
                    )
                nc.vector.tensor_copy(
                    nmT[:, qh * 2 : qh * 2 + 2, co, :], tpv[:]
                )

            return (lambda: half(0)), (lambda: half(1)), tp

        def pj_halves(qh, nt, fh):
            # one projection output group split into two PE units
            qc = qh * 2 + nt
            n0 = qc * P
            hold = []

            def a():
                ps = gen_ps.tile([P, 512], F32, tag="gps", name="gps")
                hold.append(ps)
                for co in range(4):
                    nc.tensor.matmul(
                        ps[:],
                        lhsT=nmT[:, qc, co, :],
                        rhs=wp_holder[0][:, co, fh * 512 : (fh + 1) * 512],
                        start=(co == 0),
                        stop=False,
                    )

            def b():
                ps = hold[0]
                for co in range(4, 8):
                    nc.tensor.matmul(
                        ps[:],
                        lhsT=nmT[:, qc, co, :],
                        rhs=wp_holder[0][:, co, fh * 512 : (fh + 1) * 512],
                        start=False,
                        stop=(co == 7),
                    )
                ev = oev_pool.tile([P, 512], F32, tag="oev", name="oev")
                nc.vector.tensor_add(
                    ev[:], ps[:], bp_rep[:, fh * 512 : (fh + 1) * 512]
                )
                nc.sync.dma_start(
                    out[n0 : n0 + P, fh * 512 : (fh + 1) * 512], ev[:]
                )

            return a, b

        # ---------------- schedule ----------------
        # Priority scheduler with virtual engine clocks (pe/act/dve busy-until
        # estimates under the cost model). The exp stream (ACT + DVE
        # Schraudolph, routed by backlog) is the critical path; score groups
        # are emitted as fast as the sc_ps double-buffer allows. AV pairs and
        # QK/V/proj units fill PE slack one unit at a time from per-kind
        # queues, so a unit needed soon never forces a burst-drain of
        # unrelated work (which would starve the exp engines).
        from collections import deque

        C_SC = 220.0          # score group PE (4 fp8-DR matmuls)
        C_EXP_ACT = 1110.0
        C_EXP_DVE = 1280.0
        C_AVH = 440.0         # AV half PE
        C_NORM = 800.0        # batched recip+mult DVE
        C_QK = 430.0          # QK unit PE
        C_QK_EV = 750.0       # QK evac DVE
        C_VQ = 645.0          # V quarter-unit PE (12 fp8-DR matmuls)
        C_VQ_EV = 485.0       # V quarter evac DVE
        C_PJ = 1704.0         # proj unit PE (a+b)
        C_PJ_EV = 705.0       # proj evac DVE
        C_TP = 115.0          # nm transpose PE (2 PE-array transposes)
        C_TP_EV = 320.0       # nmT evac DVE
        TARGET_BL = 3600.0

        for _ in range(14):
            warmup()

        # QK units: first 4 unblock (qh0, co0, g0) scores; the rest are
        # popped on demand (per-unit) or as slack fillers
        qk_first = [(0, 0, 0), (0, 1, 0), (4, 0, 0), (4, 1, 0)]
        qk_q = deque()
        for (g, j, ch, est) in [(4, 0, 1, 5200.0), (4, 1, 1, 5600.0),
                                (0, 0, 1, 6000.0), (0, 1, 1, 6400.0)]:
            qk_q.append((est, (g, j, ch)))
        for grp, est in ((1, 7800.0), (2, 17300.0), (3, 19500.0)):
            for (g, ch) in ((grp, 0), (4 + grp, 0), (4 + grp, 1), (grp, 1)):
                for j in range(2):
                    qk_q.append((est, (g, j, ch)))
        v_q = deque()
        for eq, v_est in ((0, 16000.0), (1, 17000.0), (2, 18200.0),
                          (3, 20000.0)):
            for nt in range(8):
                v_q.append((v_est, (eq, nt)))
        pj_q = deque()

        pe_t = 6300.0
        act_t = 0.0
        dve_t = 0.0
        qk_done = set(qk_first)
        for i, u in enumerate(qk_first):
            # evacs split across ACT and DVE so they drain in parallel
            # during startup (both engines are otherwise idle)
            if i < 2:
                qk_unit(*u, act_evac=True)
                pe_t += C_QK + 213.0
                act_t = max(act_t, pe_t + 100.0) + 700.0
            else:
                qk_unit(*u)
                pe_t += C_QK
                dve_t = max(dve_t, pe_t + 100.0) + C_QK_EV

        av_pend = deque()
        exp_fin = []
        schr_n = 0
        v_pops = [0, 0, 0, 0]
        counts = [0] * NQ
        wp_loaded = [False]

        def wall():
            return max(pe_t, act_t - 2.0 * C_EXP_ACT, dve_t - 2.0 * C_EXP_DVE)

        def emit_qk(u):
            nonlocal pe_t, dve_t
            est = None
            for (e, uu) in qk_q:
                if uu == u:
                    est = e
                    break
            qk_q.remove((est, u))
            qk_unit(*u)
            qk_done.add(u)
            pe_t = max(pe_t, est) + C_QK
            dve_t = max(dve_t, pe_t + 100.0) + C_QK_EV
            if not qk_q and not wp_loaded[0]:
                wp_loaded[0] = True
                wp_load()

        def wp_load():
            wp_holder.append(shared.tile([P, 8, D], FP16, name="wp_sb"))
            for co in range(8):
                nc.sync.dma_start(wp_holder[0][:, co], wp[:, co])

        def emit_v():
            nonlocal pe_t, dve_t
            est, (eq, nt) = v_q.popleft()
            v_unit(eq, nt)
            v_pops[eq] += 1
            pe_t = max(pe_t, est) + C_VQ
            dve_t = max(dve_t, pe_t + 100.0) + C_VQ_EV

        def emit_pj():
            nonlocal pe_t, dve_t
            est, (qh0, nt, fh) = pj_q.popleft()
            a, b = pj_halves(qh0, nt, fh)
            a()
            b()
            pe_t = max(pe_t, est) + C_PJ
            dve_t = max(dve_t, pe_t + 100.0) + C_PJ_EV

        def pick_filler():
            # one slack unit, earliest-est first; False if nothing eligible
            cands = []
            if qk_q:
                cands.append((qk_q[0][0], 0))
            if v_q:
                cands.append((v_q[0][0], 1))
            if pj_q:
                cands.append((pj_q[0][0], 2))
            cands = [c for c in cands if c[0] <= wall() + 400.0]
            if not cands:
                return False
            cands.sort()
            kind = cands[0][1]
            if kind == 0:
                emit_qk(qk_q[0][1])
            elif kind == 1:
                emit_v()
            else:
                emit_pj()
            return True

        pending_tp = []

        def flush_tp():
            nonlocal pe_t, dve_t
            while pending_tp:
                qh0, co0, tp = pending_tp.pop(0)
                tp()
                pe_t += C_TP
                dve_t = max(dve_t, pe_t + 100.0) + C_TP_EV
                counts[qh0] += 1
                if counts[qh0] == 8:
                    est_pj = max(wall(), dve_t) + 4200.0
                    for nt in range(2):
                        for fh in range(2):
                            pj_q.append((est_pj, (qh0, nt, fh)))

        def av_ready():
            if not av_pend:
                return False
            qh0, co0, _ = av_pend[0]
            return v_pops[co0 // 2] >= 8

        def av_emit():
            nonlocal pe_t, dve_t
            flush_tp()
            qh0, co0, exps0 = av_pend.popleft()
            a, b, tp = av_halves(qh0, co0, exps0)
            a()
            pe_t += C_AVH
            b()
            pe_t += C_AVH
            dve_t = max(dve_t, pe_t + 100.0) + C_NORM
            pending_tp.append((qh0, co0, tp))

        FILL_TOTAL = 24 * C_QK + 32 * C_VQ + 32 * C_PJ / 2.0
        fill_pe = [0.0]

        def quota_fill(limit):
            n = 0
            while (
                n < limit
                and fill_pe[0] < (it + 1) * (FILL_TOTAL / 32.0)
            ):
                before = (len(qk_q), len(v_q), len(pj_q))
                if not pick_filler():
                    break
                after = (len(qk_q), len(v_q), len(pj_q))
                if before[0] != after[0]:
                    fill_pe[0] += C_QK
                elif before[1] != after[1]:
                    fill_pe[0] += C_VQ
                else:
                    fill_pe[0] += C_PJ
                n += 1

        BLOCKS = [(0, 0), (1, 0), (0, 4), (2, 0), (1, 4), (3, 0), (2, 4), (3, 4)]
        it = -1
        for (qh, co0_blk) in BLOCKS:
            for co in range(co0_blk, co0_blk + 4):
                it += 1
                keep = 3 if it < 29 else 1
                if av_ready() and len(av_pend) > keep:
                    av_emit()
                quota_fill(4)
                exps = {}
                for g in range(2):
                    for hh in range(2):
                        grp = co // 2
                        for u in [(grp, 0, qh // 2), (grp, 1, qh // 2),
                                  (4 + grp, 0, g), (4 + grp, 1, g)]:
                            if u not in qk_done:
                                emit_qk(u)
                        use_dve = (
                            schr_n < SCHR_MAX
                            and it >= 4
                            and act_t - dve_t > 1200.0
                        )
                        busy_t = dve_t if use_dve else act_t
                        ni = len(exp_fin)
                        cap = exp_fin[ni - 2] if ni >= 2 else 0.0
                        # fill PE while the psum cap blocks or the exp
                        # engine is well-fed
                        while True:
                            gate = max(pe_t, cap)
                            if busy_t - gate < TARGET_BL and pe_t >= cap - 100.0:
                                break
                            if av_ready() and len(av_pend) >= 2:
                                av_emit()
                            elif pick_filler():
                                pass
                            else:
                                break
                            busy_t = dve_t if use_dve else act_t
                        # exp-pool pressure: drain AVs (or the V units
                        # blocking them) before allocating another tile
                        while 4 * len(av_pend) + 6 > 33:
                            if av_ready():
                                av_emit()
                            elif v_q:
                                emit_v()
                            elif not pick_filler():
                                break
                        pe_t = max(pe_t, cap) + C_SC
                        ex = sc_group(qh, co, g, hh, use_dve)
                        flush_tp()
                        if use_dve:
                            schr_n += 1
                            st = max(dve_t, pe_t + 100.0)
                            dve_t = st + C_EXP_DVE
                            exp_fin.append(dve_t)
                        else:
                            st = max(act_t, pe_t + 100.0)
                            act_t = st + C_EXP_ACT
                            exp_fin.append(act_t)
                        exps[(hh, g)] = ex
                av_pend.append((qh, co, exps))
        while av_pend:
            if not av_ready():
                emit_v()
                continue
            av_emit()
        flush_tp()
        while qk_q:
            emit_qk(qk_q[0][1])
        while v_q:
            emit_v()
        while pj_q:
            emit_pj()


def make_in_maps(x, c, kv_w, kv_b, shared_q_w, shared_q_b, cohort_q_w, cohort_q_b,
                 proj_w, proj_b):
    f32 = np.float32
    fp16 = np.float16
    fp8 = mybir.dt.np(FP8)
    x = np.asarray(x, dtype=f32)
    c = np.asarray(c).astype(np.int64)
    kv_w = np.asarray(kv_w, dtype=f32)
    kv_b = np.asarray(kv_b, dtype=f32)
    shared_q_w = np.asarray(shared_q_w, dtype=f32)
    shared_q_b = np.asarray(shared_q_b, dtype=f32)
    cohort_q_w = np.asarray(cohort_q_w, dtype=f32)
    cohort_q_b = np.asarray(cohort_q_b, dtype=f32)
    proj_w = np.asarray(proj_w, dtype=f32)
    proj_b = np.asarray(proj_b, dtype=f32)

    wk = kv_w[:D] * WS
    wv_ = kv_w[D:]
    bk = kv_b[:D] * WS
    bv_ = kv_b[D:]

    w32 = wv_.T * WS                      # [d, e]
    w8 = w32.astype(fp8)
    dw8 = (w32 - w8.astype(f32)).astype(fp8)
    wv8_h = np.ascontiguousarray(
        w8.reshape(4, 2, P, D).transpose(2, 0, 1, 3)
    )
    dwv8_h = np.ascontiguousarray(
        dw8.reshape(4, 2, P, D).transpose(2, 0, 1, 3)
    )
    wp_h = np.ascontiguousarray(
        proj_w.T.reshape(8, P, D).transpose(1, 0, 2)
    ).astype(fp16)

    in_maps = []
    for b in range(x.shape[0]):
        wq = np.concatenate([shared_q_w, cohort_q_w[c[b]]], axis=0) * WS
        bq = np.concatenate([shared_q_b, cohort_q_b[c[b]]], axis=0) * WS
        wqk_cols = np.concatenate([wq, wk], axis=0)     # [2048 e, 1024 d]
        # e = qk*1024 + head*64 + j*32 + i with head = 4*g4 + hh;
        # device wants [p, g(qk,g4), j, t2, dj, ec(hh,i)]
        wqk_e = wqk_cols.reshape(2, 4, 4, 2, 32, D)   # [qk, g4, hh, j, i, d]
        wqk_e = wqk_e.transpose(0, 1, 3, 2, 4, 5).reshape(8, 2, P, D)
        wqk_full = wqk_e.reshape(8, 2, P, 4, 2, P)    # [g, j, ec, t2, dj, p]
        wqk_h = np.ascontiguousarray(
            wqk_full.transpose(5, 0, 1, 3, 4, 2)
        ).astype(fp8)
        bqk_e = np.concatenate([bq, bk]).reshape(2, 4, 4, 2, 32)
        bqk_h = np.ascontiguousarray(
            bqk_e.transpose(0, 1, 3, 2, 4).reshape(8, 2, P).transpose(2, 0, 1)
        ).astype(f32)
        bqk8_h = np.ascontiguousarray(
            bqk_e.transpose(0, 1, 3, 2, 4).reshape(8, 2, P)[None]
        ).astype(fp8)
        xT = x[b].T
        x8 = xT.astype(fp8)
        dx8 = (xT - x8.astype(f32)).astype(fp8)
        xdr_h = np.ascontiguousarray(
            x8.reshape(4, 2, P, N).transpose(2, 0, 1, 3)
        )
        xdr2_h = np.ascontiguousarray(
            dx8.reshape(4, 2, P, N).transpose(2, 0, 1, 3)
        )
        m = {
            "ident": np.eye(P, dtype=fp16),
            "bqk8": bqk8_h,
            "xdr": xdr_h,
            "xdr2": xdr2_h,
            "wqk": wqk_h,
            "bqk": bqk_h,
            "wv8": wv8_h,
            "dwv8": dwv8_h,
            "bv": np.ascontiguousarray(bv_).astype(mybir.dt.np(BF16)),
            "wp": wp_h,
            "bp": np.ascontiguousarray(proj_b).astype(mybir.dt.np(BF16)),
        }
        in_maps.append(m)
    return in_maps


_NC_CACHE = {}


def kernel(**inputs) -> np.ndarray:
    in_maps = make_in_maps(**inputs)
    if "nc" not in _NC_CACHE:
        _NC_CACHE["nc"] = build_nc()
    nc = _NC_CACHE["nc"]
    res = run_bass_kernel_spmd(nc, in_maps, core_ids=list(range(NCORES)))
    out = np.stack([res.results[i]["out"] for i in range(NCORES)], axis=0)
    return out.astype(np.float32)
